# revision 37
# baseline (speedup 1.0000x reference)
"""Trainium2 Bass kernel for nn_BatchTrainableButterfly.

The reference applies, per mesh-batch b, a trainable butterfly network
(10 levels of phase shifters + 2x2 directional couplers with butterfly
permutations, plus a final phase layer and bit-reversals) to every token
row x[n, :].  For fixed phases the whole network is a linear map on
C^1024; it factors into two 128-wide PE stages (A = bitrev + levels 0..6,
block-diagonal over 8 column groups; B = levels 7..9 + final phase +
bitrev, an 8x8 mix across groups), which is 4x fewer MACs than the dense
1024x1024 matmul.

Active VERSION=5 (token-sharded): each core owns N_TOKENS/8 = 512 tokens
for ALL 4 mesh batches, which minimizes SBUF-AXI-port traffic — the
measured bottleneck (x 2 MiB + stationaries 4 MiB + inter-stage shuffle
8 MiB (counted twice: SBUF->SBUF reads AND writes cross the same 16 AXI
ports at ~435 GB/s) + out 8 MiB ~= 30 port-MiB ~= 72 us floor).  All
transposes live on the host: x arrives pre-transposed/r-grouped bf16;
out leaves position-major bf16 and the host inverts the permutation.
Per batch: 8 A-groups (4 bf16 matmuls N=512 each, fp32 PSUM pairs,
vector/scalar evacuation), a stepped-partition SBUF shuffle split into
per-(g, t2-half) DMAs (finer stage-B dependencies; stage-A's output row
order u*64+s*4+t2' makes the halves contiguous 64-partition slices),
then 8 B-groups with paired output stores (4 KiB dram lines).  Software
pipeline A0 A1 B0 A2 B1 A3 B2 B3 balances PE phases against the
port-saturated shuffle/store traffic; stationary negations (-ai, -bi)
are derived on device off the critical path.  ~98 us vs 113-118 us for
the v4 batch-sharded baseline; rel err 3.4e-3 (bf16).

An int8 shuffle variant (INT8_SHUFFLE) passes accuracy (1.1e-2) but is
slower: DVE/ACT 8-bit output casts run at half rate and gpsimd int8
upcasts at ~3 ns/elem, swamping the DMA savings.
"""

import math

import numpy as np

import concourse.tile as tile
from concourse import bacc, bass, mybir
from concourse.bass_utils import run_bass_kernel_spmd
from concourse.masks import make_identity

P = 128          # partitions
L = 1024         # butterfly length
N_TOKENS = 4096
MESH_BATCH = 4
N_CORES = 8
T = (N_TOKENS * MESH_BATCH) // N_CORES  # 2048 token-rows per core
NT = T // P      # 16 token tiles per core
KC = L // P      # 8 contraction chunks
NLEV = int(math.log2(L))  # 10

F32 = mybir.dt.float32
F32R = mybir.dt.float32r
BF16 = mybir.dt.bfloat16

TC = 512          # tokens per pipeline chunk (v3)
NCH = T // TC     # 4 chunks

I8 = mybir.dt.int8

# v5: token sharding — each core owns TOK5 tokens for ALL 4 mesh batches.
TOK5 = N_TOKENS // N_CORES   # 512 tokens per core
YSCALE = 127.0 / (4.75 * 11.3137)  # int8 shuffle: 127 / (4.75 sigma_y)

TRACE = False
LAST_RESULTS = None
VERSION = 5       # active: token-sharded two-stage butterfly (see module docstring)
INT8_SHUFFLE = False

# ----------------------------------------------------------------------
# Host side: build the per-batch transfer matrices from the phases.
# ----------------------------------------------------------------------


def _bitrev(n):
    m = int(math.log2(n))
    perm = np.arange(n).reshape(n, 1)
    for _ in range(m):
        n1 = perm.shape[0] // 2
        perm = np.hstack((perm[:n1], perm[n1:]))
    return perm.squeeze(0)


def _forward_indices(length):
    idx = []
    ar = np.arange(length)
    for level in range(int(math.log2(length)) - 1):
        bs = 2 ** (level + 2)
        ind = ar.reshape(-1, length // bs, 2, bs // 2).transpose(0, 1, 3, 2)
        idx.append(ind.reshape(-1))
    return idx


def _build_W(phases):
    """phases (B, NLEV+1, L//2, 2) -> W (B, L, L) complex64 with out = x @ W."""
    B = phases.shape[0]
    br = _bitrev(L)
    fidx = _forward_indices(L)
    dc = np.array([[1.0, 1.0j], [1.0j, 1.0]], dtype=np.complex64)

    x = np.broadcast_to(np.eye(L, dtype=np.complex64), (B, L, L)).copy()
    x = x[..., br]
    for level in range(NLEV):
        x = x.reshape(B, L, L // 2, 2)
        ph = phases[:, level : level + 1, :, :]            # (B, 1, L//2, 2)
        x = x * np.exp(1j * ph.astype(np.complex64))
        x = x @ dc
        x = x.reshape(B, L, L)
        if level < NLEV - 1:
            x = x[..., fidx[level]]
    ph = phases[:, NLEV - 1 : NLEV, :, :].reshape(B, 1, L)
    x = x * np.exp(1j * ph.astype(np.complex64))
    x = x[..., br]
    return (x / np.float32(np.sqrt(L))).astype(np.complex64)


def _rev(v, n):
    r = 0
    for _ in range(n):
        r = (r << 1) | (v & 1)
        v >>= 1
    return r


def _stage_matrices(phases):
    """Two-stage factorization of the butterfly network.

    Stage A = input bitrev + levels 0..6 (perms 0..5, no trailing perm):
    block-diagonal; column-block g is fed by x columns {i : i = 8p + r},
    r = rev3(g).  Stage B = perm fidx[6] + levels 7..9 + final phase +
    final bitrev + scale: per-position 8x8 mixing across the 8 blocks.

    Returns per batch the PE stationaries:
      Astat[b, r] (128,128) cplx : lhsT with K=p (x idx 8p+r), M=pos.
      Bstat[b,t2] (128,128) cplx : lhsT with K = g*16+s (source y(g, t2*16+s)),
                                   M = v*8+m -> out col j = 128m + 8v + rev3(t2).
    Cross-component entries of the extracted B submatrix are exactly 0.
    """
    B_ = phases.shape[0]
    br = _bitrev(L)
    fidx = _forward_indices(L)
    dc = np.array([[1.0, 1.0j], [1.0j, 1.0]], dtype=np.complex64)

    def levels(x, lo, hi, pre_br=False, post_final=False, pre_perm=None):
        if pre_br:
            x = x[..., br]
        if pre_perm is not None:
            x = x[..., pre_perm]
        for level in range(lo, hi):
            x = x.reshape(B_, L, L // 2, 2)
            x = x * np.exp(1j * phases[:, level, None, :, :].astype(np.complex64))
            x = x @ dc
            x = x.reshape(B_, L, L)
            if level < NLEV - 1 and level != 6:
                x = x[..., fidx[level]]
        if post_final:
            x = x * np.exp(
                1j * phases[:, NLEV - 1, None, :, :].reshape(B_, 1, L).astype(np.complex64)
            )
            x = x[..., br]
            x = x / np.float32(np.sqrt(L))
        return x

    eye = np.broadcast_to(np.eye(L, dtype=np.complex64), (B_, L, L)).copy()
    A = levels(eye.copy(), 0, 7, pre_br=True)
    Bm = levels(eye.copy(), 7, NLEV, post_final=True, pre_perm=fidx[6])

    # Stage-A output row order: row' = s*8 + t2 for pos p'' = t2*16 + s, so the
    # inter-stage shuffle is one plain DMA per g: yA_g[:] -> Bin[g:128:8,:,:]
    # (dst partition k = s*8 + g, free = (t2, tok)).
    ar_ = np.arange(P)
    if _USPLIT[0]:
        # row' = u*64 + s*4 + t2' with t2 = u*4 + t2': the shuffle splits
        # into per-u 64-partition DMAs (finer B dependencies, same runs).
        u_ = ar_ >> 6
        s2 = (ar_ & 63) >> 2
        t2p = ar_ & 3
        posperm = (u_ * 4 + t2p) * 16 + s2         # row' -> p''
    else:
        posperm = (ar_ & 7) * 16 + (ar_ >> 3)      # row' -> p''
    Astat = np.empty((B_, 8, P, P), dtype=np.complex64)
    for r in range(8):
        g = _rev(r, 3)
        Astat[:, r] = A[:, ar_ * 8 + r][:, :, g * P + posperm]

    s_, g_ = np.divmod(ar_, 8)                     # k = s*8 + g
    v_, m_ = np.divmod(ar_, 8)
    Bstat = np.empty((B_, 8, P, P), dtype=np.complex64)
    for t2 in range(8):
        rows = g_ * P + t2 * 16 + s_
        cols = P * m_ + 8 * v_ + _rev(t2, 3)
        Bstat[:, t2] = Bm[:, rows][:, :, cols]
    return Astat, Bstat


# ----------------------------------------------------------------------
# Device side: complex matmul kernel (SPMD, one (batch, half) per core).
# ----------------------------------------------------------------------

_USPLIT = [False]

_CACHED_NC = None


def _build_program():
    nc = bacc.Bacc(
        "TRN2", target_bir_lowering=False, debug=False, num_devices=N_CORES
    )

    xr_d = nc.declare_dram_parameter("xr", [T, L], F32, isOutput=False)
    xi_d = nc.declare_dram_parameter("xi", [T, L], F32, isOutput=False)
    wr_d = nc.declare_dram_parameter("wr", [L, L], F32R, isOutput=False)
    wi_d = nc.declare_dram_parameter("wi", [L, L], F32R, isOutput=False)
    out_d = nc.declare_dram_parameter("out", [T, 2 * L], F32, isOutput=True)

    with tile.TileContext(nc) as tc:
        with (
            tc.tile_pool(name="const", bufs=1) as const_pool,
            tc.tile_pool(name="w", bufs=1) as w_pool,
            tc.tile_pool(name="x", bufs=3) as x_pool,
            tc.tile_pool(name="xt", bufs=2) as xt_pool,
            tc.tile_pool(name="osb", bufs=3) as o_pool,
            tc.tile_pool(name="ps", bufs=8, space=bass.MemorySpace.PSUM) as ps_pool,
        ):
            ident = const_pool.tile([P, P], F32)
            make_identity(nc, ident[:])

            # Warm the PE HAM while W streams in: dummy transposes keep the
            # tensor engine busy >3.4us so it reaches full clock before the
            # real matmuls start.
            warm = ps_pool.tile([P, 4 * P], F32, tag="ps")
            for _ in range(12):
                for j in range(4):
                    nc.tensor.transpose(
                        warm[:, j * P : (j + 1) * P], ident[:], ident[:]
                    )

            # Stream W into SBUF once: per k-chunk tiles (P x L), natural layout
            # (partition = contraction row within chunk, free = output column).
            # k-major order so the first token tile's accumulation can start
            # after only a few chunks have landed.
            w_sb = {}
            for k in range(KC):
                for nm, dram in (("wr", wr_d), ("wi", wi_d)):
                    t_ = w_pool.tile([P, L], F32R, tag=f"{nm}{k}")
                    nc.sync.dma_start(out=t_[:], in_=dram[k * P : (k + 1) * P, :])
                    w_sb[nm, k] = t_
                # -Wi derived on device: saves a third of the W stream, which
                # gates the kernel head while PE waits on weights.
                nwi = w_pool.tile([P, L], F32R, tag=f"nwi{k}")
                nc.vector.tensor_scalar_mul(nwi[:], w_sb["wi", k][:], -1.0)
                w_sb["nwi", k] = nwi

            for t in range(NT):
                rows = slice(t * P, (t + 1) * P)
                xr_rows = x_pool.tile([P, L], F32, tag="xr_rows")
                xi_rows = x_pool.tile([P, L], F32, tag="xi_rows")
                nc.sync.dma_start(out=xr_rows[:], in_=xr_d[rows, :])
                nc.sync.dma_start(out=xi_rows[:], in_=xi_d[rows, :])

                # Transpose the token tile: xT chunks live at
                # xT[:, k*P:(k+1)*P] = x_rows[:, k*P:(k+1)*P].T
                xrT = xt_pool.tile([P, L], F32R, tag="xrT")
                xiT = xt_pool.tile([P, L], F32R, tag="xiT")
                for src, dst in ((xr_rows, xrT), (xi_rows, xiT)):
                    for g in range(2):
                        tp = ps_pool.tile([P, 4 * P], F32, tag="ps")
                        for j in range(4):
                            k = g * 4 + j
                            nc.tensor.transpose(
                                tp[:, j * P : (j + 1) * P],
                                src[:, k * P : (k + 1) * P],
                                ident[:],
                            )
                        nc.scalar.copy(dst[:, g * 4 * P : (g + 1) * 4 * P], tp[:])

                # Accumulate the four real matmul outputs.
                #   re_n = sum_k xrT_k @ wr_k[n] + xiT_k @ nwi_k[n]
                #   im_n = sum_k xrT_k @ wi_k[n] + xiT_k @ wr_k[n]
                out_sb = o_pool.tile([P, L, 2], F32, tag="out_sb")
                for n in range(2):
                    ncol = slice(n * 512, (n + 1) * 512)
                    acc_re = ps_pool.tile([P, 512], F32, tag="ps")
                    acc_im = ps_pool.tile([P, 512], F32, tag="ps")
                    for k in range(KC):
                        xrT_k = xrT[:, k * P : (k + 1) * P]
                        xiT_k = xiT[:, k * P : (k + 1) * P]
                        first = k == 0
                        last = k == KC - 1
                        nc.tensor.matmul(
                            acc_re[:], xrT_k, w_sb["wr", k][:, ncol],
                            start=first, stop=False,
                        )
                        nc.tensor.matmul(
                            acc_re[:], xiT_k, w_sb["nwi", k][:, ncol],
                            start=False, stop=last,
                        )
                        nc.tensor.matmul(
                            acc_im[:], xrT_k, w_sb["wi", k][:, ncol],
                            start=first, stop=False,
                        )
                        nc.tensor.matmul(
                            acc_im[:], xiT_k, w_sb["wr", k][:, ncol],
                            start=False, stop=last,
                        )
                    # Interleave re/im into complex64 memory order.
                    nc.vector.tensor_copy(out_sb[:, n * 512 : (n + 1) * 512, 0], acc_re[:])
                    nc.vector.tensor_copy(out_sb[:, n * 512 : (n + 1) * 512, 1], acc_im[:])

                nc.sync.dma_start(out=out_d[rows, :], in_=out_sb[:])

    nc.compile()
    return nc


def _build_program_v3():
    # detect_race_conditions=False: the rust race detector false-positives on
    # the stepped-partition shuffle DMA vs writes to a *different* bin buffer
    # (disjoint SBUF regions sharing a shadow zone). Same-tensor deps are
    # tracked normally and validated by the CoreSim numeric check.
    nc = bacc.Bacc(
        "TRN2", target_bir_lowering=False, debug=False, num_devices=N_CORES,
        detect_race_conditions=False,
    )

    xr_d = nc.declare_dram_parameter("xr", [T, L], F32R, isOutput=False)
    xi_d = nc.declare_dram_parameter("xi", [T, L], F32R, isOutput=False)
    ar_d = nc.declare_dram_parameter("ar", [8 * P, P], F32R, isOutput=False)
    ai_d = nc.declare_dram_parameter("ai", [8 * P, P], F32R, isOutput=False)
    nai_d = nc.declare_dram_parameter("nai", [8 * P, P], F32R, isOutput=False)
    br_d = nc.declare_dram_parameter("br", [8 * P, P], BF16, isOutput=False)
    bi_d = nc.declare_dram_parameter("bi", [8 * P, P], BF16, isOutput=False)
    nbi_d = nc.declare_dram_parameter("nbi", [8 * P, P], BF16, isOutput=False)
    out_d = nc.declare_dram_parameter("out", [T, 2 * L], F32, isOutput=True)

    with tile.TileContext(nc) as tc:
        with (
            tc.tile_pool(name="const", bufs=1) as const_pool,
            tc.tile_pool(name="mats", bufs=1) as mat_pool,
            tc.tile_pool(name="x", bufs=8) as x_pool,
            tc.tile_pool(name="xt", bufs=20) as xt_pool,
            tc.tile_pool(name="ya", bufs=12) as ya_pool,
            tc.tile_pool(name="bin", bufs=1) as bin_pool,
            tc.tile_pool(name="yb", bufs=4) as yb_pool,
            tc.tile_pool(name="osb", bufs=4) as o_pool,
            tc.tile_pool(name="ps", bufs=8, space=bass.MemorySpace.PSUM) as ps_pool,
        ):
            ident = const_pool.tile([P, P], F32)
            make_identity(nc, ident[:])
            ident_h = const_pool.tile([P, P], BF16)
            nc.vector.tensor_copy(ident_h[:], ident[:])
            ident_r = const_pool.tile([P, P], F32R)
            nc.vector.tensor_copy(ident_r[:], ident[:])

            # HAM warmup while the (small) stationaries stream in.
            warm = ps_pool.tile([P, 4 * P], F32, tag="ps")
            for _ in range(22):
                for j in range(4):
                    nc.tensor.transpose(
                        warm[:, j * P : (j + 1) * P], ident[:], ident[:]
                    )

            # Persistent double-buffered shuffle destination; memset once so
            # downstream readers of the stepped-partition DMA writes are
            # observable (sim init tracking) — overlaps with warmup/mats DMA.
            bn_bufs = []
            bn_memsets = []
            for i in range(2):
                bnb = bin_pool.tile([P, 8, 2 * TC], BF16, tag=f"bin{i}")
                bn_memsets.append(nc.gpsimd.memset(bnb[:], 0.0))
                bn_bufs.append(bnb)

            # Mats go through the gpsimd SWDGE queues so the 48 dma_starts do
            # not serialize ahead of chunk-0 row loads on the two HWDGE queues.
            mats = {}
            for nm, dram, dt_ in (
                ("ar", ar_d, F32R), ("ai", ai_d, F32R), ("nai", nai_d, F32R),
                ("br", br_d, BF16), ("bi", bi_d, BF16), ("nbi", nbi_d, BF16),
            ):
                for r in range(8):
                    t_ = mat_pool.tile([P, P], dt_, tag=f"{nm}{r}")
                    nc.gpsimd.dma_start(out=t_[:], in_=dram[r * P : (r + 1) * P, :])
                    mats[nm, r] = t_

            def emit_front(ch):
                """T_in + stage A + shuffle for chunk ch."""
                tok0 = ch * TC
                rows = {}
                for pl, dram in ((0, xr_d), (1, xi_d)):
                    for tt in range(TC // P):
                        rt = x_pool.tile([P, P, 8], F32R, tag="rows")
                        r0 = tok0 + tt * P
                        eng = nc.scalar if (tt % 2) else nc.sync
                        eng.dma_start(out=rt[:], in_=dram[r0 : r0 + P, :])
                        rows[pl, tt] = rt

                xT = {}
                for pl in range(2):
                    for r in range(8):
                        tp = ps_pool.tile([P, 4 * P], F32R, tag="ps")
                        for tt in range(TC // P):
                            nc.tensor.transpose(
                                tp[:, tt * P : (tt + 1) * P],
                                rows[pl, tt][:, :, r],
                                ident_r[:],
                            )
                        dst = xt_pool.tile([P, TC], F32R, tag="xT")
                        nc.scalar.copy(dst[:], tp[:])
                        xT[pl, r] = dst

                yA = {}
                for r in range(8):
                    g = _rev(r, 3)
                    acr = ps_pool.tile([P, TC], F32, tag="ps")
                    aci = ps_pool.tile([P, TC], F32, tag="ps")
                    nc.tensor.matmul(acr[:], mats["ar", r], xT[0, r][:], start=True, stop=False)
                    nc.tensor.matmul(acr[:], mats["nai", r], xT[1, r][:], start=False, stop=True)
                    nc.tensor.matmul(aci[:], mats["ai", r], xT[0, r][:], start=True, stop=False)
                    nc.tensor.matmul(aci[:], mats["ar", r], xT[1, r][:], start=False, stop=True)
                    ya = ya_pool.tile([P, 2 * TC], BF16, tag="ya")
                    nc.vector.tensor_copy(ya[:, 0:TC], acr[:])
                    nc.vector.tensor_copy(ya[:, TC : 2 * TC], aci[:])
                    yA[g] = ya

                # shuffle: Bin[s*8+g, t2, :] = yA[g][s*8+t2, :] — one plain DMA
                # per g; one partition per SBUF port group on both sides.
                bn = bn_bufs[ch % 2]
                for g in range(8):
                    eng = nc.scalar if (g % 2) else nc.sync
                    eng.dma_start(out=bn[g:P:8, :, :], in_=yA[g][:])
                return bn

            def emit_back(ch, bn):
                """Stage B + T_out + interleave + store for chunk ch."""
                tok0 = ch * TC
                out_sb = []
                for tt in range(TC // P):
                    osb = o_pool.tile([P, 2 * L], F32, tag="osb")
                    out_sb.append(osb)
                for t2 in range(8):
                    obr = ps_pool.tile([P, TC], F32, tag="ps")
                    obi = ps_pool.tile([P, TC], F32, tag="ps")
                    b_re = bn[:, t2, 0:TC]
                    b_im = bn[:, t2, TC : 2 * TC]
                    nc.tensor.matmul(obr[:], mats["br", t2], b_re, start=True, stop=False)
                    nc.tensor.matmul(obr[:], mats["nbi", t2], b_im, start=False, stop=True)
                    nc.tensor.matmul(obi[:], mats["bi", t2], b_re, start=True, stop=False)
                    nc.tensor.matmul(obi[:], mats["br", t2], b_im, start=False, stop=True)
                    yb = yb_pool.tile([P, 2 * TC], BF16, tag="yb")
                    nc.scalar.copy(yb[:, 0:TC], obr[:])
                    nc.scalar.copy(yb[:, TC:], obi[:])

                    base = 2 * _rev(t2, 3)
                    for tt in range(TC // P):
                        tp2 = ps_pool.tile([P, 2, 16, 8], BF16, tag="ps")
                        nc.tensor.transpose(
                            tp2[:, 0], yb[:, tt * P : (tt + 1) * P], ident_h[:]
                        )
                        nc.tensor.transpose(
                            tp2[:, 1], yb[:, TC + tt * P : TC + (tt + 1) * P], ident_h[:]
                        )
                        osr = out_sb[tt][:].rearrange(
                            "q (m v lo) -> q lo v m", m=8, v=16, lo=16
                        )
                        nc.vector.tensor_copy(osr[:, base : base + 2, :, :], tp2[:])

                for tt in range(TC // P):
                    r0 = tok0 + tt * P
                    eng = nc.scalar if (tt % 2) else nc.sync
                    eng.dma_start(out=out_d[r0 : r0 + P, :], in_=out_sb[tt][:])

            # Software pipeline: back-half of chunk ch-1 is emitted after the
            # front-half (and shuffle issue) of chunk ch, so the PE stream has
            # B/T_out work in hand while chunk ch's shuffle is in flight.
            prev = None
            for ch in range(NCH):
                bn = emit_front(ch)
                if prev is not None:
                    emit_back(prev[0], prev[1])
                prev = (ch, bn)
            emit_back(prev[0], prev[1])

    nc.compile()
    return nc


def _build_program_v4():
    """Two-stage butterfly with all transposes moved to the host.

    x arrives pre-transposed and r-grouped in HBM as bf16 rows
    (plane, r, p) x tok, so stage-A moving operands are plain contiguous
    loads.  Stage A: acc[row', tok] = A_r^T x_r with the A/B stage
    matrices stationary; the stepped-partition SBUF shuffle regroups
    (s,t2) -> (s,g) partitions for stage B; stage-B results [j', tok]
    are stored position-major and the host undoes the butterfly output
    permutation + transpose.  No PE transposes, no output interleave.
    """
    nc = bacc.Bacc(
        "TRN2", target_bir_lowering=False, debug=False, num_devices=N_CORES,
        detect_race_conditions=False,
    )

    xt_d = nc.declare_dram_parameter("xt", [16 * P, T], BF16, isOutput=False)
    # All 48 stationaries packed: [P, (6 kinds x 8 r) * P] so one DMA with
    # 12 KiB partition lines loads everything (48 separate [P,P] DMAs would
    # be 256 B/line, descriptor-overhead-bound).
    mat_d = nc.declare_dram_parameter("mat", [P, 48 * P], BF16, isOutput=False)
    # bf16 output, chunk-major: row (ch, t2, j') holds [re TC | im TC] so
    # every store writes full contiguous 2 KiB dram rows.  Host upcasts.
    out_d = nc.declare_dram_parameter("out", [NCH * 8 * P, 2 * TC], BF16, isOutput=True)

    with tile.TileContext(nc) as tc:
        with (
            tc.tile_pool(name="const", bufs=1) as const_pool,
            tc.tile_pool(name="mats", bufs=1) as mat_pool,
            tc.tile_pool(name="x", bufs=1) as x_pool,
            tc.tile_pool(name="ya", bufs=3) as ya_pool,
            tc.tile_pool(name="bin", bufs=1) as bin_pool,
            tc.tile_pool(name="osb", bufs=1) as o_pool,
            tc.tile_pool(name="ps", bufs=4, space=bass.MemorySpace.PSUM) as ps_pool,
        ):
            ident = const_pool.tile([P, P], F32)
            make_identity(nc, ident[:])

            # Short HAM warmup (~4.7us cold) covering the chunk-0 DMA window.
            warm = ps_pool.tile([P, 2, TC], F32, tag="ps")
            for _ in range(11):
                for j in range(4):
                    nc.tensor.transpose(
                        warm[:, j // 2, (j % 2) * P : (j % 2 + 1) * P],
                        ident[:], ident[:],
                    )

            # One shuffle destination per chunk: all fronts are emitted before
            # any back, so no buffer reuse hazards at all.  The stepped-
            # partition shuffle DMAs fully overwrite each buffer; no init
            # needed (race detection is disabled).
            bn_bufs = []
            for i in range(NCH):
                bnb = bin_pool.tile([P, 8, 2, TC], BF16, tag=f"bin{i}")
                bn_bufs.append(bnb)

            # Packed stationaries on the gpsimd SWDGE queue: bulk load that
            # must not block the latency-critical HWDGE queues.
            mat_all = mat_pool.tile([P, 48 * P], BF16, tag="mat")
            nc.gpsimd.dma_start(out=mat_all[:], in_=mat_d[:, :])
            mats = {}
            for ki, nm in enumerate(("ar", "ai", "nai", "br", "bi", "nbi")):
                for r in range(8):
                    idx = ki * 8 + r
                    mats[nm, r] = mat_all[:, idx * P : (idx + 1) * P]

            # Full x panel upfront on the HWDGE queues (empty at the head):
            # 16 DMAs with 4 KiB partition lines; r=0's tiles land first so
            # stage A starts after ~1.5us.  Resident all run (64 KiB/part).
            xs = {}
            for r in range(8):
                for pl in range(2):
                    xtile = x_pool.tile([P, T], BF16, tag=f"x{pl}_{r}")
                    row0 = (pl * 8 + r) * P
                    eng = nc.scalar if (pl % 2) else nc.sync
                    eng.dma_start(out=xtile[:], in_=xt_d[row0 : row0 + P, :])
                    xs[pl, r] = xtile

            def emit_front(ch):
                """Stage A + cast + shuffle for chunk ch."""
                csl = slice(ch * TC, (ch + 1) * TC)
                bn = bn_bufs[ch]
                for r in range(8):
                    g = _rev(r, 3)
                    acc = ps_pool.tile([P, 2, TC], F32, tag="ps")
                    acr = acc[:, 0, :]
                    aci = acc[:, 1, :]
                    nc.tensor.matmul(acr, mats["ar", r], xs[0, r][:, csl], start=True, stop=False)
                    nc.tensor.matmul(aci, mats["ar", r], xs[1, r][:, csl], start=True, stop=False)
                    nc.tensor.matmul(aci, mats["ai", r], xs[0, r][:, csl], start=False, stop=True)
                    nc.tensor.matmul(acr, mats["nai", r], xs[1, r][:, csl], start=False, stop=True)
                    ya = ya_pool.tile([P, 2, TC], BF16, tag=f"ya{g}")
                    if r % 2:
                        nc.scalar.copy(ya[:], acc[:])
                    else:
                        nc.vector.tensor_copy(ya[:], acc[:])
                    # Shuffle: bn[s*8+g, t2, c, :] = ya[s*8+t2, c, :]
                    eng2 = nc.scalar if (g % 2) else nc.sync
                    eng2.dma_start(out=bn[g:P:8, :, :, :], in_=ya[:])
                return bn

            def emit_back(ch, bn):
                """Stage B + store (position-major, bf16) for chunk ch."""
                for t2 in range(8):
                    ob = ps_pool.tile([P, 2, TC], F32, tag="ps")
                    obr = ob[:, 0, :]
                    obi = ob[:, 1, :]
                    b_re = bn[:, t2, 0, :]
                    b_im = bn[:, t2, 1, :]
                    nc.tensor.matmul(obr, mats["br", t2], b_re, start=True, stop=False)
                    nc.tensor.matmul(obi, mats["br", t2], b_im, start=True, stop=False)
                    nc.tensor.matmul(obi, mats["bi", t2], b_re, start=False, stop=True)
                    nc.tensor.matmul(obr, mats["nbi", t2], b_im, start=False, stop=True)
                    osb = o_pool.tile([P, 2, TC], BF16, tag=f"osb{t2}")
                    if t2 % 2:
                        nc.scalar.copy(osb[:], ob[:])
                    else:
                        nc.vector.tensor_copy(osb[:], ob[:])
                    row0 = (ch * 8 + t2) * P
                    eng = nc.sync if (t2 % 2) else nc.scalar
                    eng.dma_start(out=out_d[row0 : row0 + P, :], in_=osb[:])

            # 2-chunk lookahead: B(ch) is emitted two fronts after F(ch), so
            # its shuffle has ~2 chunks of port time to land before the PE
            # reaches it.
            bns = {}
            order = []
            for ch in range(NCH):
                order.append(("F", ch))
                if ch >= 2:
                    order.append(("B", ch - 2))
            order += [("B", NCH - 2), ("B", NCH - 1)]
            for kind, ch in order:
                if kind == "F":
                    bns[ch] = emit_front(ch)
                else:
                    emit_back(ch, bns[ch])

    nc.compile()
    return nc


def _build_program_v5(int8_shuffle: bool):
    """Token-sharded two-stage butterfly: each core runs TOK5 tokens through
    all 4 mesh-batches.  Cuts the x input to 2 MiB/core (vs 8) at the cost of
    4 MiB of stationaries, minimizing SBUF-AXI-port traffic (the measured
    bottleneck).  Optional int8 inter-stage shuffle halves the port cost of
    the partition-regroup DMA (scales folded into the stage matrices on host;
    clamped vector casts; int8->bf16 upcast split across engines)."""
    nc = bacc.Bacc(
        "TRN2", target_bir_lowering=False, debug=False, num_devices=N_CORES,
        detect_race_conditions=False,
    )
    B = MESH_BATCH
    TCv = TOK5  # 512 tokens = one chunk per batch

    xt_d = nc.declare_dram_parameter("xt", [P, 8, 2, TCv], BF16, isOutput=False)
    mat_d = nc.declare_dram_parameter("mat", [P, B, 4, 8, P], BF16, isOutput=False)
    # out rows (b*2+q)*P + j', free [t2' in 4, c in 2, TCv]; t2 = q*4+t2'.
    out_d = nc.declare_dram_parameter("out", [B * 2 * P, 4 * 2 * TCv], BF16,
                                      isOutput=True)

    ydt = I8 if int8_shuffle else BF16

    with tile.TileContext(nc) as tc:
        with (
            tc.tile_pool(name="const", bufs=1) as const_pool,
            tc.tile_pool(name="mats", bufs=1) as mat_pool,
            tc.tile_pool(name="x", bufs=1) as x_pool,
            tc.tile_pool(name="ya", bufs=10) as ya_pool,
            tc.tile_pool(name="bin", bufs=1) as bin_pool,
            tc.tile_pool(name="bnh", bufs=2) as bnh_pool,
            tc.tile_pool(name="osb", bufs=3) as o_pool,
            tc.tile_pool(name="ps", bufs=4, space=bass.MemorySpace.PSUM) as ps_pool,
        ):
            ident = const_pool.tile([P, P], F32)
            make_identity(nc, ident[:])

            # HAM warmup covering the head DMA window (~9us of PE activity).
            warm = ps_pool.tile([P, 2, TCv], F32, tag="ps")
            for _ in range(10):
                for j in range(4):
                    nc.tensor.transpose(
                        warm[:, j // 2, (j % 2) * P : (j % 2 + 1) * P],
                        ident[:], ident[:],
                    )

            # Stationaries per batch (kinds ar, ai, br, bi) + derived
            # negations.  mat0 goes FIRST on the sync ring (it gates A0 and
            # the ring is otherwise empty, so it lands in ~2.5us); the x
            # panel follows on sync; mats 1-3 stream on the gpsimd ring.
            matb, negb = [], []
            for b in range(B):
                mb = mat_pool.tile([P, 4, 8, P], BF16, tag=f"mat{b}",
                                   name=f"mat{b}")
                ng = mat_pool.tile([P, 2, 8, P], BF16, tag=f"neg{b}",
                                   name=f"neg{b}")
                matb.append(mb)
                negb.append(ng)

            # Head loads: x split across the sync and scalar rings, the
            # stationaries FIFO-ordered (batch 0 first) on the gpsimd ring.
            # Measured best; variants that serialize x on one ring or put
            # mats on the HWDGE rings delay the first shuffles and lose
            # 5-10us.
            xs = x_pool.tile([P, 8, 2, TCv], BF16, tag="xs")
            nc.sync.dma_start(out=xs[:, 0:4], in_=xt_d[:, 0:4])
            nc.scalar.dma_start(out=xs[:, 4:8], in_=xt_d[:, 4:8])
            for b in range(B):
                nc.gpsimd.dma_start(out=matb[b][:], in_=mat_d[:, b])
            negs_done = [False] * B

            bn = []
            for b in range(B):
                bnb = bin_pool.tile([P, 8, 2, TCv], ydt, tag=f"bin{b}")
                bn.append(bnb)

            def emit_A(b):
                mb, ng = matb[b], negb[b]
                if not negs_done[b]:
                    # ng[:,0] = -ai here; -bi is deferred to emit_B so each
                    # negation only costs the scalar FIFO ~1us per phase.
                    nc.scalar.mul(ng[:, 0], mb[:, 1], -1.0)
                    negs_done[b] = True
                for r in range(8):
                    g = _rev(r, 3)
                    acc = ps_pool.tile([P, 2, TCv], F32, tag="ps")
                    ar_ = mb[:, 0, r, :]
                    ai_ = mb[:, 1, r, :]
                    nai = ng[:, 0, r, :]
                    xr_ = xs[:, r, 0, :]
                    xi_ = xs[:, r, 1, :]
                    nc.tensor.matmul(acc[:, 0, :], ar_, xr_, start=True, stop=False)
                    nc.tensor.matmul(acc[:, 0, :], nai, xi_, start=False, stop=True)
                    nc.tensor.matmul(acc[:, 1, :], ai_, xr_, start=True, stop=False)
                    nc.tensor.matmul(acc[:, 1, :], ar_, xi_, start=False, stop=True)
                    ya = ya_pool.tile([P, 2, TCv], ydt, tag="ya")
                    if r in (1, 3, 5):
                        nc.scalar.copy(ya[:], acc[:])
                    else:
                        nc.vector.tensor_copy(ya[:], acc[:])
                    # per-u half shuffles: B(b) t2<4 waits only on the u=0
                    # halves; same 2 KiB descriptor runs.
                    e_lo = nc.scalar if (g % 2) else nc.sync
                    e_hi = nc.sync if (g % 2) else nc.scalar
                    e_lo.dma_start(out=bn[b][g:P:8, 0:4], in_=ya[0:64])
                    e_hi.dma_start(out=bn[b][g:P:8, 4:8], in_=ya[64:128])

            def emit_casts(b):
                """int8 bn -> bf16 for the stage-B moving operand."""
                bh = bnh_pool.tile([P, 8, 2, TCv], BF16, tag=f"bnh{b % 2}")
                nc.gpsimd.tensor_copy(bh[:, 0:3], bn[b][:, 0:3])
                nc.vector.tensor_copy(bh[:, 3:5], bn[b][:, 3:5])
                nc.gpsimd.tensor_copy(bh[:, 5:7], bn[b][:, 5:7])
                nc.scalar.copy(bh[:, 7:8], bn[b][:, 7:8])
                return bh

            def emit_B(b, bh):
                mb, ng = matb[b], negb[b]
                # ng[:,1] = -bi (see emit_A)
                nc.scalar.mul(ng[:, 1], mb[:, 3], -1.0)
                src = bh if bh is not None else bn[b]
                osb = None
                for t2 in range(8):
                    acc = ps_pool.tile([P, 2, TCv], F32, tag="ps")
                    br_ = mb[:, 2, t2, :]
                    bi_ = mb[:, 3, t2, :]
                    nbi = ng[:, 1, t2, :]
                    yre = src[:, t2, 0, :]
                    yim = src[:, t2, 1, :]
                    nc.tensor.matmul(acc[:, 0, :], br_, yre, start=True, stop=False)
                    nc.tensor.matmul(acc[:, 0, :], nbi, yim, start=False, stop=True)
                    nc.tensor.matmul(acc[:, 1, :], bi_, yre, start=True, stop=False)
                    nc.tensor.matmul(acc[:, 1, :], br_, yim, start=False, stop=True)
                    if t2 % 2 == 0:
                        osb = o_pool.tile([P, 2, 2, TCv], BF16, tag="osb")
                    if t2 % 2:
                        nc.scalar.copy(osb[:, 1], acc[:])
                    else:
                        nc.vector.tensor_copy(osb[:, 0], acc[:])
                    if t2 % 2 == 1:
                        # out_d rows (b*2+q)*P+j', free [t2' in 4, c, t]; this
                        # pair covers t2' in {t2-1, t2} of quad q = t2 // 4.
                        q, t2p = divmod(t2 - 1, 4)
                        row0 = (b * 2 + q) * P
                        dst = out_d[row0 : row0 + P, :].rearrange(
                            "p (tp c t) -> p tp c t", tp=4, c=2
                        )[:, t2p : t2p + 2]
                        eng2 = nc.sync if (t2 % 4 == 1) else nc.scalar
                        eng2.dma_start(out=dst, in_=osb[:])

            # Pipeline: B(b) emitted after A(b+1) so the b-shuffle has a full
            # A-phase of DMA time to land before the PE needs it.
            emit_A(0)
            for b in range(B):
                if b + 1 < B:
                    emit_A(b + 1)
                bh = emit_casts(b) if int8_shuffle else None
                emit_B(b, bh)

    nc.compile()
    return nc


def _build_program_v8():
    """v7 with 4-kind stationaries (ar, ai, br, bi — 4 MiB instead of 6).
    The negated operands move to the moving side: xs carries a third plane
    nxi = -xi (negated once at the head), and bn carries a third slot
    nyim = -y_im (negated after each shuffle lands, split across vector and
    scalar).  re = ar@xr + ai@nxi, im = ai@xr + ar@xi, and likewise for B."""
    nc = bacc.Bacc(
        "TRN2", target_bir_lowering=False, debug=False, num_devices=N_CORES,
        detect_race_conditions=False,
    )
    B = MESH_BATCH
    TCv = TOK5

    xt_d = nc.declare_dram_parameter("xt", [P, 8, 2, TCv], BF16, isOutput=False)
    # kinds: 0=ar 1=ai 2=br 3=bi
    mat_d = nc.declare_dram_parameter("mat", [P, B, 4, 8, P], BF16, isOutput=False)
    out_d = nc.declare_dram_parameter("out", [B * 2 * P, 4 * 2 * TCv], BF16,
                                      isOutput=True)

    with tile.TileContext(nc) as tc:
        with (
            tc.tile_pool(name="const", bufs=1) as const_pool,
            tc.tile_pool(name="mats", bufs=1) as mat_pool,
            tc.tile_pool(name="x", bufs=1) as x_pool,
            tc.tile_pool(name="ya", bufs=14) as ya_pool,
            tc.tile_pool(name="bin", bufs=1) as bin_pool,
            tc.tile_pool(name="osb", bufs=4) as o_pool,
            tc.tile_pool(name="ps", bufs=4, space=bass.MemorySpace.PSUM) as ps_pool,
        ):
            # --- all input DMAs first: nothing blocks the rings ---
            xs = x_pool.tile([P, 8, 2, TCv], BF16, tag="xs")
            for i in range(4):
                nc.sync.dma_start(out=xs[:, 2 * i : 2 * i + 2],
                                  in_=xt_d[:, 2 * i : 2 * i + 2])
            xn = x_pool.tile([P, 8, TCv], BF16, tag="xn")

            matA, matB = [], []
            for b in range(B):
                mA = mat_pool.tile([P, 2, 8, P], BF16, tag=f"matA{b}",
                                   name=f"matA{b}")
                mB = mat_pool.tile([P, 2, 8, P], BF16, tag=f"matB{b}",
                                   name=f"matB{b}")
                matA.append(mA)
                matB.append(mB)
            # balance the mats across the scalar and gpsimd rings, earliest
            # batches first, so each matX_b lands just ahead of its phase.
            for b in range(B):
                eng = nc.scalar if b < 2 else nc.gpsimd
                eng.dma_start(out=matA[b][:], in_=mat_d[:, b, 0:2])
                eng.dma_start(out=matB[b][:], in_=mat_d[:, b, 2:4])

            ident = const_pool.tile([P, P], F32)
            make_identity(nc, ident[:])

            warm = ps_pool.tile([P, 2, TCv], F32, tag="ps")
            for _ in range(5):
                for j in range(4):
                    nc.tensor.transpose(
                        warm[:, j // 2, (j % 2) * P : (j % 2 + 1) * P],
                        ident[:], ident[:],
                    )

            # nxi planes, negated per-r on vector while mats stream in.
            for r in range(8):
                nc.vector.tensor_scalar_mul(xn[:, r], xs[:, r, 1], -1.0)

            bn = [
                bin_pool.tile([P, 8, 2, TCv], BF16, tag=f"bin{b}", name=f"bin{b}")
                for b in range(B)
            ]
            # nyim planes (negated post-shuffle)
            bnn = [
                bin_pool.tile([P, 8, TCv], BF16, tag=f"binn{b}", name=f"binn{b}")
                for b in range(B)
            ]

            def emit_A(b):
                mA = matA[b]
                for r in range(8):
                    g = _rev(r, 3)
                    acc = ps_pool.tile([P, 2, TCv], F32, tag="ps")
                    ar_ = mA[:, 0, r, :]
                    ai_ = mA[:, 1, r, :]
                    xr_ = xs[:, r, 0, :]
                    xi_ = xs[:, r, 1, :]
                    nxi = xn[:, r, :]
                    nc.tensor.matmul(acc[:, 0, :], ar_, xr_, start=True, stop=False)
                    nc.tensor.matmul(acc[:, 0, :], ai_, nxi, start=False, stop=True)
                    nc.tensor.matmul(acc[:, 1, :], ai_, xr_, start=True, stop=False)
                    nc.tensor.matmul(acc[:, 1, :], ar_, xi_, start=False, stop=True)
                    ya = ya_pool.tile([P, 2, TCv], BF16, tag="ya")
                    if r % 2:
                        nc.scalar.copy(ya[:], acc[:])
                    else:
                        nc.vector.tensor_copy(ya[:], acc[:])
                    eng2 = nc.scalar if (g % 2) else nc.sync
                    eng2.dma_start(out=bn[b][g:P:8], in_=ya[:])

            def emit_negs(b):
                # nyim = -yim once the b-shuffle has landed; split across
                # engines so neither eats a full phase.
                nc.vector.tensor_scalar_mul(bnn[b][:, 0:4], bn[b][:, 0:4, 1], -1.0)
                nc.scalar.mul(bnn[b][:, 4:8], bn[b][:, 4:8, 1], -1.0)

            def emit_B(b):
                mB = matB[b]
                osb = None
                for t2 in range(8):
                    acc = ps_pool.tile([P, 2, TCv], F32, tag="ps")
                    br_ = mB[:, 0, t2, :]
                    bi_ = mB[:, 1, t2, :]
                    yre = bn[b][:, t2, 0, :]
                    yim = bn[b][:, t2, 1, :]
                    nyim = bnn[b][:, t2, :]
                    nc.tensor.matmul(acc[:, 0, :], br_, yre, start=True, stop=False)
                    nc.tensor.matmul(acc[:, 0, :], bi_, nyim, start=False, stop=True)
                    nc.tensor.matmul(acc[:, 1, :], bi_, yre, start=True, stop=False)
                    nc.tensor.matmul(acc[:, 1, :], br_, yim, start=False, stop=True)
                    if t2 % 2 == 0:
                        osb = o_pool.tile([P, 2, 2, TCv], BF16, tag="osb")
                    if t2 % 2:
                        nc.scalar.copy(osb[:, 1], acc[:])
                    else:
                        nc.vector.tensor_copy(osb[:, 0], acc[:])
                    if t2 % 2 == 1:
                        # out_d rows (b*2+q)*P+j', free [t2' in 4, c, t]; this
                        # pair covers t2' in {t2-1, t2} of quad q = t2 // 4.
                        q, t2p = divmod(t2 - 1, 4)
                        row0 = (b * 2 + q) * P
                        dst = out_d[row0 : row0 + P, :].rearrange(
                            "p (tp c t) -> p tp c t", tp=4, c=2
                        )[:, t2p : t2p + 2]
                        eng2 = nc.sync if (t2 % 4 == 1) else nc.scalar
                        eng2.dma_start(out=dst, in_=osb[:])

            # B(b) two phases after A(b): each shuffle gets ~2 phases of
            # port time before the PE needs it.
            emit_A(0)
            emit_A(1)
            emit_A(2)
            emit_negs(0)
            emit_B(0)
            emit_A(3)
            emit_negs(1)
            emit_B(1)
            emit_negs(2)
            emit_B(2)
            emit_negs(3)
            emit_B(3)

    nc.compile()
    return nc


def _build_program_v7():
    """v5 pipeline (unpaired shuffle, balanced A/B interleave) with the v6
    6-kind stationaries, plus head fixes: DMA triggers are emitted before
    make_identity (which otherwise blocks the sync/gpsimd FIFOs for ~1.5us),
    stationaries are split into A/B-kind halves with batch 0 on the scalar
    HWDGE ring so stage A can start as early as possible, and the PE warmup
    is sized to the head gap."""
    nc = bacc.Bacc(
        "TRN2", target_bir_lowering=False, debug=False, num_devices=N_CORES,
        detect_race_conditions=False,
    )
    B = MESH_BATCH
    TCv = TOK5

    xt_d = nc.declare_dram_parameter("xt", [P, 8, 2, TCv], BF16, isOutput=False)
    # kinds: 0=ar 1=ai 2=nai 3=br 4=bi 5=nbi
    mat_d = nc.declare_dram_parameter("mat", [P, B, 6, 8, P], BF16, isOutput=False)
    out_d = nc.declare_dram_parameter("out", [B * 2 * P, 4 * 2 * TCv], BF16,
                                      isOutput=True)

    with tile.TileContext(nc) as tc:
        with (
            tc.tile_pool(name="const", bufs=1) as const_pool,
            tc.tile_pool(name="mats", bufs=1) as mat_pool,
            tc.tile_pool(name="x", bufs=1) as x_pool,
            tc.tile_pool(name="ya", bufs=6) as ya_pool,
            tc.tile_pool(name="bin", bufs=1) as bin_pool,
            tc.tile_pool(name="osb", bufs=3) as o_pool,
            tc.tile_pool(name="ps", bufs=4, space=bass.MemorySpace.PSUM) as ps_pool,
        ):
            # --- all input DMAs first: nothing blocks the rings ---
            xs = x_pool.tile([P, 8, 2, TCv], BF16, tag="xs")
            nc.sync.dma_start(out=xs[:, 0:4], in_=xt_d[:, 0:4])
            nc.sync.dma_start(out=xs[:, 4:8], in_=xt_d[:, 4:8])

            matA, matB = [], []
            for b in range(B):
                mA = mat_pool.tile([P, 3, 8, P], BF16, tag=f"matA{b}",
                                   name=f"matA{b}")
                mB = mat_pool.tile([P, 3, 8, P], BF16, tag=f"matB{b}",
                                   name=f"matB{b}")
                matA.append(mA)
                matB.append(mB)
            # batch 0 on the (otherwise idle) scalar ring for earliest arrival;
            # the rest stream in FIFO order on the gpsimd SWDGE ring.
            nc.scalar.dma_start(out=matA[0][:], in_=mat_d[:, 0, 0:3])
            nc.scalar.dma_start(out=matB[0][:], in_=mat_d[:, 0, 3:6])
            for b in range(1, B):
                nc.gpsimd.dma_start(out=matA[b][:], in_=mat_d[:, b, 0:3])
                nc.gpsimd.dma_start(out=matB[b][:], in_=mat_d[:, b, 3:6])

            ident = const_pool.tile([P, P], F32)
            make_identity(nc, ident[:])

            # Short HAM warmup sized to the ~4us head gap.
            warm = ps_pool.tile([P, 2, TCv], F32, tag="ps")
            for _ in range(5):
                for j in range(4):
                    nc.tensor.transpose(
                        warm[:, j // 2, (j % 2) * P : (j % 2 + 1) * P],
                        ident[:], ident[:],
                    )

            bn = [
                bin_pool.tile([P, 8, 2, TCv], BF16, tag=f"bin{b}", name=f"bin{b}")
                for b in range(B)
            ]

            def emit_A(b):
                mA = matA[b]
                for r in range(8):
                    g = _rev(r, 3)
                    acc = ps_pool.tile([P, 2, TCv], F32, tag="ps")
                    ar_ = mA[:, 0, r, :]
                    ai_ = mA[:, 1, r, :]
                    nai = mA[:, 2, r, :]
                    xr_ = xs[:, r, 0, :]
                    xi_ = xs[:, r, 1, :]
                    nc.tensor.matmul(acc[:, 0, :], ar_, xr_, start=True, stop=False)
                    nc.tensor.matmul(acc[:, 0, :], nai, xi_, start=False, stop=True)
                    nc.tensor.matmul(acc[:, 1, :], ai_, xr_, start=True, stop=False)
                    nc.tensor.matmul(acc[:, 1, :], ar_, xi_, start=False, stop=True)
                    ya = ya_pool.tile([P, 2, TCv], BF16, tag="ya")
                    if r % 2:
                        nc.scalar.copy(ya[:], acc[:])
                    else:
                        nc.vector.tensor_copy(ya[:], acc[:])
                    eng2 = nc.scalar if (g % 2) else nc.sync
                    eng2.dma_start(out=bn[b][g:P:8], in_=ya[:])

            def emit_B(b):
                mB = matB[b]
                osb = None
                for t2 in range(8):
                    acc = ps_pool.tile([P, 2, TCv], F32, tag="ps")
                    br_ = mB[:, 0, t2, :]
                    bi_ = mB[:, 1, t2, :]
                    nbi = mB[:, 2, t2, :]
                    yre = bn[b][:, t2, 0, :]
                    yim = bn[b][:, t2, 1, :]
                    nc.tensor.matmul(acc[:, 0, :], br_, yre, start=True, stop=False)
                    nc.tensor.matmul(acc[:, 0, :], nbi, yim, start=False, stop=True)
                    nc.tensor.matmul(acc[:, 1, :], bi_, yre, start=True, stop=False)
                    nc.tensor.matmul(acc[:, 1, :], br_, yim, start=False, stop=True)
                    if t2 % 2 == 0:
                        osb = o_pool.tile([P, 2, 2, TCv], BF16, tag="osb")
                    if t2 % 2:
                        nc.scalar.copy(osb[:, 1], acc[:])
                    else:
                        nc.vector.tensor_copy(osb[:, 0], acc[:])
                    if t2 % 2 == 1:
                        # out_d rows (b*2+q)*P+j', free [t2' in 4, c, t]; this
                        # pair covers t2' in {t2-1, t2} of quad q = t2 // 4.
                        q, t2p = divmod(t2 - 1, 4)
                        row0 = (b * 2 + q) * P
                        dst = out_d[row0 : row0 + P, :].rearrange(
                            "p (tp c t) -> p tp c t", tp=4, c=2
                        )[:, t2p : t2p + 2]
                        eng2 = nc.sync if (t2 % 4 == 1) else nc.scalar
                        eng2.dma_start(out=dst, in_=osb[:])

            for step in ("A0", "A1", "B0", "A2", "B1", "A3", "B2", "B3"):
                (emit_A if step[0] == "A" else emit_B)(int(step[1]))

    nc.compile()
    return nc


def _build_program_v6():
    """v5 + (a) all 6 stationary kinds from HBM (no on-device negation: the
    1.9us scalar negates stalled the PSUM-evac path at each phase head) and
    (b) batch-paired shuffle: stage-A results for batches (2p, 2p+1) share one
    ya2 tile, so each partition-regroup DMA moves 4 KiB runs (halves the
    descriptor count of the port-bound shuffle)."""
    nc = bacc.Bacc(
        "TRN2", target_bir_lowering=False, debug=False, num_devices=N_CORES,
        detect_race_conditions=False,
    )
    B = MESH_BATCH
    TCv = TOK5

    xt_d = nc.declare_dram_parameter("xt", [P, 8, 2, TCv], BF16, isOutput=False)
    # kinds: 0=ar 1=ai 2=nai 3=br 4=bi 5=nbi
    mat_d = nc.declare_dram_parameter("mat", [P, B, 6, 8, P], BF16, isOutput=False)
    out_d = nc.declare_dram_parameter("out", [B * 2 * P, 4 * 2 * TCv], BF16,
                                      isOutput=True)

    with tile.TileContext(nc) as tc:
        with (
            tc.tile_pool(name="const", bufs=1) as const_pool,
            tc.tile_pool(name="mats", bufs=1) as mat_pool,
            tc.tile_pool(name="x", bufs=1) as x_pool,
            tc.tile_pool(name="ya", bufs=9) as ya_pool,
            tc.tile_pool(name="bin", bufs=1) as bin_pool,
            tc.tile_pool(name="osb", bufs=2) as o_pool,
            tc.tile_pool(name="ps", bufs=4, space=bass.MemorySpace.PSUM) as ps_pool,
        ):
            ident = const_pool.tile([P, P], F32)
            make_identity(nc, ident[:])

            warm = ps_pool.tile([P, 2, TCv], F32, tag="ps")
            for _ in range(9):
                for j in range(4):
                    nc.tensor.transpose(
                        warm[:, j // 2, (j % 2) * P : (j % 2 + 1) * P],
                        ident[:], ident[:],
                    )

            xs = x_pool.tile([P, 8, 2, TCv], BF16, tag="xs")
            nc.sync.dma_start(out=xs[:, 0:4], in_=xt_d[:, 0:4])
            nc.sync.dma_start(out=xs[:, 4:8], in_=xt_d[:, 4:8])

            matb = []
            for b in range(B):
                mb = mat_pool.tile([P, 6, 8, P], BF16, tag=f"mat{b}")
                nc.gpsimd.dma_start(out=mb[:], in_=mat_d[:, b])
                matb.append(mb)

            bn2 = [
                bin_pool.tile([P, 8, 2, 2, TCv], BF16, tag=f"bin{p}",
                              name=f"bin{p}")
                for p in range(2)
            ]
            ya2 = {}

            def emit_A(b):
                mb = matb[b]
                pair, half = divmod(b, 2)
                for r in range(8):
                    g = _rev(r, 3)
                    acc = ps_pool.tile([P, 2, TCv], F32, tag="ps")
                    ar_ = mb[:, 0, r, :]
                    ai_ = mb[:, 1, r, :]
                    nai = mb[:, 2, r, :]
                    xr_ = xs[:, r, 0, :]
                    xi_ = xs[:, r, 1, :]
                    nc.tensor.matmul(acc[:, 0, :], ar_, xr_, start=True, stop=False)
                    nc.tensor.matmul(acc[:, 0, :], nai, xi_, start=False, stop=True)
                    nc.tensor.matmul(acc[:, 1, :], ai_, xr_, start=True, stop=False)
                    nc.tensor.matmul(acc[:, 1, :], ar_, xi_, start=False, stop=True)
                    if half == 0:
                        ya2[pair, g] = ya_pool.tile(
                            [P, 2, 2, TCv], BF16, tag="ya", name=f"ya{pair}_{g}"
                        )
                    dst = ya2[pair, g][:, half]
                    if r % 2:
                        nc.scalar.copy(dst, acc[:])
                    else:
                        nc.vector.tensor_copy(dst, acc[:])
                    if half == 1:
                        eng2 = nc.scalar if (g % 2) else nc.sync
                        eng2.dma_start(
                            out=bn2[pair][g:P:8], in_=ya2[pair, g][:]
                        )

            def emit_B(b):
                mb = matb[b]
                pair, half = divmod(b, 2)
                osb = None
                for t2 in range(8):
                    acc = ps_pool.tile([P, 2, TCv], F32, tag="ps")
                    br_ = mb[:, 3, t2, :]
                    bi_ = mb[:, 4, t2, :]
                    nbi = mb[:, 5, t2, :]
                    yre = bn2[pair][:, t2, half, 0, :]
                    yim = bn2[pair][:, t2, half, 1, :]
                    nc.tensor.matmul(acc[:, 0, :], br_, yre, start=True, stop=False)
                    nc.tensor.matmul(acc[:, 0, :], nbi, yim, start=False, stop=True)
                    nc.tensor.matmul(acc[:, 1, :], bi_, yre, start=True, stop=False)
                    nc.tensor.matmul(acc[:, 1, :], br_, yim, start=False, stop=True)
                    if t2 % 2 == 0:
                        osb = o_pool.tile([P, 2, 2, TCv], BF16, tag="osb")
                    if t2 % 2:
                        nc.scalar.copy(osb[:, 1], acc[:])
                    else:
                        nc.vector.tensor_copy(osb[:, 0], acc[:])
                    if t2 % 2 == 1:
                        # out_d rows (b*2+q)*P+j', free [t2' in 4, c, t]; this
                        # pair covers t2' in {t2-1, t2} of quad q = t2 // 4.
                        q, t2p = divmod(t2 - 1, 4)
                        row0 = (b * 2 + q) * P
                        dst = out_d[row0 : row0 + P, :].rearrange(
                            "p (tp c t) -> p tp c t", tp=4, c=2
                        )[:, t2p : t2p + 2]
                        eng2 = nc.sync if (t2 % 4 == 1) else nc.scalar
                        eng2.dma_start(out=dst, in_=osb[:])

            for step in ("A0", "A1", "A2", "B0", "A3", "B1", "B2", "B3"):
                (emit_A if step[0] == "A" else emit_B)(int(step[1]))

    nc.compile()
    return nc


_CACHED = {}


def _host_prep_v4(x_re, x_im, phases):
    """Host-side: transposed/r-grouped bf16 x per (core-half), bf16 mats."""
    import ml_dtypes

    Astat, Bstat = _stage_matrices(phases)
    bf = ml_dtypes.bfloat16
    ar = Astat.real.reshape(MESH_BATCH, 8 * P, P).astype(bf)
    ai = Astat.imag.reshape(MESH_BATCH, 8 * P, P).astype(bf)
    br = Bstat.real.reshape(MESH_BATCH, 8 * P, P).astype(bf)
    bi = Bstat.imag.reshape(MESH_BATCH, 8 * P, P).astype(bf)

    half = N_TOKENS // 2
    xts = []
    for h in range(2):
        planes = []
        for xp in (x_re, x_im):
            # [T, L] -> [L, T] -> (p, r) rows -> [r, p, T]
            xT = xp[h * half : (h + 1) * half].T.reshape(P, 8, half)
            planes.append(xT.transpose(1, 0, 2))
        xt = np.concatenate(planes, axis=0).reshape(16 * P, half)
        xts.append(np.ascontiguousarray(xt).astype(bf))
    return ar, ai, br, bi, xts


_JCOLS = None


def _jcols():
    global _JCOLS
    if _JCOLS is None:
        idx = np.arange(P)
        v_, m_ = np.divmod(idx, 8)
        _JCOLS = [P * m_ + 8 * v_ + _rev(t2, 3) for t2 in range(8)]
    return _JCOLS


def _host_prep_v5(x_re, x_im, phases, int8_shuffle, six_kinds=False):
    """Pack stationaries [P, B, K, 8, P] bf16 (K=4: ar, ai, br, bi; K=6 adds
    nai, nbi; int8 scales folded) and per-core x panels [P, 8, 2, TOK5]."""
    import ml_dtypes

    bf = ml_dtypes.bfloat16
    _USPLIT[0] = True
    try:
        Astat, Bstat = _stage_matrices(phases)
    finally:
        _USPLIT[0] = False
    s = YSCALE if int8_shuffle else 1.0
    Astat = Astat * np.float32(s)
    Bstat = Bstat * np.float32(1.0 / s)
    if six_kinds:
        # kinds: ar, ai, nai, br, bi, nbi
        mat = np.empty((P, MESH_BATCH, 6, 8, P), dtype=bf)
        mat[:, :, 0] = Astat.real.astype(bf).transpose(2, 0, 1, 3)
        mat[:, :, 1] = Astat.imag.astype(bf).transpose(2, 0, 1, 3)
        mat[:, :, 2] = (-Astat.imag).astype(bf).transpose(2, 0, 1, 3)
        mat[:, :, 3] = Bstat.real.astype(bf).transpose(2, 0, 1, 3)
        mat[:, :, 4] = Bstat.imag.astype(bf).transpose(2, 0, 1, 3)
        mat[:, :, 5] = (-Bstat.imag).astype(bf).transpose(2, 0, 1, 3)
    else:
        # mat[p, b, kind, r, m]
        mat = np.empty((P, MESH_BATCH, 4, 8, P), dtype=bf)
        mat[:, :, 0] = Astat.real.astype(bf).transpose(2, 0, 1, 3)
        mat[:, :, 1] = Astat.imag.astype(bf).transpose(2, 0, 1, 3)
        mat[:, :, 2] = Bstat.real.astype(bf).transpose(2, 0, 1, 3)
        mat[:, :, 3] = Bstat.imag.astype(bf).transpose(2, 0, 1, 3)
    mat = np.ascontiguousarray(mat)

    xts = []
    for c in range(N_CORES):
        t0 = c * TOK5
        panes = []
        for xp in (x_re, x_im):
            # [TOK5, L] -> [L, TOK5] -> [P, 8, TOK5]  (L-index = 8p + r)
            panes.append(xp[t0 : t0 + TOK5].T.reshape(P, 8, TOK5))
        xt = np.stack(panes, axis=2)  # [P, 8, 2, TOK5]
        xts.append(np.ascontiguousarray(xt).astype(bf))
    return mat, xts


def kernel(x_re: np.ndarray, x_im: np.ndarray, phases: np.ndarray) -> np.ndarray:
    global LAST_RESULTS

    x_re = np.ascontiguousarray(x_re, dtype=np.float32)
    x_im = np.ascontiguousarray(x_im, dtype=np.float32)
    phases = np.ascontiguousarray(phases, dtype=np.float32)

    if VERSION in (5, 6, 7, 8):
        six = VERSION in (6, 7)
        mat, xts = _host_prep_v5(x_re, x_im, phases, INT8_SHUFFLE and not six,
                                 six_kinds=six)
        key = (VERSION, INT8_SHUFFLE and not six)
        if key not in _CACHED:
            _CACHED[key] = (
                _build_program_v8() if VERSION == 8
                else _build_program_v7() if VERSION == 7
                else _build_program_v6() if six
                else _build_program_v5(INT8_SHUFFLE)
            )
        nc = _CACHED[key]
        in_maps = [{"xt": xts[c], "mat": mat} for c in range(N_CORES)]
        res = run_bass_kernel_spmd(nc, in_maps, list(range(N_CORES)), trace=TRACE)
        LAST_RESULTS = res
        jcols = _jcols()
        out = np.empty((MESH_BATCH, N_TOKENS, L), dtype=np.complex64)
        for c in range(N_CORES):
            t0 = c * TOK5
            sl = slice(t0, t0 + TOK5)
            arr = np.asarray(res.results[c]["out"], dtype=np.float32).reshape(
                MESH_BATCH, 2, P, 4, 2, TOK5
            )
            for t2 in range(8):
                q, t2p = divmod(t2, 4)
                for b in range(MESH_BATCH):
                    cplx = (
                        arr[b, q, :, t2p, 0, :] + 1j * arr[b, q, :, t2p, 1, :]
                    ).astype(np.complex64)  # [P, TOK5]
                    out[b, sl, jcols[t2]] = cplx
        return out

    half = N_TOKENS // 2
    in_maps = []
    if VERSION == 4:
        ar, ai, br, bi, xts = _host_prep_v4(x_re, x_im, phases)
        if 4 not in _CACHED:
            _CACHED[4] = _build_program_v4()
        nc = _CACHED[4]
        packed = []
        for b in range(MESH_BATCH):
            kinds = [ar[b], ai[b], -ai[b], br[b], bi[b], -bi[b]]
            m = np.concatenate([k.reshape(8, P, P) for k in kinds], axis=0)
            packed.append(np.ascontiguousarray(m.transpose(1, 0, 2).reshape(P, 48 * P)))
        for c in range(N_CORES):
            b, h = c // 2, c % 2
            in_maps.append({"xt": xts[h], "mat": packed[b]})
        res = run_bass_kernel_spmd(nc, in_maps, list(range(N_CORES)), trace=TRACE)
        LAST_RESULTS = res
        jcols = _jcols()
        out = np.empty((MESH_BATCH, N_TOKENS, L), dtype=np.complex64)
        for c in range(N_CORES):
            b, h = c // 2, c % 2
            # [NCH, 8, P, 2, TC] bf16 -> upcast once
            arr = np.asarray(res.results[c]["out"], dtype=np.float32).reshape(
                NCH, 8, P, 2, TC
            )
            sl = slice(h * half, (h + 1) * half)
            for t2 in range(8):
                cplx = (arr[:, t2, :, 0, :] + 1j * arr[:, t2, :, 1, :]).astype(
                    np.complex64
                )  # [NCH, P, TC]
                out[b, sl, jcols[t2]] = cplx.transpose(1, 0, 2).reshape(P, half)
        return out
    if VERSION == 2:
        W = _build_W(phases)                  # (B, L, L) complex64
        Wr = np.ascontiguousarray(W.real, dtype=np.float32)
        Wi = np.ascontiguousarray(W.imag, dtype=np.float32)
        if 2 not in _CACHED:
            _CACHED[2] = _build_program()
        nc = _CACHED[2]
        for c in range(N_CORES):
            b, h = c // 2, c % 2
            in_maps.append(
                {
                    "xr": x_re[h * half : (h + 1) * half],
                    "xi": x_im[h * half : (h + 1) * half],
                    "wr": Wr[b],
                    "wi": Wi[b],
                }
            )
    else:
        import ml_dtypes

        Astat, Bstat = _stage_matrices(phases)
        ar = np.ascontiguousarray(Astat.real.reshape(MESH_BATCH, 8 * P, P))
        ai = np.ascontiguousarray(Astat.imag.reshape(MESH_BATCH, 8 * P, P))
        br = Bstat.real.reshape(MESH_BATCH, 8 * P, P).astype(ml_dtypes.bfloat16)
        bi = Bstat.imag.reshape(MESH_BATCH, 8 * P, P).astype(ml_dtypes.bfloat16)
        if 3 not in _CACHED:
            _CACHED[3] = _build_program_v3()
        nc = _CACHED[3]
        for c in range(N_CORES):
            b, h = c // 2, c % 2
            in_maps.append(
                {
                    "xr": x_re[h * half : (h + 1) * half],
                    "xi": x_im[h * half : (h + 1) * half],
                    "ar": ar[b],
                    "ai": ai[b],
                    "nai": np.ascontiguousarray(-ai[b]),
                    "br": br[b],
                    "bi": bi[b],
                    "nbi": np.ascontiguousarray(-bi[b]),
                }
            )

    res = run_bass_kernel_spmd(nc, in_maps, list(range(N_CORES)), trace=TRACE)
    LAST_RESULTS = res

    out = np.empty((MESH_BATCH, N_TOKENS, L), dtype=np.complex64)
    for c in range(N_CORES):
        b, h = c // 2, c % 2
        out[b, h * half : (h + 1) * half] = (
            res.results[c]["out"].view(np.complex64).reshape(half, L)
        )
    return out



# revision 42
# speedup vs baseline: 1.0824x; 1.0824x over previous
"""Trainium2 Bass kernel for nn_BatchTrainableButterfly.

The reference applies, per mesh-batch b, a trainable butterfly network
(10 levels of phase shifters + 2x2 directional couplers with butterfly
permutations, plus a final phase layer and bit-reversals) to every token
row x[n, :].  For fixed phases the whole network is a linear map on
C^1024; it factors into two 128-wide PE stages (A = bitrev + levels 0..6,
block-diagonal over 8 column groups; B = levels 7..9 + final phase +
bitrev, an 8x8 mix across groups), which is 4x fewer MACs than the dense
1024x1024 matmul.

Active VERSION=5 (token-sharded): each core owns N_TOKENS/8 = 512 tokens
for ALL 4 mesh batches, which minimizes SBUF-AXI-port traffic — the
measured bottleneck (x 2 MiB + stationaries 4 MiB + inter-stage shuffle
8 MiB (counted twice: SBUF->SBUF reads AND writes cross the same 16 AXI
ports at ~435 GB/s) + out 8 MiB ~= 30 port-MiB ~= 72 us floor).  All
transposes live on the host: x arrives pre-transposed/r-grouped bf16;
out leaves position-major bf16 and the host inverts the permutation.
Per batch: 8 A-groups (4 bf16 matmuls N=512 each, fp32 PSUM pairs,
vector/scalar evacuation), a stepped-partition SBUF shuffle split into
per-(g, t2-half) DMAs (finer stage-B dependencies; stage-A's output row
order u*64+s*4+t2' makes the halves contiguous 64-partition slices),
then 8 B-groups with paired output stores (4 KiB dram lines).  Software
pipeline A0 A1 B0 A2 B1 A3 B2 B3 balances PE phases against the
port-saturated shuffle/store traffic; stationary negations (-ai, -bi)
are derived on device off the critical path.  ~98 us vs 113-118 us for
the v4 batch-sharded baseline; rel err 3.4e-3 (bf16).

An int8 shuffle variant (INT8_SHUFFLE) passes accuracy (1.1e-2) but is
slower: DVE/ACT 8-bit output casts run at half rate and gpsimd int8
upcasts at ~3 ns/elem, swamping the DMA savings.
"""

import math

import numpy as np

import concourse.tile as tile
from concourse import bacc, bass, mybir
from concourse.bass_utils import run_bass_kernel_spmd
from concourse.masks import make_identity

P = 128          # partitions
L = 1024         # butterfly length
N_TOKENS = 4096
MESH_BATCH = 4
N_CORES = 8
T = (N_TOKENS * MESH_BATCH) // N_CORES  # 2048 token-rows per core
NT = T // P      # 16 token tiles per core
KC = L // P      # 8 contraction chunks
NLEV = int(math.log2(L))  # 10

F32 = mybir.dt.float32
F32R = mybir.dt.float32r
BF16 = mybir.dt.bfloat16

TC = 512          # tokens per pipeline chunk (v3)
NCH = T // TC     # 4 chunks

I8 = mybir.dt.int8

# v5: token sharding — each core owns TOK5 tokens for ALL 4 mesh batches.
TOK5 = N_TOKENS // N_CORES   # 512 tokens per core
YSCALE = 127.0 / (4.75 * 11.3137)  # int8 shuffle: 127 / (4.75 sigma_y)

TRACE = False
LAST_RESULTS = None
VERSION = 5       # active: token-sharded two-stage butterfly (see module docstring)
INT8_SHUFFLE = False

# ----------------------------------------------------------------------
# Host side: build the per-batch transfer matrices from the phases.
# ----------------------------------------------------------------------


def _bitrev(n):
    m = int(math.log2(n))
    perm = np.arange(n).reshape(n, 1)
    for _ in range(m):
        n1 = perm.shape[0] // 2
        perm = np.hstack((perm[:n1], perm[n1:]))
    return perm.squeeze(0)


def _forward_indices(length):
    idx = []
    ar = np.arange(length)
    for level in range(int(math.log2(length)) - 1):
        bs = 2 ** (level + 2)
        ind = ar.reshape(-1, length // bs, 2, bs // 2).transpose(0, 1, 3, 2)
        idx.append(ind.reshape(-1))
    return idx


def _build_W(phases):
    """phases (B, NLEV+1, L//2, 2) -> W (B, L, L) complex64 with out = x @ W."""
    B = phases.shape[0]
    br = _bitrev(L)
    fidx = _forward_indices(L)
    dc = np.array([[1.0, 1.0j], [1.0j, 1.0]], dtype=np.complex64)

    x = np.broadcast_to(np.eye(L, dtype=np.complex64), (B, L, L)).copy()
    x = x[..., br]
    for level in range(NLEV):
        x = x.reshape(B, L, L // 2, 2)
        ph = phases[:, level : level + 1, :, :]            # (B, 1, L//2, 2)
        x = x * np.exp(1j * ph.astype(np.complex64))
        x = x @ dc
        x = x.reshape(B, L, L)
        if level < NLEV - 1:
            x = x[..., fidx[level]]
    ph = phases[:, NLEV - 1 : NLEV, :, :].reshape(B, 1, L)
    x = x * np.exp(1j * ph.astype(np.complex64))
    x = x[..., br]
    return (x / np.float32(np.sqrt(L))).astype(np.complex64)


def _rev(v, n):
    r = 0
    for _ in range(n):
        r = (r << 1) | (v & 1)
        v >>= 1
    return r


def _stage_matrices(phases):
    """Two-stage factorization of the butterfly network.

    Stage A = input bitrev + levels 0..6 (perms 0..5, no trailing perm):
    block-diagonal; column-block g is fed by x columns {i : i = 8p + r},
    r = rev3(g).  Stage B = perm fidx[6] + levels 7..9 + final phase +
    final bitrev + scale: per-position 8x8 mixing across the 8 blocks.

    Returns per batch the PE stationaries:
      Astat[b, r] (128,128) cplx : lhsT with K=p (x idx 8p+r), M=pos.
      Bstat[b,t2] (128,128) cplx : lhsT with K = g*16+s (source y(g, t2*16+s)),
                                   M = v*8+m -> out col j = 128m + 8v + rev3(t2).
    Cross-component entries of the extracted B submatrix are exactly 0.
    """
    B_ = phases.shape[0]
    br = _bitrev(L)
    fidx = _forward_indices(L)
    dc = np.array([[1.0, 1.0j], [1.0j, 1.0]], dtype=np.complex64)

    def levels(x, lo, hi, pre_br=False, post_final=False, pre_perm=None):
        if pre_br:
            x = x[..., br]
        if pre_perm is not None:
            x = x[..., pre_perm]
        for level in range(lo, hi):
            x = x.reshape(B_, L, L // 2, 2)
            x = x * np.exp(1j * phases[:, level, None, :, :].astype(np.complex64))
            x = x @ dc
            x = x.reshape(B_, L, L)
            if level < NLEV - 1 and level != 6:
                x = x[..., fidx[level]]
        if post_final:
            x = x * np.exp(
                1j * phases[:, NLEV - 1, None, :, :].reshape(B_, 1, L).astype(np.complex64)
            )
            x = x[..., br]
            x = x / np.float32(np.sqrt(L))
        return x

    eye = np.broadcast_to(np.eye(L, dtype=np.complex64), (B_, L, L)).copy()
    A = levels(eye.copy(), 0, 7, pre_br=True)
    Bm = levels(eye.copy(), 7, NLEV, post_final=True, pre_perm=fidx[6])

    # Stage-A output row order: row' = s*8 + t2 for pos p'' = t2*16 + s, so the
    # inter-stage shuffle is one plain DMA per g: yA_g[:] -> Bin[g:128:8,:,:]
    # (dst partition k = s*8 + g, free = (t2, tok)).
    ar_ = np.arange(P)
    if _USPLIT[0]:
        # row' = u*64 + s*4 + t2' with t2 = u*4 + t2': the shuffle splits
        # into per-u 64-partition DMAs (finer B dependencies, same runs).
        u_ = ar_ >> 6
        s2 = (ar_ & 63) >> 2
        t2p = ar_ & 3
        posperm = (u_ * 4 + t2p) * 16 + s2         # row' -> p''
    else:
        posperm = (ar_ & 7) * 16 + (ar_ >> 3)      # row' -> p''
    Astat = np.empty((B_, 8, P, P), dtype=np.complex64)
    for r in range(8):
        g = _rev(r, 3)
        Astat[:, r] = A[:, ar_ * 8 + r][:, :, g * P + posperm]

    s_, g_ = np.divmod(ar_, 8)                     # k = s*8 + g
    v_, m_ = np.divmod(ar_, 8)
    Bstat = np.empty((B_, 8, P, P), dtype=np.complex64)
    for t2 in range(8):
        rows = g_ * P + t2 * 16 + s_
        cols = P * m_ + 8 * v_ + _rev(t2, 3)
        Bstat[:, t2] = Bm[:, rows][:, :, cols]
    return Astat, Bstat


# ----------------------------------------------------------------------
# Device side: complex matmul kernel (SPMD, one (batch, half) per core).
# ----------------------------------------------------------------------

_USPLIT = [False]

_CACHED_NC = None


def _build_program():
    nc = bacc.Bacc(
        "TRN2", target_bir_lowering=False, debug=False, num_devices=N_CORES
    )

    xr_d = nc.declare_dram_parameter("xr", [T, L], F32, isOutput=False)
    xi_d = nc.declare_dram_parameter("xi", [T, L], F32, isOutput=False)
    wr_d = nc.declare_dram_parameter("wr", [L, L], F32R, isOutput=False)
    wi_d = nc.declare_dram_parameter("wi", [L, L], F32R, isOutput=False)
    out_d = nc.declare_dram_parameter("out", [T, 2 * L], F32, isOutput=True)

    with tile.TileContext(nc) as tc:
        with (
            tc.tile_pool(name="const", bufs=1) as const_pool,
            tc.tile_pool(name="w", bufs=1) as w_pool,
            tc.tile_pool(name="x", bufs=3) as x_pool,
            tc.tile_pool(name="xt", bufs=2) as xt_pool,
            tc.tile_pool(name="osb", bufs=3) as o_pool,
            tc.tile_pool(name="ps", bufs=8, space=bass.MemorySpace.PSUM) as ps_pool,
        ):
            ident = const_pool.tile([P, P], F32)
            make_identity(nc, ident[:])

            # Warm the PE HAM while W streams in: dummy transposes keep the
            # tensor engine busy >3.4us so it reaches full clock before the
            # real matmuls start.
            warm = ps_pool.tile([P, 4 * P], F32, tag="ps")
            for _ in range(12):
                for j in range(4):
                    nc.tensor.transpose(
                        warm[:, j * P : (j + 1) * P], ident[:], ident[:]
                    )

            # Stream W into SBUF once: per k-chunk tiles (P x L), natural layout
            # (partition = contraction row within chunk, free = output column).
            # k-major order so the first token tile's accumulation can start
            # after only a few chunks have landed.
            w_sb = {}
            for k in range(KC):
                for nm, dram in (("wr", wr_d), ("wi", wi_d)):
                    t_ = w_pool.tile([P, L], F32R, tag=f"{nm}{k}")
                    nc.sync.dma_start(out=t_[:], in_=dram[k * P : (k + 1) * P, :])
                    w_sb[nm, k] = t_
                # -Wi derived on device: saves a third of the W stream, which
                # gates the kernel head while PE waits on weights.
                nwi = w_pool.tile([P, L], F32R, tag=f"nwi{k}")
                nc.vector.tensor_scalar_mul(nwi[:], w_sb["wi", k][:], -1.0)
                w_sb["nwi", k] = nwi

            for t in range(NT):
                rows = slice(t * P, (t + 1) * P)
                xr_rows = x_pool.tile([P, L], F32, tag="xr_rows")
                xi_rows = x_pool.tile([P, L], F32, tag="xi_rows")
                nc.sync.dma_start(out=xr_rows[:], in_=xr_d[rows, :])
                nc.sync.dma_start(out=xi_rows[:], in_=xi_d[rows, :])

                # Transpose the token tile: xT chunks live at
                # xT[:, k*P:(k+1)*P] = x_rows[:, k*P:(k+1)*P].T
                xrT = xt_pool.tile([P, L], F32R, tag="xrT")
                xiT = xt_pool.tile([P, L], F32R, tag="xiT")
                for src, dst in ((xr_rows, xrT), (xi_rows, xiT)):
                    for g in range(2):
                        tp = ps_pool.tile([P, 4 * P], F32, tag="ps")
                        for j in range(4):
                            k = g * 4 + j
                            nc.tensor.transpose(
                                tp[:, j * P : (j + 1) * P],
                                src[:, k * P : (k + 1) * P],
                                ident[:],
                            )
                        nc.scalar.copy(dst[:, g * 4 * P : (g + 1) * 4 * P], tp[:])

                # Accumulate the four real matmul outputs.
                #   re_n = sum_k xrT_k @ wr_k[n] + xiT_k @ nwi_k[n]
                #   im_n = sum_k xrT_k @ wi_k[n] + xiT_k @ wr_k[n]
                out_sb = o_pool.tile([P, L, 2], F32, tag="out_sb")
                for n in range(2):
                    ncol = slice(n * 512, (n + 1) * 512)
                    acc_re = ps_pool.tile([P, 512], F32, tag="ps")
                    acc_im = ps_pool.tile([P, 512], F32, tag="ps")
                    for k in range(KC):
                        xrT_k = xrT[:, k * P : (k + 1) * P]
                        xiT_k = xiT[:, k * P : (k + 1) * P]
                        first = k == 0
                        last = k == KC - 1
                        nc.tensor.matmul(
                            acc_re[:], xrT_k, w_sb["wr", k][:, ncol],
                            start=first, stop=False,
                        )
                        nc.tensor.matmul(
                            acc_re[:], xiT_k, w_sb["nwi", k][:, ncol],
                            start=False, stop=last,
                        )
                        nc.tensor.matmul(
                            acc_im[:], xrT_k, w_sb["wi", k][:, ncol],
                            start=first, stop=False,
                        )
                        nc.tensor.matmul(
                            acc_im[:], xiT_k, w_sb["wr", k][:, ncol],
                            start=False, stop=last,
                        )
                    # Interleave re/im into complex64 memory order.
                    nc.vector.tensor_copy(out_sb[:, n * 512 : (n + 1) * 512, 0], acc_re[:])
                    nc.vector.tensor_copy(out_sb[:, n * 512 : (n + 1) * 512, 1], acc_im[:])

                nc.sync.dma_start(out=out_d[rows, :], in_=out_sb[:])

    nc.compile()
    return nc


def _build_program_v3():
    # detect_race_conditions=False: the rust race detector false-positives on
    # the stepped-partition shuffle DMA vs writes to a *different* bin buffer
    # (disjoint SBUF regions sharing a shadow zone). Same-tensor deps are
    # tracked normally and validated by the CoreSim numeric check.
    nc = bacc.Bacc(
        "TRN2", target_bir_lowering=False, debug=False, num_devices=N_CORES,
        detect_race_conditions=False,
    )

    xr_d = nc.declare_dram_parameter("xr", [T, L], F32R, isOutput=False)
    xi_d = nc.declare_dram_parameter("xi", [T, L], F32R, isOutput=False)
    ar_d = nc.declare_dram_parameter("ar", [8 * P, P], F32R, isOutput=False)
    ai_d = nc.declare_dram_parameter("ai", [8 * P, P], F32R, isOutput=False)
    nai_d = nc.declare_dram_parameter("nai", [8 * P, P], F32R, isOutput=False)
    br_d = nc.declare_dram_parameter("br", [8 * P, P], BF16, isOutput=False)
    bi_d = nc.declare_dram_parameter("bi", [8 * P, P], BF16, isOutput=False)
    nbi_d = nc.declare_dram_parameter("nbi", [8 * P, P], BF16, isOutput=False)
    out_d = nc.declare_dram_parameter("out", [T, 2 * L], F32, isOutput=True)

    with tile.TileContext(nc) as tc:
        with (
            tc.tile_pool(name="const", bufs=1) as const_pool,
            tc.tile_pool(name="mats", bufs=1) as mat_pool,
            tc.tile_pool(name="x", bufs=8) as x_pool,
            tc.tile_pool(name="xt", bufs=20) as xt_pool,
            tc.tile_pool(name="ya", bufs=12) as ya_pool,
            tc.tile_pool(name="bin", bufs=1) as bin_pool,
            tc.tile_pool(name="yb", bufs=4) as yb_pool,
            tc.tile_pool(name="osb", bufs=4) as o_pool,
            tc.tile_pool(name="ps", bufs=8, space=bass.MemorySpace.PSUM) as ps_pool,
        ):
            ident = const_pool.tile([P, P], F32)
            make_identity(nc, ident[:])
            ident_h = const_pool.tile([P, P], BF16)
            nc.vector.tensor_copy(ident_h[:], ident[:])
            ident_r = const_pool.tile([P, P], F32R)
            nc.vector.tensor_copy(ident_r[:], ident[:])

            # HAM warmup while the (small) stationaries stream in.
            warm = ps_pool.tile([P, 4 * P], F32, tag="ps")
            for _ in range(22):
                for j in range(4):
                    nc.tensor.transpose(
                        warm[:, j * P : (j + 1) * P], ident[:], ident[:]
                    )

            # Persistent double-buffered shuffle destination; memset once so
            # downstream readers of the stepped-partition DMA writes are
            # observable (sim init tracking) — overlaps with warmup/mats DMA.
            bn_bufs = []
            bn_memsets = []
            for i in range(2):
                bnb = bin_pool.tile([P, 8, 2 * TC], BF16, tag=f"bin{i}")
                bn_memsets.append(nc.gpsimd.memset(bnb[:], 0.0))
                bn_bufs.append(bnb)

            # Mats go through the gpsimd SWDGE queues so the 48 dma_starts do
            # not serialize ahead of chunk-0 row loads on the two HWDGE queues.
            mats = {}
            for nm, dram, dt_ in (
                ("ar", ar_d, F32R), ("ai", ai_d, F32R), ("nai", nai_d, F32R),
                ("br", br_d, BF16), ("bi", bi_d, BF16), ("nbi", nbi_d, BF16),
            ):
                for r in range(8):
                    t_ = mat_pool.tile([P, P], dt_, tag=f"{nm}{r}")
                    nc.gpsimd.dma_start(out=t_[:], in_=dram[r * P : (r + 1) * P, :])
                    mats[nm, r] = t_

            def emit_front(ch):
                """T_in + stage A + shuffle for chunk ch."""
                tok0 = ch * TC
                rows = {}
                for pl, dram in ((0, xr_d), (1, xi_d)):
                    for tt in range(TC // P):
                        rt = x_pool.tile([P, P, 8], F32R, tag="rows")
                        r0 = tok0 + tt * P
                        eng = nc.scalar if (tt % 2) else nc.sync
                        eng.dma_start(out=rt[:], in_=dram[r0 : r0 + P, :])
                        rows[pl, tt] = rt

                xT = {}
                for pl in range(2):
                    for r in range(8):
                        tp = ps_pool.tile([P, 4 * P], F32R, tag="ps")
                        for tt in range(TC // P):
                            nc.tensor.transpose(
                                tp[:, tt * P : (tt + 1) * P],
                                rows[pl, tt][:, :, r],
                                ident_r[:],
                            )
                        dst = xt_pool.tile([P, TC], F32R, tag="xT")
                        nc.scalar.copy(dst[:], tp[:])
                        xT[pl, r] = dst

                yA = {}
                for r in range(8):
                    g = _rev(r, 3)
                    acr = ps_pool.tile([P, TC], F32, tag="ps")
                    aci = ps_pool.tile([P, TC], F32, tag="ps")
                    nc.tensor.matmul(acr[:], mats["ar", r], xT[0, r][:], start=True, stop=False)
                    nc.tensor.matmul(acr[:], mats["nai", r], xT[1, r][:], start=False, stop=True)
                    nc.tensor.matmul(aci[:], mats["ai", r], xT[0, r][:], start=True, stop=False)
                    nc.tensor.matmul(aci[:], mats["ar", r], xT[1, r][:], start=False, stop=True)
                    ya = ya_pool.tile([P, 2 * TC], BF16, tag="ya")
                    nc.vector.tensor_copy(ya[:, 0:TC], acr[:])
                    nc.vector.tensor_copy(ya[:, TC : 2 * TC], aci[:])
                    yA[g] = ya

                # shuffle: Bin[s*8+g, t2, :] = yA[g][s*8+t2, :] — one plain DMA
                # per g; one partition per SBUF port group on both sides.
                bn = bn_bufs[ch % 2]
                for g in range(8):
                    eng = nc.scalar if (g % 2) else nc.sync
                    eng.dma_start(out=bn[g:P:8, :, :], in_=yA[g][:])
                return bn

            def emit_back(ch, bn):
                """Stage B + T_out + interleave + store for chunk ch."""
                tok0 = ch * TC
                out_sb = []
                for tt in range(TC // P):
                    osb = o_pool.tile([P, 2 * L], F32, tag="osb")
                    out_sb.append(osb)
                for t2 in range(8):
                    obr = ps_pool.tile([P, TC], F32, tag="ps")
                    obi = ps_pool.tile([P, TC], F32, tag="ps")
                    b_re = bn[:, t2, 0:TC]
                    b_im = bn[:, t2, TC : 2 * TC]
                    nc.tensor.matmul(obr[:], mats["br", t2], b_re, start=True, stop=False)
                    nc.tensor.matmul(obr[:], mats["nbi", t2], b_im, start=False, stop=True)
                    nc.tensor.matmul(obi[:], mats["bi", t2], b_re, start=True, stop=False)
                    nc.tensor.matmul(obi[:], mats["br", t2], b_im, start=False, stop=True)
                    yb = yb_pool.tile([P, 2 * TC], BF16, tag="yb")
                    nc.scalar.copy(yb[:, 0:TC], obr[:])
                    nc.scalar.copy(yb[:, TC:], obi[:])

                    base = 2 * _rev(t2, 3)
                    for tt in range(TC // P):
                        tp2 = ps_pool.tile([P, 2, 16, 8], BF16, tag="ps")
                        nc.tensor.transpose(
                            tp2[:, 0], yb[:, tt * P : (tt + 1) * P], ident_h[:]
                        )
                        nc.tensor.transpose(
                            tp2[:, 1], yb[:, TC + tt * P : TC + (tt + 1) * P], ident_h[:]
                        )
                        osr = out_sb[tt][:].rearrange(
                            "q (m v lo) -> q lo v m", m=8, v=16, lo=16
                        )
                        nc.vector.tensor_copy(osr[:, base : base + 2, :, :], tp2[:])

                for tt in range(TC // P):
                    r0 = tok0 + tt * P
                    eng = nc.scalar if (tt % 2) else nc.sync
                    eng.dma_start(out=out_d[r0 : r0 + P, :], in_=out_sb[tt][:])

            # Software pipeline: back-half of chunk ch-1 is emitted after the
            # front-half (and shuffle issue) of chunk ch, so the PE stream has
            # B/T_out work in hand while chunk ch's shuffle is in flight.
            prev = None
            for ch in range(NCH):
                bn = emit_front(ch)
                if prev is not None:
                    emit_back(prev[0], prev[1])
                prev = (ch, bn)
            emit_back(prev[0], prev[1])

    nc.compile()
    return nc


def _build_program_v4():
    """Two-stage butterfly with all transposes moved to the host.

    x arrives pre-transposed and r-grouped in HBM as bf16 rows
    (plane, r, p) x tok, so stage-A moving operands are plain contiguous
    loads.  Stage A: acc[row', tok] = A_r^T x_r with the A/B stage
    matrices stationary; the stepped-partition SBUF shuffle regroups
    (s,t2) -> (s,g) partitions for stage B; stage-B results [j', tok]
    are stored position-major and the host undoes the butterfly output
    permutation + transpose.  No PE transposes, no output interleave.
    """
    nc = bacc.Bacc(
        "TRN2", target_bir_lowering=False, debug=False, num_devices=N_CORES,
        detect_race_conditions=False,
    )

    xt_d = nc.declare_dram_parameter("xt", [16 * P, T], BF16, isOutput=False)
    # All 48 stationaries packed: [P, (6 kinds x 8 r) * P] so one DMA with
    # 12 KiB partition lines loads everything (48 separate [P,P] DMAs would
    # be 256 B/line, descriptor-overhead-bound).
    mat_d = nc.declare_dram_parameter("mat", [P, 48 * P], BF16, isOutput=False)
    # bf16 output, chunk-major: row (ch, t2, j') holds [re TC | im TC] so
    # every store writes full contiguous 2 KiB dram rows.  Host upcasts.
    out_d = nc.declare_dram_parameter("out", [NCH * 8 * P, 2 * TC], BF16, isOutput=True)

    with tile.TileContext(nc) as tc:
        with (
            tc.tile_pool(name="const", bufs=1) as const_pool,
            tc.tile_pool(name="mats", bufs=1) as mat_pool,
            tc.tile_pool(name="x", bufs=1) as x_pool,
            tc.tile_pool(name="ya", bufs=3) as ya_pool,
            tc.tile_pool(name="bin", bufs=1) as bin_pool,
            tc.tile_pool(name="osb", bufs=1) as o_pool,
            tc.tile_pool(name="ps", bufs=4, space=bass.MemorySpace.PSUM) as ps_pool,
        ):
            ident = const_pool.tile([P, P], F32)
            make_identity(nc, ident[:])

            # Short HAM warmup (~4.7us cold) covering the chunk-0 DMA window.
            warm = ps_pool.tile([P, 2, TC], F32, tag="ps")
            for _ in range(11):
                for j in range(4):
                    nc.tensor.transpose(
                        warm[:, j // 2, (j % 2) * P : (j % 2 + 1) * P],
                        ident[:], ident[:],
                    )

            # One shuffle destination per chunk: all fronts are emitted before
            # any back, so no buffer reuse hazards at all.  The stepped-
            # partition shuffle DMAs fully overwrite each buffer; no init
            # needed (race detection is disabled).
            bn_bufs = []
            for i in range(NCH):
                bnb = bin_pool.tile([P, 8, 2, TC], BF16, tag=f"bin{i}")
                bn_bufs.append(bnb)

            # Packed stationaries on the gpsimd SWDGE queue: bulk load that
            # must not block the latency-critical HWDGE queues.
            mat_all = mat_pool.tile([P, 48 * P], BF16, tag="mat")
            nc.gpsimd.dma_start(out=mat_all[:], in_=mat_d[:, :])
            mats = {}
            for ki, nm in enumerate(("ar", "ai", "nai", "br", "bi", "nbi")):
                for r in range(8):
                    idx = ki * 8 + r
                    mats[nm, r] = mat_all[:, idx * P : (idx + 1) * P]

            # Full x panel upfront on the HWDGE queues (empty at the head):
            # 16 DMAs with 4 KiB partition lines; r=0's tiles land first so
            # stage A starts after ~1.5us.  Resident all run (64 KiB/part).
            xs = {}
            for r in range(8):
                for pl in range(2):
                    xtile = x_pool.tile([P, T], BF16, tag=f"x{pl}_{r}")
                    row0 = (pl * 8 + r) * P
                    eng = nc.scalar if (pl % 2) else nc.sync
                    eng.dma_start(out=xtile[:], in_=xt_d[row0 : row0 + P, :])
                    xs[pl, r] = xtile

            def emit_front(ch):
                """Stage A + cast + shuffle for chunk ch."""
                csl = slice(ch * TC, (ch + 1) * TC)
                bn = bn_bufs[ch]
                for r in range(8):
                    g = _rev(r, 3)
                    acc = ps_pool.tile([P, 2, TC], F32, tag="ps")
                    acr = acc[:, 0, :]
                    aci = acc[:, 1, :]
                    nc.tensor.matmul(acr, mats["ar", r], xs[0, r][:, csl], start=True, stop=False)
                    nc.tensor.matmul(aci, mats["ar", r], xs[1, r][:, csl], start=True, stop=False)
                    nc.tensor.matmul(aci, mats["ai", r], xs[0, r][:, csl], start=False, stop=True)
                    nc.tensor.matmul(acr, mats["nai", r], xs[1, r][:, csl], start=False, stop=True)
                    ya = ya_pool.tile([P, 2, TC], BF16, tag=f"ya{g}")
                    if r % 2:
                        nc.scalar.copy(ya[:], acc[:])
                    else:
                        nc.vector.tensor_copy(ya[:], acc[:])
                    # Shuffle: bn[s*8+g, t2, c, :] = ya[s*8+t2, c, :]
                    eng2 = nc.scalar if (g % 2) else nc.sync
                    eng2.dma_start(out=bn[g:P:8, :, :, :], in_=ya[:])
                return bn

            def emit_back(ch, bn):
                """Stage B + store (position-major, bf16) for chunk ch."""
                for t2 in range(8):
                    ob = ps_pool.tile([P, 2, TC], F32, tag="ps")
                    obr = ob[:, 0, :]
                    obi = ob[:, 1, :]
                    b_re = bn[:, t2, 0, :]
                    b_im = bn[:, t2, 1, :]
                    nc.tensor.matmul(obr, mats["br", t2], b_re, start=True, stop=False)
                    nc.tensor.matmul(obi, mats["br", t2], b_im, start=True, stop=False)
                    nc.tensor.matmul(obi, mats["bi", t2], b_re, start=False, stop=True)
                    nc.tensor.matmul(obr, mats["nbi", t2], b_im, start=False, stop=True)
                    osb = o_pool.tile([P, 2, TC], BF16, tag=f"osb{t2}")
                    if t2 % 2:
                        nc.scalar.copy(osb[:], ob[:])
                    else:
                        nc.vector.tensor_copy(osb[:], ob[:])
                    row0 = (ch * 8 + t2) * P
                    eng = nc.sync if (t2 % 2) else nc.scalar
                    eng.dma_start(out=out_d[row0 : row0 + P, :], in_=osb[:])

            # 2-chunk lookahead: B(ch) is emitted two fronts after F(ch), so
            # its shuffle has ~2 chunks of port time to land before the PE
            # reaches it.
            bns = {}
            order = []
            for ch in range(NCH):
                order.append(("F", ch))
                if ch >= 2:
                    order.append(("B", ch - 2))
            order += [("B", NCH - 2), ("B", NCH - 1)]
            for kind, ch in order:
                if kind == "F":
                    bns[ch] = emit_front(ch)
                else:
                    emit_back(ch, bns[ch])

    nc.compile()
    return nc


def _build_program_v5(int8_shuffle: bool):
    """Token-sharded two-stage butterfly: each core runs TOK5 tokens through
    all 4 mesh-batches.  Cuts the x input to 2 MiB/core (vs 8) at the cost of
    4 MiB of stationaries, minimizing SBUF-AXI-port traffic (the measured
    bottleneck).  Optional int8 inter-stage shuffle halves the port cost of
    the partition-regroup DMA (scales folded into the stage matrices on host;
    clamped vector casts; int8->bf16 upcast split across engines)."""
    nc = bacc.Bacc(
        "TRN2", target_bir_lowering=False, debug=False, num_devices=N_CORES,
        detect_race_conditions=False,
    )
    B = MESH_BATCH
    TCv = TOK5  # 512 tokens = one chunk per batch

    xt_d = nc.declare_dram_parameter("xt", [P, 8, 2, TCv], BF16, isOutput=False)
    mat_d = nc.declare_dram_parameter("mat", [P, B, 4, 8, P], BF16, isOutput=False)
    # out rows (b*2+q)*P + j', free [t2' in 4, c in 2, TCv]; t2 = q*4+t2'.
    out_d = nc.declare_dram_parameter("out", [B * 2 * P, 4 * 2 * TCv], BF16,
                                      isOutput=True)

    ydt = I8 if int8_shuffle else BF16

    with tile.TileContext(nc) as tc:
        with (
            tc.tile_pool(name="const", bufs=1) as const_pool,
            tc.tile_pool(name="mats", bufs=1) as mat_pool,
            tc.tile_pool(name="x", bufs=1) as x_pool,
            tc.tile_pool(name="ya", bufs=10) as ya_pool,
            tc.tile_pool(name="bin", bufs=1) as bin_pool,
            tc.tile_pool(name="bnh", bufs=2) as bnh_pool,
            tc.tile_pool(name="osb", bufs=3) as o_pool,
            tc.tile_pool(name="ps", bufs=4, space=bass.MemorySpace.PSUM) as ps_pool,
        ):
            ident = const_pool.tile([P, P], F32)
            make_identity(nc, ident[:])

            # HAM warmup covering the head DMA window (~9us of PE activity).
            warm = ps_pool.tile([P, 2, TCv], F32, tag="ps")
            for _ in range(10):
                for j in range(4):
                    nc.tensor.transpose(
                        warm[:, j // 2, (j % 2) * P : (j % 2 + 1) * P],
                        ident[:], ident[:],
                    )

            # Stationaries per batch (kinds ar, ai, br, bi) + derived
            # negations.  mat0 goes FIRST on the sync ring (it gates A0 and
            # the ring is otherwise empty, so it lands in ~2.5us); the x
            # panel follows on sync; mats 1-3 stream on the gpsimd ring.
            matb, negb = [], []
            for b in range(B):
                mb = mat_pool.tile([P, 4, 8, P], BF16, tag=f"mat{b}",
                                   name=f"mat{b}")
                ng = mat_pool.tile([P, 2, 8, P], BF16, tag=f"neg{b}",
                                   name=f"neg{b}")
                matb.append(mb)
                negb.append(ng)

            # Head loads: x split across the sync and scalar rings, the
            # stationaries FIFO-ordered (batch 0 first) on the gpsimd ring.
            # Measured best; variants that serialize x on one ring or put
            # mats on the HWDGE rings delay the first shuffles and lose
            # 5-10us.
            xs = x_pool.tile([P, 8, 2, TCv], BF16, tag="xs")
            nc.sync.dma_start(out=xs[:, 0:4], in_=xt_d[:, 0:4])
            nc.scalar.dma_start(out=xs[:, 4:8], in_=xt_d[:, 4:8])
            for b in range(B):
                nc.gpsimd.dma_start(out=matb[b][:], in_=mat_d[:, b])
            negs_done = [False] * B

            bn = []
            for b in range(B):
                bnb = bin_pool.tile([P, 8, 2, TCv], ydt, tag=f"bin{b}")
                bn.append(bnb)

            def emit_A(b):
                mb, ng = matb[b], negb[b]
                if not negs_done[b]:
                    # ng[:,0] = -ai here; -bi is deferred to emit_B so each
                    # negation only costs the scalar FIFO ~1us per phase.
                    nc.scalar.mul(ng[:, 0], mb[:, 1], -1.0)
                    negs_done[b] = True
                for r in range(8):
                    g = _rev(r, 3)
                    acc = ps_pool.tile([P, 2, TCv], F32, tag="ps")
                    ar_ = mb[:, 0, r, :]
                    ai_ = mb[:, 1, r, :]
                    nai = ng[:, 0, r, :]
                    xr_ = xs[:, r, 0, :]
                    xi_ = xs[:, r, 1, :]
                    nc.tensor.matmul(acc[:, 0, :], ar_, xr_, start=True, stop=False)
                    nc.tensor.matmul(acc[:, 0, :], nai, xi_, start=False, stop=True)
                    nc.tensor.matmul(acc[:, 1, :], ai_, xr_, start=True, stop=False)
                    nc.tensor.matmul(acc[:, 1, :], ar_, xi_, start=False, stop=True)
                    ya = ya_pool.tile([P, 2, TCv], ydt, tag="ya")
                    if r in (1, 3, 5):
                        nc.scalar.copy(ya[:], acc[:])
                    else:
                        nc.vector.tensor_copy(ya[:], acc[:])
                    # per-u half shuffles: B(b) t2<4 waits only on the u=0
                    # halves; same 2 KiB descriptor runs.
                    e_lo = nc.scalar if (g % 2) else nc.sync
                    e_hi = nc.sync if (g % 2) else nc.scalar
                    e_lo.dma_start(out=bn[b][g:P:8, 0:4], in_=ya[0:64])
                    e_hi.dma_start(out=bn[b][g:P:8, 4:8], in_=ya[64:128])

            def emit_casts(b):
                """int8 bn -> bf16 for the stage-B moving operand."""
                bh = bnh_pool.tile([P, 8, 2, TCv], BF16, tag=f"bnh{b % 2}")
                nc.gpsimd.tensor_copy(bh[:, 0:3], bn[b][:, 0:3])
                nc.vector.tensor_copy(bh[:, 3:5], bn[b][:, 3:5])
                nc.gpsimd.tensor_copy(bh[:, 5:7], bn[b][:, 5:7])
                nc.scalar.copy(bh[:, 7:8], bn[b][:, 7:8])
                return bh

            def emit_B(b, bh):
                mb, ng = matb[b], negb[b]
                # ng[:,1] = -bi (see emit_A)
                nc.scalar.mul(ng[:, 1], mb[:, 3], -1.0)
                src = bh if bh is not None else bn[b]
                osb = None
                for t2 in range(8):
                    acc = ps_pool.tile([P, 2, TCv], F32, tag="ps")
                    br_ = mb[:, 2, t2, :]
                    bi_ = mb[:, 3, t2, :]
                    nbi = ng[:, 1, t2, :]
                    yre = src[:, t2, 0, :]
                    yim = src[:, t2, 1, :]
                    nc.tensor.matmul(acc[:, 0, :], br_, yre, start=True, stop=False)
                    nc.tensor.matmul(acc[:, 0, :], nbi, yim, start=False, stop=True)
                    nc.tensor.matmul(acc[:, 1, :], bi_, yre, start=True, stop=False)
                    nc.tensor.matmul(acc[:, 1, :], br_, yim, start=False, stop=True)
                    if t2 % 2 == 0:
                        osb = o_pool.tile([P, 2, 2, TCv], BF16, tag="osb")
                    if t2 % 2:
                        nc.scalar.copy(osb[:, 1], acc[:])
                    else:
                        nc.vector.tensor_copy(osb[:, 0], acc[:])
                    if t2 % 2 == 1:
                        # out_d rows (b*2+q)*P+j', free [t2' in 4, c, t]; this
                        # pair covers t2' in {t2-1, t2} of quad q = t2 // 4.
                        q, t2p = divmod(t2 - 1, 4)
                        row0 = (b * 2 + q) * P
                        dst = out_d[row0 : row0 + P, :].rearrange(
                            "p (tp c t) -> p tp c t", tp=4, c=2
                        )[:, t2p : t2p + 2]
                        eng2 = nc.sync if (t2 % 4 == 1) else nc.scalar
                        eng2.dma_start(out=dst, in_=osb[:])

            # Pipeline: B(b) emitted after A(b+1) so the b-shuffle has a full
            # A-phase of DMA time to land before the PE needs it.
            emit_A(0)
            for b in range(B):
                if b + 1 < B:
                    emit_A(b + 1)
                bh = emit_casts(b) if int8_shuffle else None
                emit_B(b, bh)

    nc.compile()
    return nc


def _build_program_v8():
    """v7 with 4-kind stationaries (ar, ai, br, bi — 4 MiB instead of 6).
    The negated operands move to the moving side: xs carries a third plane
    nxi = -xi (negated once at the head), and bn carries a third slot
    nyim = -y_im (negated after each shuffle lands, split across vector and
    scalar).  re = ar@xr + ai@nxi, im = ai@xr + ar@xi, and likewise for B."""
    nc = bacc.Bacc(
        "TRN2", target_bir_lowering=False, debug=False, num_devices=N_CORES,
        detect_race_conditions=False,
    )
    B = MESH_BATCH
    TCv = TOK5

    xt_d = nc.declare_dram_parameter("xt", [P, 8, 2, TCv], BF16, isOutput=False)
    # kinds: 0=ar 1=ai 2=br 3=bi
    mat_d = nc.declare_dram_parameter("mat", [P, B, 4, 8, P], BF16, isOutput=False)
    out_d = nc.declare_dram_parameter("out", [B * 2 * P, 4 * 2 * TCv], BF16,
                                      isOutput=True)

    with tile.TileContext(nc) as tc:
        with (
            tc.tile_pool(name="const", bufs=1) as const_pool,
            tc.tile_pool(name="mats", bufs=1) as mat_pool,
            tc.tile_pool(name="x", bufs=1) as x_pool,
            tc.tile_pool(name="ya", bufs=14) as ya_pool,
            tc.tile_pool(name="bin", bufs=1) as bin_pool,
            tc.tile_pool(name="osb", bufs=4) as o_pool,
            tc.tile_pool(name="ps", bufs=4, space=bass.MemorySpace.PSUM) as ps_pool,
        ):
            # --- all input DMAs first: nothing blocks the rings ---
            xs = x_pool.tile([P, 8, 2, TCv], BF16, tag="xs")
            for i in range(4):
                nc.sync.dma_start(out=xs[:, 2 * i : 2 * i + 2],
                                  in_=xt_d[:, 2 * i : 2 * i + 2])
            xn = x_pool.tile([P, 8, TCv], BF16, tag="xn")

            matA, matB = [], []
            for b in range(B):
                mA = mat_pool.tile([P, 2, 8, P], BF16, tag=f"matA{b}",
                                   name=f"matA{b}")
                mB = mat_pool.tile([P, 2, 8, P], BF16, tag=f"matB{b}",
                                   name=f"matB{b}")
                matA.append(mA)
                matB.append(mB)
            # balance the mats across the scalar and gpsimd rings, earliest
            # batches first, so each matX_b lands just ahead of its phase.
            for b in range(B):
                eng = nc.scalar if b < 2 else nc.gpsimd
                eng.dma_start(out=matA[b][:], in_=mat_d[:, b, 0:2])
                eng.dma_start(out=matB[b][:], in_=mat_d[:, b, 2:4])

            ident = const_pool.tile([P, P], F32)
            make_identity(nc, ident[:])

            warm = ps_pool.tile([P, 2, TCv], F32, tag="ps")
            for _ in range(5):
                for j in range(4):
                    nc.tensor.transpose(
                        warm[:, j // 2, (j % 2) * P : (j % 2 + 1) * P],
                        ident[:], ident[:],
                    )

            # nxi planes, negated per-r on vector while mats stream in.
            for r in range(8):
                nc.vector.tensor_scalar_mul(xn[:, r], xs[:, r, 1], -1.0)

            bn = [
                bin_pool.tile([P, 8, 2, TCv], BF16, tag=f"bin{b}", name=f"bin{b}")
                for b in range(B)
            ]
            # nyim planes (negated post-shuffle)
            bnn = [
                bin_pool.tile([P, 8, TCv], BF16, tag=f"binn{b}", name=f"binn{b}")
                for b in range(B)
            ]

            def emit_A(b):
                mA = matA[b]
                for r in range(8):
                    g = _rev(r, 3)
                    acc = ps_pool.tile([P, 2, TCv], F32, tag="ps")
                    ar_ = mA[:, 0, r, :]
                    ai_ = mA[:, 1, r, :]
                    xr_ = xs[:, r, 0, :]
                    xi_ = xs[:, r, 1, :]
                    nxi = xn[:, r, :]
                    nc.tensor.matmul(acc[:, 0, :], ar_, xr_, start=True, stop=False)
                    nc.tensor.matmul(acc[:, 0, :], ai_, nxi, start=False, stop=True)
                    nc.tensor.matmul(acc[:, 1, :], ai_, xr_, start=True, stop=False)
                    nc.tensor.matmul(acc[:, 1, :], ar_, xi_, start=False, stop=True)
                    ya = ya_pool.tile([P, 2, TCv], BF16, tag="ya")
                    if r % 2:
                        nc.scalar.copy(ya[:], acc[:])
                    else:
                        nc.vector.tensor_copy(ya[:], acc[:])
                    eng2 = nc.scalar if (g % 2) else nc.sync
                    eng2.dma_start(out=bn[b][g:P:8], in_=ya[:])

            def emit_negs(b):
                # nyim = -yim once the b-shuffle has landed; split across
                # engines so neither eats a full phase.
                nc.vector.tensor_scalar_mul(bnn[b][:, 0:4], bn[b][:, 0:4, 1], -1.0)
                nc.scalar.mul(bnn[b][:, 4:8], bn[b][:, 4:8, 1], -1.0)

            def emit_B(b):
                mB = matB[b]
                osb = None
                for t2 in range(8):
                    acc = ps_pool.tile([P, 2, TCv], F32, tag="ps")
                    br_ = mB[:, 0, t2, :]
                    bi_ = mB[:, 1, t2, :]
                    yre = bn[b][:, t2, 0, :]
                    yim = bn[b][:, t2, 1, :]
                    nyim = bnn[b][:, t2, :]
                    nc.tensor.matmul(acc[:, 0, :], br_, yre, start=True, stop=False)
                    nc.tensor.matmul(acc[:, 0, :], bi_, nyim, start=False, stop=True)
                    nc.tensor.matmul(acc[:, 1, :], bi_, yre, start=True, stop=False)
                    nc.tensor.matmul(acc[:, 1, :], br_, yim, start=False, stop=True)
                    if t2 % 2 == 0:
                        osb = o_pool.tile([P, 2, 2, TCv], BF16, tag="osb")
                    if t2 % 2:
                        nc.scalar.copy(osb[:, 1], acc[:])
                    else:
                        nc.vector.tensor_copy(osb[:, 0], acc[:])
                    if t2 % 2 == 1:
                        # out_d rows (b*2+q)*P+j', free [t2' in 4, c, t]; this
                        # pair covers t2' in {t2-1, t2} of quad q = t2 // 4.
                        q, t2p = divmod(t2 - 1, 4)
                        row0 = (b * 2 + q) * P
                        dst = out_d[row0 : row0 + P, :].rearrange(
                            "p (tp c t) -> p tp c t", tp=4, c=2
                        )[:, t2p : t2p + 2]
                        eng2 = nc.sync if (t2 % 4 == 1) else nc.scalar
                        eng2.dma_start(out=dst, in_=osb[:])

            # B(b) two phases after A(b): each shuffle gets ~2 phases of
            # port time before the PE needs it.
            emit_A(0)
            emit_A(1)
            emit_A(2)
            emit_negs(0)
            emit_B(0)
            emit_A(3)
            emit_negs(1)
            emit_B(1)
            emit_negs(2)
            emit_B(2)
            emit_negs(3)
            emit_B(3)

    nc.compile()
    return nc


def _build_program_v7():
    """v5 pipeline (unpaired shuffle, balanced A/B interleave) with the v6
    6-kind stationaries, plus head fixes: DMA triggers are emitted before
    make_identity (which otherwise blocks the sync/gpsimd FIFOs for ~1.5us),
    stationaries are split into A/B-kind halves with batch 0 on the scalar
    HWDGE ring so stage A can start as early as possible, and the PE warmup
    is sized to the head gap."""
    nc = bacc.Bacc(
        "TRN2", target_bir_lowering=False, debug=False, num_devices=N_CORES,
        detect_race_conditions=False,
    )
    B = MESH_BATCH
    TCv = TOK5

    xt_d = nc.declare_dram_parameter("xt", [P, 8, 2, TCv], BF16, isOutput=False)
    # kinds: 0=ar 1=ai 2=nai 3=br 4=bi 5=nbi
    mat_d = nc.declare_dram_parameter("mat", [P, B, 6, 8, P], BF16, isOutput=False)
    out_d = nc.declare_dram_parameter("out", [B * 2 * P, 4 * 2 * TCv], BF16,
                                      isOutput=True)

    with tile.TileContext(nc) as tc:
        with (
            tc.tile_pool(name="const", bufs=1) as const_pool,
            tc.tile_pool(name="mats", bufs=1) as mat_pool,
            tc.tile_pool(name="x", bufs=1) as x_pool,
            tc.tile_pool(name="ya", bufs=6) as ya_pool,
            tc.tile_pool(name="bin", bufs=1) as bin_pool,
            tc.tile_pool(name="osb", bufs=3) as o_pool,
            tc.tile_pool(name="ps", bufs=4, space=bass.MemorySpace.PSUM) as ps_pool,
        ):
            # --- all input DMAs first: nothing blocks the rings ---
            xs = x_pool.tile([P, 8, 2, TCv], BF16, tag="xs")
            nc.sync.dma_start(out=xs[:, 0:4], in_=xt_d[:, 0:4])
            nc.sync.dma_start(out=xs[:, 4:8], in_=xt_d[:, 4:8])

            matA, matB = [], []
            for b in range(B):
                mA = mat_pool.tile([P, 3, 8, P], BF16, tag=f"matA{b}",
                                   name=f"matA{b}")
                mB = mat_pool.tile([P, 3, 8, P], BF16, tag=f"matB{b}",
                                   name=f"matB{b}")
                matA.append(mA)
                matB.append(mB)
            # batch 0 on the (otherwise idle) scalar ring for earliest arrival;
            # the rest stream in FIFO order on the gpsimd SWDGE ring.
            nc.scalar.dma_start(out=matA[0][:], in_=mat_d[:, 0, 0:3])
            nc.scalar.dma_start(out=matB[0][:], in_=mat_d[:, 0, 3:6])
            for b in range(1, B):
                nc.gpsimd.dma_start(out=matA[b][:], in_=mat_d[:, b, 0:3])
                nc.gpsimd.dma_start(out=matB[b][:], in_=mat_d[:, b, 3:6])

            ident = const_pool.tile([P, P], F32)
            make_identity(nc, ident[:])

            # Short HAM warmup sized to the ~4us head gap.
            warm = ps_pool.tile([P, 2, TCv], F32, tag="ps")
            for _ in range(5):
                for j in range(4):
                    nc.tensor.transpose(
                        warm[:, j // 2, (j % 2) * P : (j % 2 + 1) * P],
                        ident[:], ident[:],
                    )

            bn = [
                bin_pool.tile([P, 8, 2, TCv], BF16, tag=f"bin{b}", name=f"bin{b}")
                for b in range(B)
            ]

            def emit_A(b):
                mA = matA[b]
                for r in range(8):
                    g = _rev(r, 3)
                    acc = ps_pool.tile([P, 2, TCv], F32, tag="ps")
                    ar_ = mA[:, 0, r, :]
                    ai_ = mA[:, 1, r, :]
                    nai = mA[:, 2, r, :]
                    xr_ = xs[:, r, 0, :]
                    xi_ = xs[:, r, 1, :]
                    nc.tensor.matmul(acc[:, 0, :], ar_, xr_, start=True, stop=False)
                    nc.tensor.matmul(acc[:, 0, :], nai, xi_, start=False, stop=True)
                    nc.tensor.matmul(acc[:, 1, :], ai_, xr_, start=True, stop=False)
                    nc.tensor.matmul(acc[:, 1, :], ar_, xi_, start=False, stop=True)
                    ya = ya_pool.tile([P, 2, TCv], BF16, tag="ya")
                    if r % 2:
                        nc.scalar.copy(ya[:], acc[:])
                    else:
                        nc.vector.tensor_copy(ya[:], acc[:])
                    eng2 = nc.scalar if (g % 2) else nc.sync
                    eng2.dma_start(out=bn[b][g:P:8], in_=ya[:])

            def emit_B(b):
                mB = matB[b]
                osb = None
                for t2 in range(8):
                    acc = ps_pool.tile([P, 2, TCv], F32, tag="ps")
                    br_ = mB[:, 0, t2, :]
                    bi_ = mB[:, 1, t2, :]
                    nbi = mB[:, 2, t2, :]
                    yre = bn[b][:, t2, 0, :]
                    yim = bn[b][:, t2, 1, :]
                    nc.tensor.matmul(acc[:, 0, :], br_, yre, start=True, stop=False)
                    nc.tensor.matmul(acc[:, 0, :], nbi, yim, start=False, stop=True)
                    nc.tensor.matmul(acc[:, 1, :], bi_, yre, start=True, stop=False)
                    nc.tensor.matmul(acc[:, 1, :], br_, yim, start=False, stop=True)
                    if t2 % 2 == 0:
                        osb = o_pool.tile([P, 2, 2, TCv], BF16, tag="osb")
                    if t2 % 2:
                        nc.scalar.copy(osb[:, 1], acc[:])
                    else:
                        nc.vector.tensor_copy(osb[:, 0], acc[:])
                    if t2 % 2 == 1:
                        # out_d rows (b*2+q)*P+j', free [t2' in 4, c, t]; this
                        # pair covers t2' in {t2-1, t2} of quad q = t2 // 4.
                        q, t2p = divmod(t2 - 1, 4)
                        row0 = (b * 2 + q) * P
                        dst = out_d[row0 : row0 + P, :].rearrange(
                            "p (tp c t) -> p tp c t", tp=4, c=2
                        )[:, t2p : t2p + 2]
                        eng2 = nc.sync if (t2 % 4 == 1) else nc.scalar
                        eng2.dma_start(out=dst, in_=osb[:])

            for step in ("A0", "A1", "B0", "A2", "B1", "A3", "B2", "B3"):
                (emit_A if step[0] == "A" else emit_B)(int(step[1]))

    nc.compile()
    return nc


def _build_program_v6():
    """v5 + (a) all 6 stationary kinds from HBM (no on-device negation: the
    1.9us scalar negates stalled the PSUM-evac path at each phase head) and
    (b) batch-paired shuffle: stage-A results for batches (2p, 2p+1) share one
    ya2 tile, so each partition-regroup DMA moves 4 KiB runs (halves the
    descriptor count of the port-bound shuffle)."""
    nc = bacc.Bacc(
        "TRN2", target_bir_lowering=False, debug=False, num_devices=N_CORES,
        detect_race_conditions=False,
    )
    B = MESH_BATCH
    TCv = TOK5

    xt_d = nc.declare_dram_parameter("xt", [P, 8, 2, TCv], BF16, isOutput=False)
    # kinds: 0=ar 1=ai 2=nai 3=br 4=bi 5=nbi
    mat_d = nc.declare_dram_parameter("mat", [P, B, 6, 8, P], BF16, isOutput=False)
    out_d = nc.declare_dram_parameter("out", [B * 2 * P, 4 * 2 * TCv], BF16,
                                      isOutput=True)

    with tile.TileContext(nc) as tc:
        with (
            tc.tile_pool(name="const", bufs=1) as const_pool,
            tc.tile_pool(name="mats", bufs=1) as mat_pool,
            tc.tile_pool(name="x", bufs=1) as x_pool,
            tc.tile_pool(name="ya", bufs=9) as ya_pool,
            tc.tile_pool(name="bin", bufs=1) as bin_pool,
            tc.tile_pool(name="osb", bufs=2) as o_pool,
            tc.tile_pool(name="ps", bufs=4, space=bass.MemorySpace.PSUM) as ps_pool,
        ):
            ident = const_pool.tile([P, P], F32)
            make_identity(nc, ident[:])

            warm = ps_pool.tile([P, 2, TCv], F32, tag="ps")
            for _ in range(9):
                for j in range(4):
                    nc.tensor.transpose(
                        warm[:, j // 2, (j % 2) * P : (j % 2 + 1) * P],
                        ident[:], ident[:],
                    )

            xs = x_pool.tile([P, 8, 2, TCv], BF16, tag="xs")
            nc.sync.dma_start(out=xs[:, 0:4], in_=xt_d[:, 0:4])
            nc.sync.dma_start(out=xs[:, 4:8], in_=xt_d[:, 4:8])

            matb = []
            for b in range(B):
                mb = mat_pool.tile([P, 6, 8, P], BF16, tag=f"mat{b}")
                nc.gpsimd.dma_start(out=mb[:], in_=mat_d[:, b])
                matb.append(mb)

            bn2 = [
                bin_pool.tile([P, 8, 2, 2, TCv], BF16, tag=f"bin{p}",
                              name=f"bin{p}")
                for p in range(2)
            ]
            ya2 = {}

            def emit_A(b):
                mb = matb[b]
                pair, half = divmod(b, 2)
                for r in range(8):
                    g = _rev(r, 3)
                    acc = ps_pool.tile([P, 2, TCv], F32, tag="ps")
                    ar_ = mb[:, 0, r, :]
                    ai_ = mb[:, 1, r, :]
                    nai = mb[:, 2, r, :]
                    xr_ = xs[:, r, 0, :]
                    xi_ = xs[:, r, 1, :]
                    nc.tensor.matmul(acc[:, 0, :], ar_, xr_, start=True, stop=False)
                    nc.tensor.matmul(acc[:, 0, :], nai, xi_, start=False, stop=True)
                    nc.tensor.matmul(acc[:, 1, :], ai_, xr_, start=True, stop=False)
                    nc.tensor.matmul(acc[:, 1, :], ar_, xi_, start=False, stop=True)
                    if half == 0:
                        ya2[pair, g] = ya_pool.tile(
                            [P, 2, 2, TCv], BF16, tag="ya", name=f"ya{pair}_{g}"
                        )
                    dst = ya2[pair, g][:, half]
                    if r % 2:
                        nc.scalar.copy(dst, acc[:])
                    else:
                        nc.vector.tensor_copy(dst, acc[:])
                    if half == 1:
                        eng2 = nc.scalar if (g % 2) else nc.sync
                        eng2.dma_start(
                            out=bn2[pair][g:P:8], in_=ya2[pair, g][:]
                        )

            def emit_B(b):
                mb = matb[b]
                pair, half = divmod(b, 2)
                osb = None
                for t2 in range(8):
                    acc = ps_pool.tile([P, 2, TCv], F32, tag="ps")
                    br_ = mb[:, 3, t2, :]
                    bi_ = mb[:, 4, t2, :]
                    nbi = mb[:, 5, t2, :]
                    yre = bn2[pair][:, t2, half, 0, :]
                    yim = bn2[pair][:, t2, half, 1, :]
                    nc.tensor.matmul(acc[:, 0, :], br_, yre, start=True, stop=False)
                    nc.tensor.matmul(acc[:, 0, :], nbi, yim, start=False, stop=True)
                    nc.tensor.matmul(acc[:, 1, :], bi_, yre, start=True, stop=False)
                    nc.tensor.matmul(acc[:, 1, :], br_, yim, start=False, stop=True)
                    if t2 % 2 == 0:
                        osb = o_pool.tile([P, 2, 2, TCv], BF16, tag="osb")
                    if t2 % 2:
                        nc.scalar.copy(osb[:, 1], acc[:])
                    else:
                        nc.vector.tensor_copy(osb[:, 0], acc[:])
                    if t2 % 2 == 1:
                        # out_d rows (b*2+q)*P+j', free [t2' in 4, c, t]; this
                        # pair covers t2' in {t2-1, t2} of quad q = t2 // 4.
                        q, t2p = divmod(t2 - 1, 4)
                        row0 = (b * 2 + q) * P
                        dst = out_d[row0 : row0 + P, :].rearrange(
                            "p (tp c t) -> p tp c t", tp=4, c=2
                        )[:, t2p : t2p + 2]
                        eng2 = nc.sync if (t2 % 4 == 1) else nc.scalar
                        eng2.dma_start(out=dst, in_=osb[:])

            for step in ("A0", "A1", "A2", "B0", "A3", "B1", "B2", "B3"):
                (emit_A if step[0] == "A" else emit_B)(int(step[1]))

    nc.compile()
    return nc


_CACHED = {}


def _host_prep_v4(x_re, x_im, phases):
    """Host-side: transposed/r-grouped bf16 x per (core-half), bf16 mats."""
    import ml_dtypes

    Astat, Bstat = _stage_matrices(phases)
    bf = ml_dtypes.bfloat16
    ar = Astat.real.reshape(MESH_BATCH, 8 * P, P).astype(bf)
    ai = Astat.imag.reshape(MESH_BATCH, 8 * P, P).astype(bf)
    br = Bstat.real.reshape(MESH_BATCH, 8 * P, P).astype(bf)
    bi = Bstat.imag.reshape(MESH_BATCH, 8 * P, P).astype(bf)

    half = N_TOKENS // 2
    xts = []
    for h in range(2):
        planes = []
        for xp in (x_re, x_im):
            # [T, L] -> [L, T] -> (p, r) rows -> [r, p, T]
            xT = xp[h * half : (h + 1) * half].T.reshape(P, 8, half)
            planes.append(xT.transpose(1, 0, 2))
        xt = np.concatenate(planes, axis=0).reshape(16 * P, half)
        xts.append(np.ascontiguousarray(xt).astype(bf))
    return ar, ai, br, bi, xts


_JCOLS = None


def _jcols():
    global _JCOLS
    if _JCOLS is None:
        idx = np.arange(P)
        v_, m_ = np.divmod(idx, 8)
        _JCOLS = [P * m_ + 8 * v_ + _rev(t2, 3) for t2 in range(8)]
    return _JCOLS


def _host_prep_v5(x_re, x_im, phases, int8_shuffle, six_kinds=False):
    """Pack stationaries [P, B, K, 8, P] bf16 (K=4: ar, ai, br, bi; K=6 adds
    nai, nbi; int8 scales folded) and per-core x panels [P, 8, 2, TOK5]."""
    import ml_dtypes

    bf = ml_dtypes.bfloat16
    _USPLIT[0] = True
    try:
        Astat, Bstat = _stage_matrices(phases)
    finally:
        _USPLIT[0] = False
    s = YSCALE if int8_shuffle else 1.0
    Astat = Astat * np.float32(s)
    Bstat = Bstat * np.float32(1.0 / s)
    if six_kinds:
        # kinds: ar, ai, nai, br, bi, nbi
        mat = np.empty((P, MESH_BATCH, 6, 8, P), dtype=bf)
        mat[:, :, 0] = Astat.real.astype(bf).transpose(2, 0, 1, 3)
        mat[:, :, 1] = Astat.imag.astype(bf).transpose(2, 0, 1, 3)
        mat[:, :, 2] = (-Astat.imag).astype(bf).transpose(2, 0, 1, 3)
        mat[:, :, 3] = Bstat.real.astype(bf).transpose(2, 0, 1, 3)
        mat[:, :, 4] = Bstat.imag.astype(bf).transpose(2, 0, 1, 3)
        mat[:, :, 5] = (-Bstat.imag).astype(bf).transpose(2, 0, 1, 3)
    else:
        # mat[p, b, kind, r, m]
        mat = np.empty((P, MESH_BATCH, 4, 8, P), dtype=bf)
        mat[:, :, 0] = Astat.real.astype(bf).transpose(2, 0, 1, 3)
        mat[:, :, 1] = Astat.imag.astype(bf).transpose(2, 0, 1, 3)
        mat[:, :, 2] = Bstat.real.astype(bf).transpose(2, 0, 1, 3)
        mat[:, :, 3] = Bstat.imag.astype(bf).transpose(2, 0, 1, 3)
    mat = np.ascontiguousarray(mat)

    xts = []
    for c in range(N_CORES):
        t0 = c * TOK5
        panes = []
        for xp in (x_re, x_im):
            # [TOK5, L] -> [L, TOK5] -> [P, 8, TOK5]  (L-index = 8p + r)
            panes.append(xp[t0 : t0 + TOK5].T.reshape(P, 8, TOK5))
        xt = np.stack(panes, axis=2)  # [P, 8, 2, TOK5]
        xts.append(np.ascontiguousarray(xt).astype(bf))
    return mat, xts


def kernel(x_re: np.ndarray, x_im: np.ndarray, phases: np.ndarray) -> np.ndarray:
    global LAST_RESULTS

    x_re = np.ascontiguousarray(x_re, dtype=np.float32)
    x_im = np.ascontiguousarray(x_im, dtype=np.float32)
    phases = np.ascontiguousarray(phases, dtype=np.float32)

    if VERSION in (5, 6, 7, 8):
        six = VERSION in (6, 7)
        mat, xts = _host_prep_v5(x_re, x_im, phases, INT8_SHUFFLE and not six,
                                 six_kinds=six)
        key = (VERSION, INT8_SHUFFLE and not six)
        if key not in _CACHED:
            _CACHED[key] = (
                _build_program_v8() if VERSION == 8
                else _build_program_v7() if VERSION == 7
                else _build_program_v6() if six
                else _build_program_v5(INT8_SHUFFLE)
            )
        nc = _CACHED[key]
        in_maps = [{"xt": xts[c], "mat": mat} for c in range(N_CORES)]
        res = run_bass_kernel_spmd(nc, in_maps, list(range(N_CORES)), trace=TRACE)
        LAST_RESULTS = res
        jcols = _jcols()
        out = np.empty((MESH_BATCH, N_TOKENS, L), dtype=np.complex64)
        for c in range(N_CORES):
            t0 = c * TOK5
            sl = slice(t0, t0 + TOK5)
            arr = np.asarray(res.results[c]["out"], dtype=np.float32).reshape(
                MESH_BATCH, 2, P, 4, 2, TOK5
            )
            for t2 in range(8):
                q, t2p = divmod(t2, 4)
                for b in range(MESH_BATCH):
                    cplx = (
                        arr[b, q, :, t2p, 0, :] + 1j * arr[b, q, :, t2p, 1, :]
                    ).astype(np.complex64)  # [P, TOK5]
                    out[b, sl, jcols[t2]] = cplx
        return out

    half = N_TOKENS // 2
    in_maps = []
    if VERSION == 4:
        ar, ai, br, bi, xts = _host_prep_v4(x_re, x_im, phases)
        if 4 not in _CACHED:
            _CACHED[4] = _build_program_v4()
        nc = _CACHED[4]
        packed = []
        for b in range(MESH_BATCH):
            kinds = [ar[b], ai[b], -ai[b], br[b], bi[b], -bi[b]]
            m = np.concatenate([k.reshape(8, P, P) for k in kinds], axis=0)
            packed.append(np.ascontiguousarray(m.transpose(1, 0, 2).reshape(P, 48 * P)))
        for c in range(N_CORES):
            b, h = c // 2, c % 2
            in_maps.append({"xt": xts[h], "mat": packed[b]})
        res = run_bass_kernel_spmd(nc, in_maps, list(range(N_CORES)), trace=TRACE)
        LAST_RESULTS = res
        jcols = _jcols()
        out = np.empty((MESH_BATCH, N_TOKENS, L), dtype=np.complex64)
        for c in range(N_CORES):
            b, h = c // 2, c % 2
            # [NCH, 8, P, 2, TC] bf16 -> upcast once
            arr = np.asarray(res.results[c]["out"], dtype=np.float32).reshape(
                NCH, 8, P, 2, TC
            )
            sl = slice(h * half, (h + 1) * half)
            for t2 in range(8):
                cplx = (arr[:, t2, :, 0, :] + 1j * arr[:, t2, :, 1, :]).astype(
                    np.complex64
                )  # [NCH, P, TC]
                out[b, sl, jcols[t2]] = cplx.transpose(1, 0, 2).reshape(P, half)
        return out
    if VERSION == 2:
        W = _build_W(phases)                  # (B, L, L) complex64
        Wr = np.ascontiguousarray(W.real, dtype=np.float32)
        Wi = np.ascontiguousarray(W.imag, dtype=np.float32)
        if 2 not in _CACHED:
            _CACHED[2] = _build_program()
        nc = _CACHED[2]
        for c in range(N_CORES):
            b, h = c // 2, c % 2
            in_maps.append(
                {
                    "xr": x_re[h * half : (h + 1) * half],
                    "xi": x_im[h * half : (h + 1) * half],
                    "wr": Wr[b],
                    "wi": Wi[b],
                }
            )
    else:
        import ml_dtypes

        Astat, Bstat = _stage_matrices(phases)
        ar = np.ascontiguousarray(Astat.real.reshape(MESH_BATCH, 8 * P, P))
        ai = np.ascontiguousarray(Astat.imag.reshape(MESH_BATCH, 8 * P, P))
        br = Bstat.real.reshape(MESH_BATCH, 8 * P, P).astype(ml_dtypes.bfloat16)
        bi = Bstat.imag.reshape(MESH_BATCH, 8 * P, P).astype(ml_dtypes.bfloat16)
        if 3 not in _CACHED:
            _CACHED[3] = _build_program_v3()
        nc = _CACHED[3]
        for c in range(N_CORES):
            b, h = c // 2, c % 2
            in_maps.append(
                {
                    "xr": x_re[h * half : (h + 1) * half],
                    "xi": x_im[h * half : (h + 1) * half],
                    "ar": ar[b],
                    "ai": ai[b],
                    "nai": np.ascontiguousarray(-ai[b]),
                    "br": br[b],
                    "bi": bi[b],
                    "nbi": np.ascontiguousarray(-bi[b]),
                }
            )

    res = run_bass_kernel_spmd(nc, in_maps, list(range(N_CORES)), trace=TRACE)
    LAST_RESULTS = res

    out = np.empty((MESH_BATCH, N_TOKENS, L), dtype=np.complex64)
    for c in range(N_CORES):
        b, h = c // 2, c % 2
        out[b, h * half : (h + 1) * half] = (
            res.results[c]["out"].view(np.complex64).reshape(half, L)
        )
    return out



# revision 44
# speedup vs baseline: 1.0948x; 1.0114x over previous
"""Trainium2 Bass kernel for nn_BatchTrainableButterfly.

The reference applies, per mesh-batch b, a trainable butterfly network
(10 levels of phase shifters + 2x2 directional couplers with butterfly
permutations, plus a final phase layer and bit-reversals) to every token
row x[n, :].  For fixed phases the whole network is a linear map on
C^1024; it factors into two 128-wide PE stages (A = bitrev + levels 0..6,
block-diagonal over 8 column groups; B = levels 7..9 + final phase +
bitrev, an 8x8 mix across groups), which is 4x fewer MACs than the dense
1024x1024 matmul.

Active VERSION=5 (token-sharded): each core owns N_TOKENS/8 = 512 tokens
for ALL 4 mesh batches, which minimizes SBUF-AXI-port traffic — the
measured bottleneck (x 2 MiB + stationaries 4 MiB + inter-stage shuffle
8 MiB (counted twice: SBUF->SBUF reads AND writes cross the same 16 AXI
ports at ~435 GB/s) + out 8 MiB ~= 30 port-MiB ~= 72 us floor).  All
transposes live on the host: x arrives pre-transposed/r-grouped bf16;
out leaves position-major bf16 and the host inverts the permutation.
Per batch: 8 A-groups (4 bf16 matmuls N=512 each, fp32 PSUM pairs,
vector/scalar evacuation), a stepped-partition SBUF shuffle split into
per-(g, t2-half) DMAs (finer stage-B dependencies; stage-A's output row
order u*64+s*4+t2' makes the halves contiguous 64-partition slices),
then 8 B-groups with paired output stores (4 KiB dram lines).  Software
pipeline A0 A1 B0 A2 B1 A3 B2 B3 balances PE phases against the
port-saturated shuffle/store traffic; stationary negations (-ai, -bi)
are derived on device off the critical path.  ~98 us vs 113-118 us for
the v4 batch-sharded baseline; rel err 3.4e-3 (bf16).

An int8 shuffle variant (INT8_SHUFFLE) passes accuracy (1.1e-2) but is
slower: DVE/ACT 8-bit output casts run at half rate and gpsimd int8
upcasts at ~3 ns/elem, swamping the DMA savings.
"""

import math

import numpy as np

import concourse.tile as tile
from concourse import bacc, bass, mybir
from concourse.bass_utils import run_bass_kernel_spmd
from concourse.masks import make_identity

P = 128          # partitions
L = 1024         # butterfly length
N_TOKENS = 4096
MESH_BATCH = 4
N_CORES = 8
T = (N_TOKENS * MESH_BATCH) // N_CORES  # 2048 token-rows per core
NT = T // P      # 16 token tiles per core
KC = L // P      # 8 contraction chunks
NLEV = int(math.log2(L))  # 10

F32 = mybir.dt.float32
F32R = mybir.dt.float32r
BF16 = mybir.dt.bfloat16

TC = 512          # tokens per pipeline chunk (v3)
NCH = T // TC     # 4 chunks

I8 = mybir.dt.int8

# v5: token sharding — each core owns TOK5 tokens for ALL 4 mesh batches.
TOK5 = N_TOKENS // N_CORES   # 512 tokens per core
YSCALE = 127.0 / (4.75 * 11.3137)  # int8 shuffle: 127 / (4.75 sigma_y)

TRACE = False
LAST_RESULTS = None
VERSION = 5       # active: token-sharded two-stage butterfly (see module docstring)
INT8_SHUFFLE = False

# ----------------------------------------------------------------------
# Host side: build the per-batch transfer matrices from the phases.
# ----------------------------------------------------------------------


def _bitrev(n):
    m = int(math.log2(n))
    perm = np.arange(n).reshape(n, 1)
    for _ in range(m):
        n1 = perm.shape[0] // 2
        perm = np.hstack((perm[:n1], perm[n1:]))
    return perm.squeeze(0)


def _forward_indices(length):
    idx = []
    ar = np.arange(length)
    for level in range(int(math.log2(length)) - 1):
        bs = 2 ** (level + 2)
        ind = ar.reshape(-1, length // bs, 2, bs // 2).transpose(0, 1, 3, 2)
        idx.append(ind.reshape(-1))
    return idx


def _build_W(phases):
    """phases (B, NLEV+1, L//2, 2) -> W (B, L, L) complex64 with out = x @ W."""
    B = phases.shape[0]
    br = _bitrev(L)
    fidx = _forward_indices(L)
    dc = np.array([[1.0, 1.0j], [1.0j, 1.0]], dtype=np.complex64)

    x = np.broadcast_to(np.eye(L, dtype=np.complex64), (B, L, L)).copy()
    x = x[..., br]
    for level in range(NLEV):
        x = x.reshape(B, L, L // 2, 2)
        ph = phases[:, level : level + 1, :, :]            # (B, 1, L//2, 2)
        x = x * np.exp(1j * ph.astype(np.complex64))
        x = x @ dc
        x = x.reshape(B, L, L)
        if level < NLEV - 1:
            x = x[..., fidx[level]]
    ph = phases[:, NLEV - 1 : NLEV, :, :].reshape(B, 1, L)
    x = x * np.exp(1j * ph.astype(np.complex64))
    x = x[..., br]
    return (x / np.float32(np.sqrt(L))).astype(np.complex64)


def _rev(v, n):
    r = 0
    for _ in range(n):
        r = (r << 1) | (v & 1)
        v >>= 1
    return r


def _stage_matrices(phases):
    """Two-stage factorization of the butterfly network.

    Stage A = input bitrev + levels 0..6 (perms 0..5, no trailing perm):
    block-diagonal; column-block g is fed by x columns {i : i = 8p + r},
    r = rev3(g).  Stage B = perm fidx[6] + levels 7..9 + final phase +
    final bitrev + scale: per-position 8x8 mixing across the 8 blocks.

    Returns per batch the PE stationaries:
      Astat[b, r] (128,128) cplx : lhsT with K=p (x idx 8p+r), M=pos.
      Bstat[b,t2] (128,128) cplx : lhsT with K = g*16+s (source y(g, t2*16+s)),
                                   M = v*8+m -> out col j = 128m + 8v + rev3(t2).
    Cross-component entries of the extracted B submatrix are exactly 0.
    """
    B_ = phases.shape[0]
    br = _bitrev(L)
    fidx = _forward_indices(L)
    dc = np.array([[1.0, 1.0j], [1.0j, 1.0]], dtype=np.complex64)

    def levels(x, lo, hi, pre_br=False, post_final=False, pre_perm=None):
        if pre_br:
            x = x[..., br]
        if pre_perm is not None:
            x = x[..., pre_perm]
        for level in range(lo, hi):
            x = x.reshape(B_, L, L // 2, 2)
            x = x * np.exp(1j * phases[:, level, None, :, :].astype(np.complex64))
            x = x @ dc
            x = x.reshape(B_, L, L)
            if level < NLEV - 1 and level != 6:
                x = x[..., fidx[level]]
        if post_final:
            x = x * np.exp(
                1j * phases[:, NLEV - 1, None, :, :].reshape(B_, 1, L).astype(np.complex64)
            )
            x = x[..., br]
            x = x / np.float32(np.sqrt(L))
        return x

    eye = np.broadcast_to(np.eye(L, dtype=np.complex64), (B_, L, L)).copy()
    A = levels(eye.copy(), 0, 7, pre_br=True)
    Bm = levels(eye.copy(), 7, NLEV, post_final=True, pre_perm=fidx[6])

    # Stage-A output row order: row' = s*8 + t2 for pos p'' = t2*16 + s, so the
    # inter-stage shuffle is one plain DMA per g: yA_g[:] -> Bin[g:128:8,:,:]
    # (dst partition k = s*8 + g, free = (t2, tok)).
    ar_ = np.arange(P)
    if _USPLIT[0]:
        # row' = u*64 + s*4 + t2' with t2 = u*4 + t2': the shuffle splits
        # into per-u 64-partition DMAs (finer B dependencies, same runs).
        u_ = ar_ >> 6
        s2 = (ar_ & 63) >> 2
        t2p = ar_ & 3
        posperm = (u_ * 4 + t2p) * 16 + s2         # row' -> p''
    else:
        posperm = (ar_ & 7) * 16 + (ar_ >> 3)      # row' -> p''
    Astat = np.empty((B_, 8, P, P), dtype=np.complex64)
    for r in range(8):
        g = _rev(r, 3)
        Astat[:, r] = A[:, ar_ * 8 + r][:, :, g * P + posperm]

    s_, g_ = np.divmod(ar_, 8)                     # k = s*8 + g
    v_, m_ = np.divmod(ar_, 8)
    Bstat = np.empty((B_, 8, P, P), dtype=np.complex64)
    for t2 in range(8):
        rows = g_ * P + t2 * 16 + s_
        cols = P * m_ + 8 * v_ + _rev(t2, 3)
        Bstat[:, t2] = Bm[:, rows][:, :, cols]
    return Astat, Bstat


# ----------------------------------------------------------------------
# Device side: complex matmul kernel (SPMD, one (batch, half) per core).
# ----------------------------------------------------------------------

_USPLIT = [False]

_CACHED_NC = None


def _build_program():
    nc = bacc.Bacc(
        "TRN2", target_bir_lowering=False, debug=False, num_devices=N_CORES
    )

    xr_d = nc.declare_dram_parameter("xr", [T, L], F32, isOutput=False)
    xi_d = nc.declare_dram_parameter("xi", [T, L], F32, isOutput=False)
    wr_d = nc.declare_dram_parameter("wr", [L, L], F32R, isOutput=False)
    wi_d = nc.declare_dram_parameter("wi", [L, L], F32R, isOutput=False)
    out_d = nc.declare_dram_parameter("out", [T, 2 * L], F32, isOutput=True)

    with tile.TileContext(nc) as tc:
        with (
            tc.tile_pool(name="const", bufs=1) as const_pool,
            tc.tile_pool(name="w", bufs=1) as w_pool,
            tc.tile_pool(name="x", bufs=3) as x_pool,
            tc.tile_pool(name="xt", bufs=2) as xt_pool,
            tc.tile_pool(name="osb", bufs=3) as o_pool,
            tc.tile_pool(name="ps", bufs=8, space=bass.MemorySpace.PSUM) as ps_pool,
        ):
            ident = const_pool.tile([P, P], F32)
            make_identity(nc, ident[:])

            # Warm the PE HAM while W streams in: dummy transposes keep the
            # tensor engine busy >3.4us so it reaches full clock before the
            # real matmuls start.
            warm = ps_pool.tile([P, 4 * P], F32, tag="ps")
            for _ in range(12):
                for j in range(4):
                    nc.tensor.transpose(
                        warm[:, j * P : (j + 1) * P], ident[:], ident[:]
                    )

            # Stream W into SBUF once: per k-chunk tiles (P x L), natural layout
            # (partition = contraction row within chunk, free = output column).
            # k-major order so the first token tile's accumulation can start
            # after only a few chunks have landed.
            w_sb = {}
            for k in range(KC):
                for nm, dram in (("wr", wr_d), ("wi", wi_d)):
                    t_ = w_pool.tile([P, L], F32R, tag=f"{nm}{k}")
                    nc.sync.dma_start(out=t_[:], in_=dram[k * P : (k + 1) * P, :])
                    w_sb[nm, k] = t_
                # -Wi derived on device: saves a third of the W stream, which
                # gates the kernel head while PE waits on weights.
                nwi = w_pool.tile([P, L], F32R, tag=f"nwi{k}")
                nc.vector.tensor_scalar_mul(nwi[:], w_sb["wi", k][:], -1.0)
                w_sb["nwi", k] = nwi

            for t in range(NT):
                rows = slice(t * P, (t + 1) * P)
                xr_rows = x_pool.tile([P, L], F32, tag="xr_rows")
                xi_rows = x_pool.tile([P, L], F32, tag="xi_rows")
                nc.sync.dma_start(out=xr_rows[:], in_=xr_d[rows, :])
                nc.sync.dma_start(out=xi_rows[:], in_=xi_d[rows, :])

                # Transpose the token tile: xT chunks live at
                # xT[:, k*P:(k+1)*P] = x_rows[:, k*P:(k+1)*P].T
                xrT = xt_pool.tile([P, L], F32R, tag="xrT")
                xiT = xt_pool.tile([P, L], F32R, tag="xiT")
                for src, dst in ((xr_rows, xrT), (xi_rows, xiT)):
                    for g in range(2):
                        tp = ps_pool.tile([P, 4 * P], F32, tag="ps")
                        for j in range(4):
                            k = g * 4 + j
                            nc.tensor.transpose(
                                tp[:, j * P : (j + 1) * P],
                                src[:, k * P : (k + 1) * P],
                                ident[:],
                            )
                        nc.scalar.copy(dst[:, g * 4 * P : (g + 1) * 4 * P], tp[:])

                # Accumulate the four real matmul outputs.
                #   re_n = sum_k xrT_k @ wr_k[n] + xiT_k @ nwi_k[n]
                #   im_n = sum_k xrT_k @ wi_k[n] + xiT_k @ wr_k[n]
                out_sb = o_pool.tile([P, L, 2], F32, tag="out_sb")
                for n in range(2):
                    ncol = slice(n * 512, (n + 1) * 512)
                    acc_re = ps_pool.tile([P, 512], F32, tag="ps")
                    acc_im = ps_pool.tile([P, 512], F32, tag="ps")
                    for k in range(KC):
                        xrT_k = xrT[:, k * P : (k + 1) * P]
                        xiT_k = xiT[:, k * P : (k + 1) * P]
                        first = k == 0
                        last = k == KC - 1
                        nc.tensor.matmul(
                            acc_re[:], xrT_k, w_sb["wr", k][:, ncol],
                            start=first, stop=False,
                        )
                        nc.tensor.matmul(
                            acc_re[:], xiT_k, w_sb["nwi", k][:, ncol],
                            start=False, stop=last,
                        )
                        nc.tensor.matmul(
                            acc_im[:], xrT_k, w_sb["wi", k][:, ncol],
                            start=first, stop=False,
                        )
                        nc.tensor.matmul(
                            acc_im[:], xiT_k, w_sb["wr", k][:, ncol],
                            start=False, stop=last,
                        )
                    # Interleave re/im into complex64 memory order.
                    nc.vector.tensor_copy(out_sb[:, n * 512 : (n + 1) * 512, 0], acc_re[:])
                    nc.vector.tensor_copy(out_sb[:, n * 512 : (n + 1) * 512, 1], acc_im[:])

                nc.sync.dma_start(out=out_d[rows, :], in_=out_sb[:])

    nc.compile()
    return nc


def _build_program_v3():
    # detect_race_conditions=False: the rust race detector false-positives on
    # the stepped-partition shuffle DMA vs writes to a *different* bin buffer
    # (disjoint SBUF regions sharing a shadow zone). Same-tensor deps are
    # tracked normally and validated by the CoreSim numeric check.
    nc = bacc.Bacc(
        "TRN2", target_bir_lowering=False, debug=False, num_devices=N_CORES,
        detect_race_conditions=False,
    )

    xr_d = nc.declare_dram_parameter("xr", [T, L], F32R, isOutput=False)
    xi_d = nc.declare_dram_parameter("xi", [T, L], F32R, isOutput=False)
    ar_d = nc.declare_dram_parameter("ar", [8 * P, P], F32R, isOutput=False)
    ai_d = nc.declare_dram_parameter("ai", [8 * P, P], F32R, isOutput=False)
    nai_d = nc.declare_dram_parameter("nai", [8 * P, P], F32R, isOutput=False)
    br_d = nc.declare_dram_parameter("br", [8 * P, P], BF16, isOutput=False)
    bi_d = nc.declare_dram_parameter("bi", [8 * P, P], BF16, isOutput=False)
    nbi_d = nc.declare_dram_parameter("nbi", [8 * P, P], BF16, isOutput=False)
    out_d = nc.declare_dram_parameter("out", [T, 2 * L], F32, isOutput=True)

    with tile.TileContext(nc) as tc:
        with (
            tc.tile_pool(name="const", bufs=1) as const_pool,
            tc.tile_pool(name="mats", bufs=1) as mat_pool,
            tc.tile_pool(name="x", bufs=8) as x_pool,
            tc.tile_pool(name="xt", bufs=20) as xt_pool,
            tc.tile_pool(name="ya", bufs=12) as ya_pool,
            tc.tile_pool(name="bin", bufs=1) as bin_pool,
            tc.tile_pool(name="yb", bufs=4) as yb_pool,
            tc.tile_pool(name="osb", bufs=4) as o_pool,
            tc.tile_pool(name="ps", bufs=8, space=bass.MemorySpace.PSUM) as ps_pool,
        ):
            ident = const_pool.tile([P, P], F32)
            make_identity(nc, ident[:])
            ident_h = const_pool.tile([P, P], BF16)
            nc.vector.tensor_copy(ident_h[:], ident[:])
            ident_r = const_pool.tile([P, P], F32R)
            nc.vector.tensor_copy(ident_r[:], ident[:])

            # HAM warmup while the (small) stationaries stream in.
            warm = ps_pool.tile([P, 4 * P], F32, tag="ps")
            for _ in range(22):
                for j in range(4):
                    nc.tensor.transpose(
                        warm[:, j * P : (j + 1) * P], ident[:], ident[:]
                    )

            # Persistent double-buffered shuffle destination; memset once so
            # downstream readers of the stepped-partition DMA writes are
            # observable (sim init tracking) — overlaps with warmup/mats DMA.
            bn_bufs = []
            bn_memsets = []
            for i in range(2):
                bnb = bin_pool.tile([P, 8, 2 * TC], BF16, tag=f"bin{i}")
                bn_memsets.append(nc.gpsimd.memset(bnb[:], 0.0))
                bn_bufs.append(bnb)

            # Mats go through the gpsimd SWDGE queues so the 48 dma_starts do
            # not serialize ahead of chunk-0 row loads on the two HWDGE queues.
            mats = {}
            for nm, dram, dt_ in (
                ("ar", ar_d, F32R), ("ai", ai_d, F32R), ("nai", nai_d, F32R),
                ("br", br_d, BF16), ("bi", bi_d, BF16), ("nbi", nbi_d, BF16),
            ):
                for r in range(8):
                    t_ = mat_pool.tile([P, P], dt_, tag=f"{nm}{r}")
                    nc.gpsimd.dma_start(out=t_[:], in_=dram[r * P : (r + 1) * P, :])
                    mats[nm, r] = t_

            def emit_front(ch):
                """T_in + stage A + shuffle for chunk ch."""
                tok0 = ch * TC
                rows = {}
                for pl, dram in ((0, xr_d), (1, xi_d)):
                    for tt in range(TC // P):
                        rt = x_pool.tile([P, P, 8], F32R, tag="rows")
                        r0 = tok0 + tt * P
                        eng = nc.scalar if (tt % 2) else nc.sync
                        eng.dma_start(out=rt[:], in_=dram[r0 : r0 + P, :])
                        rows[pl, tt] = rt

                xT = {}
                for pl in range(2):
                    for r in range(8):
                        tp = ps_pool.tile([P, 4 * P], F32R, tag="ps")
                        for tt in range(TC // P):
                            nc.tensor.transpose(
                                tp[:, tt * P : (tt + 1) * P],
                                rows[pl, tt][:, :, r],
                                ident_r[:],
                            )
                        dst = xt_pool.tile([P, TC], F32R, tag="xT")
                        nc.scalar.copy(dst[:], tp[:])
                        xT[pl, r] = dst

                yA = {}
                for r in range(8):
                    g = _rev(r, 3)
                    acr = ps_pool.tile([P, TC], F32, tag="ps")
                    aci = ps_pool.tile([P, TC], F32, tag="ps")
                    nc.tensor.matmul(acr[:], mats["ar", r], xT[0, r][:], start=True, stop=False)
                    nc.tensor.matmul(acr[:], mats["nai", r], xT[1, r][:], start=False, stop=True)
                    nc.tensor.matmul(aci[:], mats["ai", r], xT[0, r][:], start=True, stop=False)
                    nc.tensor.matmul(aci[:], mats["ar", r], xT[1, r][:], start=False, stop=True)
                    ya = ya_pool.tile([P, 2 * TC], BF16, tag="ya")
                    nc.vector.tensor_copy(ya[:, 0:TC], acr[:])
                    nc.vector.tensor_copy(ya[:, TC : 2 * TC], aci[:])
                    yA[g] = ya

                # shuffle: Bin[s*8+g, t2, :] = yA[g][s*8+t2, :] — one plain DMA
                # per g; one partition per SBUF port group on both sides.
                bn = bn_bufs[ch % 2]
                for g in range(8):
                    eng = nc.scalar if (g % 2) else nc.sync
                    eng.dma_start(out=bn[g:P:8, :, :], in_=yA[g][:])
                return bn

            def emit_back(ch, bn):
                """Stage B + T_out + interleave + store for chunk ch."""
                tok0 = ch * TC
                out_sb = []
                for tt in range(TC // P):
                    osb = o_pool.tile([P, 2 * L], F32, tag="osb")
                    out_sb.append(osb)
                for t2 in range(8):
                    obr = ps_pool.tile([P, TC], F32, tag="ps")
                    obi = ps_pool.tile([P, TC], F32, tag="ps")
                    b_re = bn[:, t2, 0:TC]
                    b_im = bn[:, t2, TC : 2 * TC]
                    nc.tensor.matmul(obr[:], mats["br", t2], b_re, start=True, stop=False)
                    nc.tensor.matmul(obr[:], mats["nbi", t2], b_im, start=False, stop=True)
                    nc.tensor.matmul(obi[:], mats["bi", t2], b_re, start=True, stop=False)
                    nc.tensor.matmul(obi[:], mats["br", t2], b_im, start=False, stop=True)
                    yb = yb_pool.tile([P, 2 * TC], BF16, tag="yb")
                    nc.scalar.copy(yb[:, 0:TC], obr[:])
                    nc.scalar.copy(yb[:, TC:], obi[:])

                    base = 2 * _rev(t2, 3)
                    for tt in range(TC // P):
                        tp2 = ps_pool.tile([P, 2, 16, 8], BF16, tag="ps")
                        nc.tensor.transpose(
                            tp2[:, 0], yb[:, tt * P : (tt + 1) * P], ident_h[:]
                        )
                        nc.tensor.transpose(
                            tp2[:, 1], yb[:, TC + tt * P : TC + (tt + 1) * P], ident_h[:]
                        )
                        osr = out_sb[tt][:].rearrange(
                            "q (m v lo) -> q lo v m", m=8, v=16, lo=16
                        )
                        nc.vector.tensor_copy(osr[:, base : base + 2, :, :], tp2[:])

                for tt in range(TC // P):
                    r0 = tok0 + tt * P
                    eng = nc.scalar if (tt % 2) else nc.sync
                    eng.dma_start(out=out_d[r0 : r0 + P, :], in_=out_sb[tt][:])

            # Software pipeline: back-half of chunk ch-1 is emitted after the
            # front-half (and shuffle issue) of chunk ch, so the PE stream has
            # B/T_out work in hand while chunk ch's shuffle is in flight.
            prev = None
            for ch in range(NCH):
                bn = emit_front(ch)
                if prev is not None:
                    emit_back(prev[0], prev[1])
                prev = (ch, bn)
            emit_back(prev[0], prev[1])

    nc.compile()
    return nc


def _build_program_v4():
    """Two-stage butterfly with all transposes moved to the host.

    x arrives pre-transposed and r-grouped in HBM as bf16 rows
    (plane, r, p) x tok, so stage-A moving operands are plain contiguous
    loads.  Stage A: acc[row', tok] = A_r^T x_r with the A/B stage
    matrices stationary; the stepped-partition SBUF shuffle regroups
    (s,t2) -> (s,g) partitions for stage B; stage-B results [j', tok]
    are stored position-major and the host undoes the butterfly output
    permutation + transpose.  No PE transposes, no output interleave.
    """
    nc = bacc.Bacc(
        "TRN2", target_bir_lowering=False, debug=False, num_devices=N_CORES,
        detect_race_conditions=False,
    )

    xt_d = nc.declare_dram_parameter("xt", [16 * P, T], BF16, isOutput=False)
    # All 48 stationaries packed: [P, (6 kinds x 8 r) * P] so one DMA with
    # 12 KiB partition lines loads everything (48 separate [P,P] DMAs would
    # be 256 B/line, descriptor-overhead-bound).
    mat_d = nc.declare_dram_parameter("mat", [P, 48 * P], BF16, isOutput=False)
    # bf16 output, chunk-major: row (ch, t2, j') holds [re TC | im TC] so
    # every store writes full contiguous 2 KiB dram rows.  Host upcasts.
    out_d = nc.declare_dram_parameter("out", [NCH * 8 * P, 2 * TC], BF16, isOutput=True)

    with tile.TileContext(nc) as tc:
        with (
            tc.tile_pool(name="const", bufs=1) as const_pool,
            tc.tile_pool(name="mats", bufs=1) as mat_pool,
            tc.tile_pool(name="x", bufs=1) as x_pool,
            tc.tile_pool(name="ya", bufs=3) as ya_pool,
            tc.tile_pool(name="bin", bufs=1) as bin_pool,
            tc.tile_pool(name="osb", bufs=1) as o_pool,
            tc.tile_pool(name="ps", bufs=4, space=bass.MemorySpace.PSUM) as ps_pool,
        ):
            ident = const_pool.tile([P, P], F32)
            make_identity(nc, ident[:])

            # Short HAM warmup (~4.7us cold) covering the chunk-0 DMA window.
            warm = ps_pool.tile([P, 2, TC], F32, tag="ps")
            for _ in range(11):
                for j in range(4):
                    nc.tensor.transpose(
                        warm[:, j // 2, (j % 2) * P : (j % 2 + 1) * P],
                        ident[:], ident[:],
                    )

            # One shuffle destination per chunk: all fronts are emitted before
            # any back, so no buffer reuse hazards at all.  The stepped-
            # partition shuffle DMAs fully overwrite each buffer; no init
            # needed (race detection is disabled).
            bn_bufs = []
            for i in range(NCH):
                bnb = bin_pool.tile([P, 8, 2, TC], BF16, tag=f"bin{i}")
                bn_bufs.append(bnb)

            # Packed stationaries on the gpsimd SWDGE queue: bulk load that
            # must not block the latency-critical HWDGE queues.
            mat_all = mat_pool.tile([P, 48 * P], BF16, tag="mat")
            nc.gpsimd.dma_start(out=mat_all[:], in_=mat_d[:, :])
            mats = {}
            for ki, nm in enumerate(("ar", "ai", "nai", "br", "bi", "nbi")):
                for r in range(8):
                    idx = ki * 8 + r
                    mats[nm, r] = mat_all[:, idx * P : (idx + 1) * P]

            # Full x panel upfront on the HWDGE queues (empty at the head):
            # 16 DMAs with 4 KiB partition lines; r=0's tiles land first so
            # stage A starts after ~1.5us.  Resident all run (64 KiB/part).
            xs = {}
            for r in range(8):
                for pl in range(2):
                    xtile = x_pool.tile([P, T], BF16, tag=f"x{pl}_{r}")
                    row0 = (pl * 8 + r) * P
                    eng = nc.scalar if (pl % 2) else nc.sync
                    eng.dma_start(out=xtile[:], in_=xt_d[row0 : row0 + P, :])
                    xs[pl, r] = xtile

            def emit_front(ch):
                """Stage A + cast + shuffle for chunk ch."""
                csl = slice(ch * TC, (ch + 1) * TC)
                bn = bn_bufs[ch]
                for r in range(8):
                    g = _rev(r, 3)
                    acc = ps_pool.tile([P, 2, TC], F32, tag="ps")
                    acr = acc[:, 0, :]
                    aci = acc[:, 1, :]
                    nc.tensor.matmul(acr, mats["ar", r], xs[0, r][:, csl], start=True, stop=False)
                    nc.tensor.matmul(aci, mats["ar", r], xs[1, r][:, csl], start=True, stop=False)
                    nc.tensor.matmul(aci, mats["ai", r], xs[0, r][:, csl], start=False, stop=True)
                    nc.tensor.matmul(acr, mats["nai", r], xs[1, r][:, csl], start=False, stop=True)
                    ya = ya_pool.tile([P, 2, TC], BF16, tag=f"ya{g}")
                    if r % 2:
                        nc.scalar.copy(ya[:], acc[:])
                    else:
                        nc.vector.tensor_copy(ya[:], acc[:])
                    # Shuffle: bn[s*8+g, t2, c, :] = ya[s*8+t2, c, :]
                    eng2 = nc.scalar if (g % 2) else nc.sync
                    eng2.dma_start(out=bn[g:P:8, :, :, :], in_=ya[:])
                return bn

            def emit_back(ch, bn):
                """Stage B + store (position-major, bf16) for chunk ch."""
                for t2 in range(8):
                    ob = ps_pool.tile([P, 2, TC], F32, tag="ps")
                    obr = ob[:, 0, :]
                    obi = ob[:, 1, :]
                    b_re = bn[:, t2, 0, :]
                    b_im = bn[:, t2, 1, :]
                    nc.tensor.matmul(obr, mats["br", t2], b_re, start=True, stop=False)
                    nc.tensor.matmul(obi, mats["br", t2], b_im, start=True, stop=False)
                    nc.tensor.matmul(obi, mats["bi", t2], b_re, start=False, stop=True)
                    nc.tensor.matmul(obr, mats["nbi", t2], b_im, start=False, stop=True)
                    osb = o_pool.tile([P, 2, TC], BF16, tag=f"osb{t2}")
                    if t2 % 2:
                        nc.scalar.copy(osb[:], ob[:])
                    else:
                        nc.vector.tensor_copy(osb[:], ob[:])
                    row0 = (ch * 8 + t2) * P
                    eng = nc.sync if (t2 % 2) else nc.scalar
                    eng.dma_start(out=out_d[row0 : row0 + P, :], in_=osb[:])

            # 2-chunk lookahead: B(ch) is emitted two fronts after F(ch), so
            # its shuffle has ~2 chunks of port time to land before the PE
            # reaches it.
            bns = {}
            order = []
            for ch in range(NCH):
                order.append(("F", ch))
                if ch >= 2:
                    order.append(("B", ch - 2))
            order += [("B", NCH - 2), ("B", NCH - 1)]
            for kind, ch in order:
                if kind == "F":
                    bns[ch] = emit_front(ch)
                else:
                    emit_back(ch, bns[ch])

    nc.compile()
    return nc


def _build_program_v5(int8_shuffle: bool):
    """Token-sharded two-stage butterfly: each core runs TOK5 tokens through
    all 4 mesh-batches.  Cuts the x input to 2 MiB/core (vs 8) at the cost of
    4 MiB of stationaries, minimizing SBUF-AXI-port traffic (the measured
    bottleneck).  Optional int8 inter-stage shuffle halves the port cost of
    the partition-regroup DMA (scales folded into the stage matrices on host;
    clamped vector casts; int8->bf16 upcast split across engines)."""
    nc = bacc.Bacc(
        "TRN2", target_bir_lowering=False, debug=False, num_devices=N_CORES,
        detect_race_conditions=False,
    )
    B = MESH_BATCH
    TCv = TOK5  # 512 tokens = one chunk per batch

    xt_d = nc.declare_dram_parameter("xt", [P, 8, 2, TCv], BF16, isOutput=False)
    mat_d = nc.declare_dram_parameter("mat", [P, B, 4, 8, P], BF16, isOutput=False)
    # out rows (b*2+q)*P + j', free [t2' in 4, c in 2, TCv]; t2 = q*4+t2'.
    out_d = nc.declare_dram_parameter("out", [B * 2 * P, 4 * 2 * TCv], BF16,
                                      isOutput=True)

    ydt = I8 if int8_shuffle else BF16

    with tile.TileContext(nc) as tc:
        with (
            tc.tile_pool(name="const", bufs=1) as const_pool,
            tc.tile_pool(name="mats", bufs=1) as mat_pool,
            tc.tile_pool(name="x", bufs=1) as x_pool,
            tc.tile_pool(name="ya", bufs=10) as ya_pool,
            tc.tile_pool(name="bin", bufs=1) as bin_pool,
            tc.tile_pool(name="bnh", bufs=2) as bnh_pool,
            tc.tile_pool(name="osb", bufs=3) as o_pool,
            tc.tile_pool(name="ps", bufs=4, space=bass.MemorySpace.PSUM) as ps_pool,
        ):
            ident = const_pool.tile([P, P], F32)
            make_identity(nc, ident[:])

            # HAM warmup covering the head DMA window (~9us of PE activity).
            warm = ps_pool.tile([P, 2, TCv], F32, tag="ps")
            for _ in range(10):
                for j in range(4):
                    nc.tensor.transpose(
                        warm[:, j // 2, (j % 2) * P : (j % 2 + 1) * P],
                        ident[:], ident[:],
                    )

            # Stationaries per batch (kinds ar, ai, br, bi) + derived
            # negations.  mat0 goes FIRST on the sync ring (it gates A0 and
            # the ring is otherwise empty, so it lands in ~2.5us); the x
            # panel follows on sync; mats 1-3 stream on the gpsimd ring.
            matb, negb = [], []
            for b in range(B):
                mb = mat_pool.tile([P, 4, 8, P], BF16, tag=f"mat{b}",
                                   name=f"mat{b}")
                ng = mat_pool.tile([P, 2, 8, P], BF16, tag=f"neg{b}",
                                   name=f"neg{b}")
                matb.append(mb)
                negb.append(ng)

            # Head loads: x split across the sync and scalar rings, the
            # stationaries FIFO-ordered (batch 0 first) on the gpsimd ring.
            # Measured best; variants that serialize x on one ring or put
            # mats on the HWDGE rings delay the first shuffles and lose
            # 5-10us.
            xs = x_pool.tile([P, 8, 2, TCv], BF16, tag="xs")
            nc.sync.dma_start(out=xs[:, 0:4], in_=xt_d[:, 0:4])
            nc.scalar.dma_start(out=xs[:, 4:8], in_=xt_d[:, 4:8])
            for b in range(B):
                nc.gpsimd.dma_start(out=matb[b][:], in_=mat_d[:, b])
            negs_done = [False] * B

            bn = []
            for b in range(B):
                bnb = bin_pool.tile([P, 8, 2, TCv], ydt, tag=f"bin{b}")
                bn.append(bnb)

            def emit_A(b):
                mb, ng = matb[b], negb[b]
                if not negs_done[b]:
                    # ng[:,0] = -ai here; -bi is deferred to emit_B so each
                    # negation only costs the scalar FIFO ~1us per phase.
                    nc.scalar.mul(ng[:, 0], mb[:, 1], -1.0)
                    negs_done[b] = True
                for r in range(8):
                    g = _rev(r, 3)
                    acc = ps_pool.tile([P, 2, TCv], F32, tag="ps")
                    ar_ = mb[:, 0, r, :]
                    ai_ = mb[:, 1, r, :]
                    nai = ng[:, 0, r, :]
                    xr_ = xs[:, r, 0, :]
                    xi_ = xs[:, r, 1, :]
                    nc.tensor.matmul(acc[:, 0, :], ar_, xr_, start=True, stop=False)
                    nc.tensor.matmul(acc[:, 0, :], nai, xi_, start=False, stop=True)
                    nc.tensor.matmul(acc[:, 1, :], ai_, xr_, start=True, stop=False)
                    nc.tensor.matmul(acc[:, 1, :], ar_, xi_, start=False, stop=True)
                    ya = ya_pool.tile([P, 2, TCv], ydt, tag="ya")
                    if r in (1, 3, 5):
                        nc.scalar.copy(ya[:], acc[:])
                    else:
                        nc.vector.tensor_copy(ya[:], acc[:])
                    # per-u half shuffles: B(b) t2<4 waits only on the u=0
                    # halves; same 2 KiB descriptor runs.
                    e_lo = nc.scalar if (g % 2) else nc.sync
                    e_hi = nc.sync if (g % 2) else nc.scalar
                    e_lo.dma_start(out=bn[b][g:P:8, 0:4], in_=ya[0:64])
                    e_hi.dma_start(out=bn[b][g:P:8, 4:8], in_=ya[64:128])

            def emit_casts(b):
                """int8 bn -> bf16 for the stage-B moving operand."""
                bh = bnh_pool.tile([P, 8, 2, TCv], BF16, tag=f"bnh{b % 2}")
                nc.gpsimd.tensor_copy(bh[:, 0:3], bn[b][:, 0:3])
                nc.vector.tensor_copy(bh[:, 3:5], bn[b][:, 3:5])
                nc.gpsimd.tensor_copy(bh[:, 5:7], bn[b][:, 5:7])
                nc.scalar.copy(bh[:, 7:8], bn[b][:, 7:8])
                return bh

            def emit_B(b, bh):
                mb, ng = matb[b], negb[b]
                # ng[:,1] = -bi (see emit_A)
                nc.scalar.mul(ng[:, 1], mb[:, 3], -1.0)
                src = bh if bh is not None else bn[b]
                osb = None
                for t2 in range(8):
                    acc = ps_pool.tile([P, 2, TCv], F32, tag="ps")
                    br_ = mb[:, 2, t2, :]
                    bi_ = mb[:, 3, t2, :]
                    nbi = ng[:, 1, t2, :]
                    yre = src[:, t2, 0, :]
                    yim = src[:, t2, 1, :]
                    nc.tensor.matmul(acc[:, 0, :], br_, yre, start=True, stop=False)
                    nc.tensor.matmul(acc[:, 0, :], nbi, yim, start=False, stop=True)
                    nc.tensor.matmul(acc[:, 1, :], bi_, yre, start=True, stop=False)
                    nc.tensor.matmul(acc[:, 1, :], br_, yim, start=False, stop=True)
                    if t2 % 2 == 0:
                        osb = o_pool.tile([P, 2, 2, TCv], BF16, tag="osb")
                    if t2 % 2:
                        nc.scalar.copy(osb[:, 1], acc[:])
                    else:
                        nc.vector.tensor_copy(osb[:, 0], acc[:])
                    if t2 % 2 == 1:
                        # out_d rows (b*2+q)*P+j', free [t2' in 4, c, t]; this
                        # pair covers t2' in {t2-1, t2} of quad q = t2 // 4.
                        q, t2p = divmod(t2 - 1, 4)
                        row0 = (b * 2 + q) * P
                        dst = out_d[row0 : row0 + P, :].rearrange(
                            "p (tp c t) -> p tp c t", tp=4, c=2
                        )[:, t2p : t2p + 2]
                        eng2 = nc.sync if (t2 % 4 == 1) else nc.scalar
                        eng2.dma_start(out=dst, in_=osb[:])

            # Pipeline: B(b) emitted after A(b+1) so the b-shuffle has a full
            # A-phase of DMA time to land before the PE needs it.
            emit_A(0)
            for b in range(B):
                if b + 1 < B:
                    emit_A(b + 1)
                bh = emit_casts(b) if int8_shuffle else None
                emit_B(b, bh)

    nc.compile()
    return nc


def _build_program_v8():
    """v7 with 4-kind stationaries (ar, ai, br, bi — 4 MiB instead of 6).
    The negated operands move to the moving side: xs carries a third plane
    nxi = -xi (negated once at the head), and bn carries a third slot
    nyim = -y_im (negated after each shuffle lands, split across vector and
    scalar).  re = ar@xr + ai@nxi, im = ai@xr + ar@xi, and likewise for B."""
    nc = bacc.Bacc(
        "TRN2", target_bir_lowering=False, debug=False, num_devices=N_CORES,
        detect_race_conditions=False,
    )
    B = MESH_BATCH
    TCv = TOK5

    xt_d = nc.declare_dram_parameter("xt", [P, 8, 2, TCv], BF16, isOutput=False)
    # kinds: 0=ar 1=ai 2=br 3=bi
    mat_d = nc.declare_dram_parameter("mat", [P, B, 4, 8, P], BF16, isOutput=False)
    out_d = nc.declare_dram_parameter("out", [B * 2 * P, 4 * 2 * TCv], BF16,
                                      isOutput=True)

    with tile.TileContext(nc) as tc:
        with (
            tc.tile_pool(name="const", bufs=1) as const_pool,
            tc.tile_pool(name="mats", bufs=1) as mat_pool,
            tc.tile_pool(name="x", bufs=1) as x_pool,
            tc.tile_pool(name="ya", bufs=14) as ya_pool,
            tc.tile_pool(name="bin", bufs=1) as bin_pool,
            tc.tile_pool(name="osb", bufs=4) as o_pool,
            tc.tile_pool(name="ps", bufs=4, space=bass.MemorySpace.PSUM) as ps_pool,
        ):
            # --- all input DMAs first: nothing blocks the rings ---
            xs = x_pool.tile([P, 8, 2, TCv], BF16, tag="xs")
            for i in range(4):
                nc.sync.dma_start(out=xs[:, 2 * i : 2 * i + 2],
                                  in_=xt_d[:, 2 * i : 2 * i + 2])
            xn = x_pool.tile([P, 8, TCv], BF16, tag="xn")

            matA, matB = [], []
            for b in range(B):
                mA = mat_pool.tile([P, 2, 8, P], BF16, tag=f"matA{b}",
                                   name=f"matA{b}")
                mB = mat_pool.tile([P, 2, 8, P], BF16, tag=f"matB{b}",
                                   name=f"matB{b}")
                matA.append(mA)
                matB.append(mB)
            # balance the mats across the scalar and gpsimd rings, earliest
            # batches first, so each matX_b lands just ahead of its phase.
            for b in range(B):
                eng = nc.scalar if b < 2 else nc.gpsimd
                eng.dma_start(out=matA[b][:], in_=mat_d[:, b, 0:2])
                eng.dma_start(out=matB[b][:], in_=mat_d[:, b, 2:4])

            ident = const_pool.tile([P, P], F32)
            make_identity(nc, ident[:])

            warm = ps_pool.tile([P, 2, TCv], F32, tag="ps")
            for _ in range(5):
                for j in range(4):
                    nc.tensor.transpose(
                        warm[:, j // 2, (j % 2) * P : (j % 2 + 1) * P],
                        ident[:], ident[:],
                    )

            # nxi planes, negated per-r on vector while mats stream in.
            for r in range(8):
                nc.vector.tensor_scalar_mul(xn[:, r], xs[:, r, 1], -1.0)

            bn = [
                bin_pool.tile([P, 8, 2, TCv], BF16, tag=f"bin{b}", name=f"bin{b}")
                for b in range(B)
            ]
            # nyim planes (negated post-shuffle)
            bnn = [
                bin_pool.tile([P, 8, TCv], BF16, tag=f"binn{b}", name=f"binn{b}")
                for b in range(B)
            ]

            def emit_A(b):
                mA = matA[b]
                for r in range(8):
                    g = _rev(r, 3)
                    acc = ps_pool.tile([P, 2, TCv], F32, tag="ps")
                    ar_ = mA[:, 0, r, :]
                    ai_ = mA[:, 1, r, :]
                    xr_ = xs[:, r, 0, :]
                    xi_ = xs[:, r, 1, :]
                    nxi = xn[:, r, :]
                    nc.tensor.matmul(acc[:, 0, :], ar_, xr_, start=True, stop=False)
                    nc.tensor.matmul(acc[:, 0, :], ai_, nxi, start=False, stop=True)
                    nc.tensor.matmul(acc[:, 1, :], ai_, xr_, start=True, stop=False)
                    nc.tensor.matmul(acc[:, 1, :], ar_, xi_, start=False, stop=True)
                    ya = ya_pool.tile([P, 2, TCv], BF16, tag="ya")
                    if r % 2:
                        nc.scalar.copy(ya[:], acc[:])
                    else:
                        nc.vector.tensor_copy(ya[:], acc[:])
                    eng2 = nc.scalar if (g % 2) else nc.sync
                    eng2.dma_start(out=bn[b][g:P:8], in_=ya[:])

            def emit_negs(b):
                # nyim = -yim once the b-shuffle has landed; split across
                # engines so neither eats a full phase.
                nc.vector.tensor_scalar_mul(bnn[b][:, 0:4], bn[b][:, 0:4, 1], -1.0)
                nc.scalar.mul(bnn[b][:, 4:8], bn[b][:, 4:8, 1], -1.0)

            def emit_B(b):
                mB = matB[b]
                osb = None
                for t2 in range(8):
                    acc = ps_pool.tile([P, 2, TCv], F32, tag="ps")
                    br_ = mB[:, 0, t2, :]
                    bi_ = mB[:, 1, t2, :]
                    yre = bn[b][:, t2, 0, :]
                    yim = bn[b][:, t2, 1, :]
                    nyim = bnn[b][:, t2, :]
                    nc.tensor.matmul(acc[:, 0, :], br_, yre, start=True, stop=False)
                    nc.tensor.matmul(acc[:, 0, :], bi_, nyim, start=False, stop=True)
                    nc.tensor.matmul(acc[:, 1, :], bi_, yre, start=True, stop=False)
                    nc.tensor.matmul(acc[:, 1, :], br_, yim, start=False, stop=True)
                    if t2 % 2 == 0:
                        osb = o_pool.tile([P, 2, 2, TCv], BF16, tag="osb")
                    if t2 % 2:
                        nc.scalar.copy(osb[:, 1], acc[:])
                    else:
                        nc.vector.tensor_copy(osb[:, 0], acc[:])
                    if t2 % 2 == 1:
                        # out_d rows (b*2+q)*P+j', free [t2' in 4, c, t]; this
                        # pair covers t2' in {t2-1, t2} of quad q = t2 // 4.
                        q, t2p = divmod(t2 - 1, 4)
                        row0 = (b * 2 + q) * P
                        dst = out_d[row0 : row0 + P, :].rearrange(
                            "p (tp c t) -> p tp c t", tp=4, c=2
                        )[:, t2p : t2p + 2]
                        eng2 = nc.sync if (t2 % 4 == 1) else nc.scalar
                        eng2.dma_start(out=dst, in_=osb[:])

            # B(b) two phases after A(b): each shuffle gets ~2 phases of
            # port time before the PE needs it.
            emit_A(0)
            emit_A(1)
            emit_A(2)
            emit_negs(0)
            emit_B(0)
            emit_A(3)
            emit_negs(1)
            emit_B(1)
            emit_negs(2)
            emit_B(2)
            emit_negs(3)
            emit_B(3)

    nc.compile()
    return nc


def _build_program_v7():
    """v5 pipeline (unpaired shuffle, balanced A/B interleave) with the v6
    6-kind stationaries, plus head fixes: DMA triggers are emitted before
    make_identity (which otherwise blocks the sync/gpsimd FIFOs for ~1.5us),
    stationaries are split into A/B-kind halves with batch 0 on the scalar
    HWDGE ring so stage A can start as early as possible, and the PE warmup
    is sized to the head gap."""
    nc = bacc.Bacc(
        "TRN2", target_bir_lowering=False, debug=False, num_devices=N_CORES,
        detect_race_conditions=False,
    )
    B = MESH_BATCH
    TCv = TOK5

    xt_d = nc.declare_dram_parameter("xt", [P, 8, 2, TCv], BF16, isOutput=False)
    # kinds: 0=ar 1=ai 2=nai 3=br 4=bi 5=nbi
    mat_d = nc.declare_dram_parameter("mat", [P, B, 6, 8, P], BF16, isOutput=False)
    out_d = nc.declare_dram_parameter("out", [B * 2 * P, 4 * 2 * TCv], BF16,
                                      isOutput=True)

    with tile.TileContext(nc) as tc:
        with (
            tc.tile_pool(name="const", bufs=1) as const_pool,
            tc.tile_pool(name="mats", bufs=1) as mat_pool,
            tc.tile_pool(name="x", bufs=1) as x_pool,
            tc.tile_pool(name="ya", bufs=6) as ya_pool,
            tc.tile_pool(name="bin", bufs=1) as bin_pool,
            tc.tile_pool(name="osb", bufs=3) as o_pool,
            tc.tile_pool(name="ps", bufs=4, space=bass.MemorySpace.PSUM) as ps_pool,
        ):
            # --- all input DMAs first: nothing blocks the rings ---
            xs = x_pool.tile([P, 8, 2, TCv], BF16, tag="xs")
            nc.sync.dma_start(out=xs[:, 0:4], in_=xt_d[:, 0:4])
            nc.sync.dma_start(out=xs[:, 4:8], in_=xt_d[:, 4:8])

            matA, matB = [], []
            for b in range(B):
                mA = mat_pool.tile([P, 3, 8, P], BF16, tag=f"matA{b}",
                                   name=f"matA{b}")
                mB = mat_pool.tile([P, 3, 8, P], BF16, tag=f"matB{b}",
                                   name=f"matB{b}")
                matA.append(mA)
                matB.append(mB)
            # batch 0 on the (otherwise idle) scalar ring for earliest arrival;
            # the rest stream in FIFO order on the gpsimd SWDGE ring.
            nc.scalar.dma_start(out=matA[0][:], in_=mat_d[:, 0, 0:3])
            nc.scalar.dma_start(out=matB[0][:], in_=mat_d[:, 0, 3:6])
            for b in range(1, B):
                nc.gpsimd.dma_start(out=matA[b][:], in_=mat_d[:, b, 0:3])
                nc.gpsimd.dma_start(out=matB[b][:], in_=mat_d[:, b, 3:6])

            ident = const_pool.tile([P, P], F32)
            make_identity(nc, ident[:])

            # Short HAM warmup sized to the ~4us head gap.
            warm = ps_pool.tile([P, 2, TCv], F32, tag="ps")
            for _ in range(5):
                for j in range(4):
                    nc.tensor.transpose(
                        warm[:, j // 2, (j % 2) * P : (j % 2 + 1) * P],
                        ident[:], ident[:],
                    )

            bn = [
                bin_pool.tile([P, 8, 2, TCv], BF16, tag=f"bin{b}", name=f"bin{b}")
                for b in range(B)
            ]

            def emit_A(b):
                mA = matA[b]
                for r in range(8):
                    g = _rev(r, 3)
                    acc = ps_pool.tile([P, 2, TCv], F32, tag="ps")
                    ar_ = mA[:, 0, r, :]
                    ai_ = mA[:, 1, r, :]
                    nai = mA[:, 2, r, :]
                    xr_ = xs[:, r, 0, :]
                    xi_ = xs[:, r, 1, :]
                    nc.tensor.matmul(acc[:, 0, :], ar_, xr_, start=True, stop=False)
                    nc.tensor.matmul(acc[:, 0, :], nai, xi_, start=False, stop=True)
                    nc.tensor.matmul(acc[:, 1, :], ai_, xr_, start=True, stop=False)
                    nc.tensor.matmul(acc[:, 1, :], ar_, xi_, start=False, stop=True)
                    ya = ya_pool.tile([P, 2, TCv], BF16, tag="ya")
                    if r % 2:
                        nc.scalar.copy(ya[:], acc[:])
                    else:
                        nc.vector.tensor_copy(ya[:], acc[:])
                    eng2 = nc.scalar if (g % 2) else nc.sync
                    eng2.dma_start(out=bn[b][g:P:8], in_=ya[:])

            def emit_B(b):
                mB = matB[b]
                osb = None
                for t2 in range(8):
                    acc = ps_pool.tile([P, 2, TCv], F32, tag="ps")
                    br_ = mB[:, 0, t2, :]
                    bi_ = mB[:, 1, t2, :]
                    nbi = mB[:, 2, t2, :]
                    yre = bn[b][:, t2, 0, :]
                    yim = bn[b][:, t2, 1, :]
                    nc.tensor.matmul(acc[:, 0, :], br_, yre, start=True, stop=False)
                    nc.tensor.matmul(acc[:, 0, :], nbi, yim, start=False, stop=True)
                    nc.tensor.matmul(acc[:, 1, :], bi_, yre, start=True, stop=False)
                    nc.tensor.matmul(acc[:, 1, :], br_, yim, start=False, stop=True)
                    if t2 % 2 == 0:
                        osb = o_pool.tile([P, 2, 2, TCv], BF16, tag="osb")
                    if t2 % 2:
                        nc.scalar.copy(osb[:, 1], acc[:])
                    else:
                        nc.vector.tensor_copy(osb[:, 0], acc[:])
                    if t2 % 2 == 1:
                        # out_d rows (b*2+q)*P+j', free [t2' in 4, c, t]; this
                        # pair covers t2' in {t2-1, t2} of quad q = t2 // 4.
                        q, t2p = divmod(t2 - 1, 4)
                        row0 = (b * 2 + q) * P
                        dst = out_d[row0 : row0 + P, :].rearrange(
                            "p (tp c t) -> p tp c t", tp=4, c=2
                        )[:, t2p : t2p + 2]
                        eng2 = nc.sync if (t2 % 4 == 1) else nc.scalar
                        eng2.dma_start(out=dst, in_=osb[:])

            for step in ("A0", "A1", "B0", "A2", "B1", "A3", "B2", "B3"):
                (emit_A if step[0] == "A" else emit_B)(int(step[1]))

    nc.compile()
    return nc


def _build_program_v6():
    """v5 + (a) all 6 stationary kinds from HBM (no on-device negation: the
    1.9us scalar negates stalled the PSUM-evac path at each phase head) and
    (b) batch-paired shuffle: stage-A results for batches (2p, 2p+1) share one
    ya2 tile, so each partition-regroup DMA moves 4 KiB runs (halves the
    descriptor count of the port-bound shuffle)."""
    nc = bacc.Bacc(
        "TRN2", target_bir_lowering=False, debug=False, num_devices=N_CORES,
        detect_race_conditions=False,
    )
    B = MESH_BATCH
    TCv = TOK5

    xt_d = nc.declare_dram_parameter("xt", [P, 8, 2, TCv], BF16, isOutput=False)
    # kinds: 0=ar 1=ai 2=nai 3=br 4=bi 5=nbi
    mat_d = nc.declare_dram_parameter("mat", [P, B, 6, 8, P], BF16, isOutput=False)
    out_d = nc.declare_dram_parameter("out", [B * 2 * P, 4 * 2 * TCv], BF16,
                                      isOutput=True)

    with tile.TileContext(nc) as tc:
        with (
            tc.tile_pool(name="const", bufs=1) as const_pool,
            tc.tile_pool(name="mats", bufs=1) as mat_pool,
            tc.tile_pool(name="x", bufs=1) as x_pool,
            tc.tile_pool(name="ya", bufs=9) as ya_pool,
            tc.tile_pool(name="bin", bufs=1) as bin_pool,
            tc.tile_pool(name="osb", bufs=2) as o_pool,
            tc.tile_pool(name="ps", bufs=4, space=bass.MemorySpace.PSUM) as ps_pool,
        ):
            ident = const_pool.tile([P, P], F32)
            make_identity(nc, ident[:])

            warm = ps_pool.tile([P, 2, TCv], F32, tag="ps")
            for _ in range(9):
                for j in range(4):
                    nc.tensor.transpose(
                        warm[:, j // 2, (j % 2) * P : (j % 2 + 1) * P],
                        ident[:], ident[:],
                    )

            xs = x_pool.tile([P, 8, 2, TCv], BF16, tag="xs")
            nc.sync.dma_start(out=xs[:, 0:4], in_=xt_d[:, 0:4])
            nc.sync.dma_start(out=xs[:, 4:8], in_=xt_d[:, 4:8])

            matb = []
            for b in range(B):
                mb = mat_pool.tile([P, 6, 8, P], BF16, tag=f"mat{b}")
                nc.gpsimd.dma_start(out=mb[:], in_=mat_d[:, b])
                matb.append(mb)

            bn2 = [
                bin_pool.tile([P, 8, 2, 2, TCv], BF16, tag=f"bin{p}",
                              name=f"bin{p}")
                for p in range(2)
            ]
            ya2 = {}

            def emit_A(b):
                mb = matb[b]
                pair, half = divmod(b, 2)
                for r in range(8):
                    g = _rev(r, 3)
                    acc = ps_pool.tile([P, 2, TCv], F32, tag="ps")
                    ar_ = mb[:, 0, r, :]
                    ai_ = mb[:, 1, r, :]
                    nai = mb[:, 2, r, :]
                    xr_ = xs[:, r, 0, :]
                    xi_ = xs[:, r, 1, :]
                    nc.tensor.matmul(acc[:, 0, :], ar_, xr_, start=True, stop=False)
                    nc.tensor.matmul(acc[:, 0, :], nai, xi_, start=False, stop=True)
                    nc.tensor.matmul(acc[:, 1, :], ai_, xr_, start=True, stop=False)
                    nc.tensor.matmul(acc[:, 1, :], ar_, xi_, start=False, stop=True)
                    if half == 0:
                        ya2[pair, g] = ya_pool.tile(
                            [P, 2, 2, TCv], BF16, tag="ya", name=f"ya{pair}_{g}"
                        )
                    dst = ya2[pair, g][:, half]
                    if r % 2:
                        nc.scalar.copy(dst, acc[:])
                    else:
                        nc.vector.tensor_copy(dst, acc[:])
                    if half == 1:
                        eng2 = nc.scalar if (g % 2) else nc.sync
                        eng2.dma_start(
                            out=bn2[pair][g:P:8], in_=ya2[pair, g][:]
                        )

            def emit_B(b):
                mb = matb[b]
                pair, half = divmod(b, 2)
                osb = None
                for t2 in range(8):
                    acc = ps_pool.tile([P, 2, TCv], F32, tag="ps")
                    br_ = mb[:, 3, t2, :]
                    bi_ = mb[:, 4, t2, :]
                    nbi = mb[:, 5, t2, :]
                    yre = bn2[pair][:, t2, half, 0, :]
                    yim = bn2[pair][:, t2, half, 1, :]
                    nc.tensor.matmul(acc[:, 0, :], br_, yre, start=True, stop=False)
                    nc.tensor.matmul(acc[:, 0, :], nbi, yim, start=False, stop=True)
                    nc.tensor.matmul(acc[:, 1, :], bi_, yre, start=True, stop=False)
                    nc.tensor.matmul(acc[:, 1, :], br_, yim, start=False, stop=True)
                    if t2 % 2 == 0:
                        osb = o_pool.tile([P, 2, 2, TCv], BF16, tag="osb")
                    if t2 % 2:
                        nc.scalar.copy(osb[:, 1], acc[:])
                    else:
                        nc.vector.tensor_copy(osb[:, 0], acc[:])
                    if t2 % 2 == 1:
                        # out_d rows (b*2+q)*P+j', free [t2' in 4, c, t]; this
                        # pair covers t2' in {t2-1, t2} of quad q = t2 // 4.
                        q, t2p = divmod(t2 - 1, 4)
                        row0 = (b * 2 + q) * P
                        dst = out_d[row0 : row0 + P, :].rearrange(
                            "p (tp c t) -> p tp c t", tp=4, c=2
                        )[:, t2p : t2p + 2]
                        eng2 = nc.sync if (t2 % 4 == 1) else nc.scalar
                        eng2.dma_start(out=dst, in_=osb[:])

            for step in ("A0", "A1", "A2", "B0", "A3", "B1", "B2", "B3"):
                (emit_A if step[0] == "A" else emit_B)(int(step[1]))

    nc.compile()
    return nc


_CACHED = {}


def _host_prep_v4(x_re, x_im, phases):
    """Host-side: transposed/r-grouped bf16 x per (core-half), bf16 mats."""
    import ml_dtypes

    Astat, Bstat = _stage_matrices(phases)
    bf = ml_dtypes.bfloat16
    ar = Astat.real.reshape(MESH_BATCH, 8 * P, P).astype(bf)
    ai = Astat.imag.reshape(MESH_BATCH, 8 * P, P).astype(bf)
    br = Bstat.real.reshape(MESH_BATCH, 8 * P, P).astype(bf)
    bi = Bstat.imag.reshape(MESH_BATCH, 8 * P, P).astype(bf)

    half = N_TOKENS // 2
    xts = []
    for h in range(2):
        planes = []
        for xp in (x_re, x_im):
            # [T, L] -> [L, T] -> (p, r) rows -> [r, p, T]
            xT = xp[h * half : (h + 1) * half].T.reshape(P, 8, half)
            planes.append(xT.transpose(1, 0, 2))
        xt = np.concatenate(planes, axis=0).reshape(16 * P, half)
        xts.append(np.ascontiguousarray(xt).astype(bf))
    return ar, ai, br, bi, xts


_JCOLS = None


def _jcols():
    global _JCOLS
    if _JCOLS is None:
        idx = np.arange(P)
        v_, m_ = np.divmod(idx, 8)
        _JCOLS = [P * m_ + 8 * v_ + _rev(t2, 3) for t2 in range(8)]
    return _JCOLS


def _host_prep_v5(x_re, x_im, phases, int8_shuffle, six_kinds=False):
    """Pack stationaries [P, B, K, 8, P] bf16 (K=4: ar, ai, br, bi; K=6 adds
    nai, nbi; int8 scales folded) and per-core x panels [P, 8, 2, TOK5]."""
    import ml_dtypes

    bf = ml_dtypes.bfloat16
    _USPLIT[0] = True
    try:
        Astat, Bstat = _stage_matrices(phases)
    finally:
        _USPLIT[0] = False
    s = YSCALE if int8_shuffle else 1.0
    Astat = Astat * np.float32(s)
    Bstat = Bstat * np.float32(1.0 / s)
    if six_kinds:
        # kinds: ar, ai, nai, br, bi, nbi
        mat = np.empty((P, MESH_BATCH, 6, 8, P), dtype=bf)
        mat[:, :, 0] = Astat.real.astype(bf).transpose(2, 0, 1, 3)
        mat[:, :, 1] = Astat.imag.astype(bf).transpose(2, 0, 1, 3)
        mat[:, :, 2] = (-Astat.imag).astype(bf).transpose(2, 0, 1, 3)
        mat[:, :, 3] = Bstat.real.astype(bf).transpose(2, 0, 1, 3)
        mat[:, :, 4] = Bstat.imag.astype(bf).transpose(2, 0, 1, 3)
        mat[:, :, 5] = (-Bstat.imag).astype(bf).transpose(2, 0, 1, 3)
    else:
        # mat[p, b, kind, r, m]
        mat = np.empty((P, MESH_BATCH, 4, 8, P), dtype=bf)
        mat[:, :, 0] = Astat.real.astype(bf).transpose(2, 0, 1, 3)
        mat[:, :, 1] = Astat.imag.astype(bf).transpose(2, 0, 1, 3)
        mat[:, :, 2] = Bstat.real.astype(bf).transpose(2, 0, 1, 3)
        mat[:, :, 3] = Bstat.imag.astype(bf).transpose(2, 0, 1, 3)
    mat = np.ascontiguousarray(mat)

    xts = []
    for c in range(N_CORES):
        t0 = c * TOK5
        panes = []
        for xp in (x_re, x_im):
            # [TOK5, L] -> [L, TOK5] -> [P, 8, TOK5]  (L-index = 8p + r)
            panes.append(xp[t0 : t0 + TOK5].T.reshape(P, 8, TOK5))
        xt = np.stack(panes, axis=2)  # [P, 8, 2, TOK5]
        xts.append(np.ascontiguousarray(xt).astype(bf))
    return mat, xts


def kernel(x_re: np.ndarray, x_im: np.ndarray, phases: np.ndarray) -> np.ndarray:
    global LAST_RESULTS

    x_re = np.ascontiguousarray(x_re, dtype=np.float32)
    x_im = np.ascontiguousarray(x_im, dtype=np.float32)
    phases = np.ascontiguousarray(phases, dtype=np.float32)

    if VERSION in (5, 6, 7, 8):
        six = VERSION in (6, 7)
        mat, xts = _host_prep_v5(x_re, x_im, phases, INT8_SHUFFLE and not six,
                                 six_kinds=six)
        key = (VERSION, INT8_SHUFFLE and not six)
        if key not in _CACHED:
            _CACHED[key] = (
                _build_program_v8() if VERSION == 8
                else _build_program_v7() if VERSION == 7
                else _build_program_v6() if six
                else _build_program_v5(INT8_SHUFFLE)
            )
        nc = _CACHED[key]
        in_maps = [{"xt": xts[c], "mat": mat} for c in range(N_CORES)]
        res = run_bass_kernel_spmd(nc, in_maps, list(range(N_CORES)), trace=TRACE)
        LAST_RESULTS = res
        jcols = _jcols()
        out = np.empty((MESH_BATCH, N_TOKENS, L), dtype=np.complex64)
        for c in range(N_CORES):
            t0 = c * TOK5
            sl = slice(t0, t0 + TOK5)
            arr = np.asarray(res.results[c]["out"], dtype=np.float32).reshape(
                MESH_BATCH, 2, P, 4, 2, TOK5
            )
            for t2 in range(8):
                q, t2p = divmod(t2, 4)
                for b in range(MESH_BATCH):
                    cplx = (
                        arr[b, q, :, t2p, 0, :] + 1j * arr[b, q, :, t2p, 1, :]
                    ).astype(np.complex64)  # [P, TOK5]
                    out[b, sl, jcols[t2]] = cplx
        return out

    half = N_TOKENS // 2
    in_maps = []
    if VERSION == 4:
        ar, ai, br, bi, xts = _host_prep_v4(x_re, x_im, phases)
        if 4 not in _CACHED:
            _CACHED[4] = _build_program_v4()
        nc = _CACHED[4]
        packed = []
        for b in range(MESH_BATCH):
            kinds = [ar[b], ai[b], -ai[b], br[b], bi[b], -bi[b]]
            m = np.concatenate([k.reshape(8, P, P) for k in kinds], axis=0)
            packed.append(np.ascontiguousarray(m.transpose(1, 0, 2).reshape(P, 48 * P)))
        for c in range(N_CORES):
            b, h = c // 2, c % 2
            in_maps.append({"xt": xts[h], "mat": packed[b]})
        res = run_bass_kernel_spmd(nc, in_maps, list(range(N_CORES)), trace=TRACE)
        LAST_RESULTS = res
        jcols = _jcols()
        out = np.empty((MESH_BATCH, N_TOKENS, L), dtype=np.complex64)
        for c in range(N_CORES):
            b, h = c // 2, c % 2
            # [NCH, 8, P, 2, TC] bf16 -> upcast once
            arr = np.asarray(res.results[c]["out"], dtype=np.float32).reshape(
                NCH, 8, P, 2, TC
            )
            sl = slice(h * half, (h + 1) * half)
            for t2 in range(8):
                cplx = (arr[:, t2, :, 0, :] + 1j * arr[:, t2, :, 1, :]).astype(
                    np.complex64
                )  # [NCH, P, TC]
                out[b, sl, jcols[t2]] = cplx.transpose(1, 0, 2).reshape(P, half)
        return out
    if VERSION == 2:
        W = _build_W(phases)                  # (B, L, L) complex64
        Wr = np.ascontiguousarray(W.real, dtype=np.float32)
        Wi = np.ascontiguousarray(W.imag, dtype=np.float32)
        if 2 not in _CACHED:
            _CACHED[2] = _build_program()
        nc = _CACHED[2]
        for c in range(N_CORES):
            b, h = c // 2, c % 2
            in_maps.append(
                {
                    "xr": x_re[h * half : (h + 1) * half],
                    "xi": x_im[h * half : (h + 1) * half],
                    "wr": Wr[b],
                    "wi": Wi[b],
                }
            )
    else:
        import ml_dtypes

        Astat, Bstat = _stage_matrices(phases)
        ar = np.ascontiguousarray(Astat.real.reshape(MESH_BATCH, 8 * P, P))
        ai = np.ascontiguousarray(Astat.imag.reshape(MESH_BATCH, 8 * P, P))
        br = Bstat.real.reshape(MESH_BATCH, 8 * P, P).astype(ml_dtypes.bfloat16)
        bi = Bstat.imag.reshape(MESH_BATCH, 8 * P, P).astype(ml_dtypes.bfloat16)
        if 3 not in _CACHED:
            _CACHED[3] = _build_program_v3()
        nc = _CACHED[3]
        for c in range(N_CORES):
            b, h = c // 2, c % 2
            in_maps.append(
                {
                    "xr": x_re[h * half : (h + 1) * half],
                    "xi": x_im[h * half : (h + 1) * half],
                    "ar": ar[b],
                    "ai": ai[b],
                    "nai": np.ascontiguousarray(-ai[b]),
                    "br": br[b],
                    "bi": bi[b],
                    "nbi": np.ascontiguousarray(-bi[b]),
                }
            )

    res = run_bass_kernel_spmd(nc, in_maps, list(range(N_CORES)), trace=TRACE)
    LAST_RESULTS = res

    out = np.empty((MESH_BATCH, N_TOKENS, L), dtype=np.complex64)
    for c in range(N_CORES):
        b, h = c // 2, c % 2
        out[b, h * half : (h + 1) * half] = (
            res.results[c]["out"].view(np.complex64).reshape(half, L)
        )
    return out



# revision 46
# speedup vs baseline: 1.1103x; 1.0141x over previous
"""Trainium2 Bass kernel for nn_BatchTrainableButterfly.

The reference applies, per mesh-batch b, a trainable butterfly network
(10 levels of phase shifters + 2x2 directional couplers with butterfly
permutations, plus a final phase layer and bit-reversals) to every token
row x[n, :].  For fixed phases the whole network is a linear map on
C^1024; it factors into two 128-wide PE stages (A = bitrev + levels 0..6,
block-diagonal over 8 column groups; B = levels 7..9 + final phase +
bitrev, an 8x8 mix across groups), which is 4x fewer MACs than the dense
1024x1024 matmul.

Active VERSION=5 (token-sharded): each core owns N_TOKENS/8 = 512 tokens
for ALL 4 mesh batches, which minimizes SBUF-AXI-port traffic — the
measured bottleneck (x 2 MiB + stationaries 4 MiB + inter-stage shuffle
8 MiB (counted twice: SBUF->SBUF reads AND writes cross the same 16 AXI
ports at ~435 GB/s) + out 8 MiB ~= 30 port-MiB ~= 72 us floor).  All
transposes live on the host: x arrives pre-transposed/r-grouped bf16;
out leaves position-major bf16 and the host inverts the permutation.
Per batch: 8 A-groups (4 bf16 matmuls N=512 each, fp32 PSUM pairs,
vector/scalar evacuation), a stepped-partition SBUF shuffle split into
per-(g, t2-half) DMAs (finer stage-B dependencies; stage-A's output row
order u*64+s*4+t2' makes the halves contiguous 64-partition slices),
then 8 B-groups with paired output stores (4 KiB dram lines).  Software
pipeline A0 A1 B0 A2 B1 A3 B2 B3 balances PE phases against the
port-saturated shuffle/store traffic; stationary negations (-ai, -bi)
are derived on device off the critical path.  ~98 us vs 113-118 us for
the v4 batch-sharded baseline; rel err 3.4e-3 (bf16).

An int8 shuffle variant (INT8_SHUFFLE) passes accuracy (1.1e-2) but is
slower: DVE/ACT 8-bit output casts run at half rate and gpsimd int8
upcasts at ~3 ns/elem, swamping the DMA savings.
"""

import math

import numpy as np

import concourse.tile as tile
from concourse import bacc, bass, mybir
from concourse.bass_utils import run_bass_kernel_spmd
from concourse.masks import make_identity

P = 128          # partitions
L = 1024         # butterfly length
N_TOKENS = 4096
MESH_BATCH = 4
N_CORES = 8
T = (N_TOKENS * MESH_BATCH) // N_CORES  # 2048 token-rows per core
NT = T // P      # 16 token tiles per core
KC = L // P      # 8 contraction chunks
NLEV = int(math.log2(L))  # 10

F32 = mybir.dt.float32
F32R = mybir.dt.float32r
BF16 = mybir.dt.bfloat16

TC = 512          # tokens per pipeline chunk (v3)
NCH = T // TC     # 4 chunks

I8 = mybir.dt.int8

# v5: token sharding — each core owns TOK5 tokens for ALL 4 mesh batches.
TOK5 = N_TOKENS // N_CORES   # 512 tokens per core
YSCALE = 127.0 / (4.75 * 11.3137)  # int8 shuffle: 127 / (4.75 sigma_y)

TRACE = False
LAST_RESULTS = None
VERSION = 5       # active: token-sharded two-stage butterfly (see module docstring)
INT8_SHUFFLE = False

# ----------------------------------------------------------------------
# Host side: build the per-batch transfer matrices from the phases.
# ----------------------------------------------------------------------


def _bitrev(n):
    m = int(math.log2(n))
    perm = np.arange(n).reshape(n, 1)
    for _ in range(m):
        n1 = perm.shape[0] // 2
        perm = np.hstack((perm[:n1], perm[n1:]))
    return perm.squeeze(0)


def _forward_indices(length):
    idx = []
    ar = np.arange(length)
    for level in range(int(math.log2(length)) - 1):
        bs = 2 ** (level + 2)
        ind = ar.reshape(-1, length // bs, 2, bs // 2).transpose(0, 1, 3, 2)
        idx.append(ind.reshape(-1))
    return idx


def _build_W(phases):
    """phases (B, NLEV+1, L//2, 2) -> W (B, L, L) complex64 with out = x @ W."""
    B = phases.shape[0]
    br = _bitrev(L)
    fidx = _forward_indices(L)
    dc = np.array([[1.0, 1.0j], [1.0j, 1.0]], dtype=np.complex64)

    x = np.broadcast_to(np.eye(L, dtype=np.complex64), (B, L, L)).copy()
    x = x[..., br]
    for level in range(NLEV):
        x = x.reshape(B, L, L // 2, 2)
        ph = phases[:, level : level + 1, :, :]            # (B, 1, L//2, 2)
        x = x * np.exp(1j * ph.astype(np.complex64))
        x = x @ dc
        x = x.reshape(B, L, L)
        if level < NLEV - 1:
            x = x[..., fidx[level]]
    ph = phases[:, NLEV - 1 : NLEV, :, :].reshape(B, 1, L)
    x = x * np.exp(1j * ph.astype(np.complex64))
    x = x[..., br]
    return (x / np.float32(np.sqrt(L))).astype(np.complex64)


def _rev(v, n):
    r = 0
    for _ in range(n):
        r = (r << 1) | (v & 1)
        v >>= 1
    return r


def _stage_matrices(phases):
    """Two-stage factorization of the butterfly network.

    Stage A = input bitrev + levels 0..6 (perms 0..5, no trailing perm):
    block-diagonal; column-block g is fed by x columns {i : i = 8p + r},
    r = rev3(g).  Stage B = perm fidx[6] + levels 7..9 + final phase +
    final bitrev + scale: per-position 8x8 mixing across the 8 blocks.

    Returns per batch the PE stationaries:
      Astat[b, r] (128,128) cplx : lhsT with K=p (x idx 8p+r), M=pos.
      Bstat[b,t2] (128,128) cplx : lhsT with K = g*16+s (source y(g, t2*16+s)),
                                   M = v*8+m -> out col j = 128m + 8v + rev3(t2).
    Cross-component entries of the extracted B submatrix are exactly 0.
    """
    B_ = phases.shape[0]
    br = _bitrev(L)
    fidx = _forward_indices(L)
    dc = np.array([[1.0, 1.0j], [1.0j, 1.0]], dtype=np.complex64)

    def levels(x, lo, hi, pre_br=False, post_final=False, pre_perm=None):
        if pre_br:
            x = x[..., br]
        if pre_perm is not None:
            x = x[..., pre_perm]
        for level in range(lo, hi):
            x = x.reshape(B_, L, L // 2, 2)
            x = x * np.exp(1j * phases[:, level, None, :, :].astype(np.complex64))
            x = x @ dc
            x = x.reshape(B_, L, L)
            if level < NLEV - 1 and level != 6:
                x = x[..., fidx[level]]
        if post_final:
            x = x * np.exp(
                1j * phases[:, NLEV - 1, None, :, :].reshape(B_, 1, L).astype(np.complex64)
            )
            x = x[..., br]
            x = x / np.float32(np.sqrt(L))
        return x

    eye = np.broadcast_to(np.eye(L, dtype=np.complex64), (B_, L, L)).copy()
    A = levels(eye.copy(), 0, 7, pre_br=True)
    Bm = levels(eye.copy(), 7, NLEV, post_final=True, pre_perm=fidx[6])

    # Stage-A output row order: row' = s*8 + t2 for pos p'' = t2*16 + s, so the
    # inter-stage shuffle is one plain DMA per g: yA_g[:] -> Bin[g:128:8,:,:]
    # (dst partition k = s*8 + g, free = (t2, tok)).
    ar_ = np.arange(P)
    if _USPLIT[0]:
        # row' = u*64 + s*4 + t2' with t2 = u*4 + t2': the shuffle splits
        # into per-u 64-partition DMAs (finer B dependencies, same runs).
        u_ = ar_ >> 6
        s2 = (ar_ & 63) >> 2
        t2p = ar_ & 3
        posperm = (u_ * 4 + t2p) * 16 + s2         # row' -> p''
    else:
        posperm = (ar_ & 7) * 16 + (ar_ >> 3)      # row' -> p''
    Astat = np.empty((B_, 8, P, P), dtype=np.complex64)
    for r in range(8):
        g = _rev(r, 3)
        Astat[:, r] = A[:, ar_ * 8 + r][:, :, g * P + posperm]

    s_, g_ = np.divmod(ar_, 8)                     # k = s*8 + g
    v_, m_ = np.divmod(ar_, 8)
    Bstat = np.empty((B_, 8, P, P), dtype=np.complex64)
    for t2 in range(8):
        rows = g_ * P + t2 * 16 + s_
        cols = P * m_ + 8 * v_ + _rev(t2, 3)
        Bstat[:, t2] = Bm[:, rows][:, :, cols]
    return Astat, Bstat


# ----------------------------------------------------------------------
# Device side: complex matmul kernel (SPMD, one (batch, half) per core).
# ----------------------------------------------------------------------

_USPLIT = [False]

_CACHED_NC = None


def _build_program():
    nc = bacc.Bacc(
        "TRN2", target_bir_lowering=False, debug=False, num_devices=N_CORES
    )

    xr_d = nc.declare_dram_parameter("xr", [T, L], F32, isOutput=False)
    xi_d = nc.declare_dram_parameter("xi", [T, L], F32, isOutput=False)
    wr_d = nc.declare_dram_parameter("wr", [L, L], F32R, isOutput=False)
    wi_d = nc.declare_dram_parameter("wi", [L, L], F32R, isOutput=False)
    out_d = nc.declare_dram_parameter("out", [T, 2 * L], F32, isOutput=True)

    with tile.TileContext(nc) as tc:
        with (
            tc.tile_pool(name="const", bufs=1) as const_pool,
            tc.tile_pool(name="w", bufs=1) as w_pool,
            tc.tile_pool(name="x", bufs=3) as x_pool,
            tc.tile_pool(name="xt", bufs=2) as xt_pool,
            tc.tile_pool(name="osb", bufs=3) as o_pool,
            tc.tile_pool(name="ps", bufs=8, space=bass.MemorySpace.PSUM) as ps_pool,
        ):
            ident = const_pool.tile([P, P], F32)
            make_identity(nc, ident[:])

            # Warm the PE HAM while W streams in: dummy transposes keep the
            # tensor engine busy >3.4us so it reaches full clock before the
            # real matmuls start.
            warm = ps_pool.tile([P, 4 * P], F32, tag="ps")
            for _ in range(12):
                for j in range(4):
                    nc.tensor.transpose(
                        warm[:, j * P : (j + 1) * P], ident[:], ident[:]
                    )

            # Stream W into SBUF once: per k-chunk tiles (P x L), natural layout
            # (partition = contraction row within chunk, free = output column).
            # k-major order so the first token tile's accumulation can start
            # after only a few chunks have landed.
            w_sb = {}
            for k in range(KC):
                for nm, dram in (("wr", wr_d), ("wi", wi_d)):
                    t_ = w_pool.tile([P, L], F32R, tag=f"{nm}{k}")
                    nc.sync.dma_start(out=t_[:], in_=dram[k * P : (k + 1) * P, :])
                    w_sb[nm, k] = t_
                # -Wi derived on device: saves a third of the W stream, which
                # gates the kernel head while PE waits on weights.
                nwi = w_pool.tile([P, L], F32R, tag=f"nwi{k}")
                nc.vector.tensor_scalar_mul(nwi[:], w_sb["wi", k][:], -1.0)
                w_sb["nwi", k] = nwi

            for t in range(NT):
                rows = slice(t * P, (t + 1) * P)
                xr_rows = x_pool.tile([P, L], F32, tag="xr_rows")
                xi_rows = x_pool.tile([P, L], F32, tag="xi_rows")
                nc.sync.dma_start(out=xr_rows[:], in_=xr_d[rows, :])
                nc.sync.dma_start(out=xi_rows[:], in_=xi_d[rows, :])

                # Transpose the token tile: xT chunks live at
                # xT[:, k*P:(k+1)*P] = x_rows[:, k*P:(k+1)*P].T
                xrT = xt_pool.tile([P, L], F32R, tag="xrT")
                xiT = xt_pool.tile([P, L], F32R, tag="xiT")
                for src, dst in ((xr_rows, xrT), (xi_rows, xiT)):
                    for g in range(2):
                        tp = ps_pool.tile([P, 4 * P], F32, tag="ps")
                        for j in range(4):
                            k = g * 4 + j
                            nc.tensor.transpose(
                                tp[:, j * P : (j + 1) * P],
                                src[:, k * P : (k + 1) * P],
                                ident[:],
                            )
                        nc.scalar.copy(dst[:, g * 4 * P : (g + 1) * 4 * P], tp[:])

                # Accumulate the four real matmul outputs.
                #   re_n = sum_k xrT_k @ wr_k[n] + xiT_k @ nwi_k[n]
                #   im_n = sum_k xrT_k @ wi_k[n] + xiT_k @ wr_k[n]
                out_sb = o_pool.tile([P, L, 2], F32, tag="out_sb")
                for n in range(2):
                    ncol = slice(n * 512, (n + 1) * 512)
                    acc_re = ps_pool.tile([P, 512], F32, tag="ps")
                    acc_im = ps_pool.tile([P, 512], F32, tag="ps")
                    for k in range(KC):
                        xrT_k = xrT[:, k * P : (k + 1) * P]
                        xiT_k = xiT[:, k * P : (k + 1) * P]
                        first = k == 0
                        last = k == KC - 1
                        nc.tensor.matmul(
                            acc_re[:], xrT_k, w_sb["wr", k][:, ncol],
                            start=first, stop=False,
                        )
                        nc.tensor.matmul(
                            acc_re[:], xiT_k, w_sb["nwi", k][:, ncol],
                            start=False, stop=last,
                        )
                        nc.tensor.matmul(
                            acc_im[:], xrT_k, w_sb["wi", k][:, ncol],
                            start=first, stop=False,
                        )
                        nc.tensor.matmul(
                            acc_im[:], xiT_k, w_sb["wr", k][:, ncol],
                            start=False, stop=last,
                        )
                    # Interleave re/im into complex64 memory order.
                    nc.vector.tensor_copy(out_sb[:, n * 512 : (n + 1) * 512, 0], acc_re[:])
                    nc.vector.tensor_copy(out_sb[:, n * 512 : (n + 1) * 512, 1], acc_im[:])

                nc.sync.dma_start(out=out_d[rows, :], in_=out_sb[:])

    nc.compile()
    return nc


def _build_program_v3():
    # detect_race_conditions=False: the rust race detector false-positives on
    # the stepped-partition shuffle DMA vs writes to a *different* bin buffer
    # (disjoint SBUF regions sharing a shadow zone). Same-tensor deps are
    # tracked normally and validated by the CoreSim numeric check.
    nc = bacc.Bacc(
        "TRN2", target_bir_lowering=False, debug=False, num_devices=N_CORES,
        detect_race_conditions=False,
    )

    xr_d = nc.declare_dram_parameter("xr", [T, L], F32R, isOutput=False)
    xi_d = nc.declare_dram_parameter("xi", [T, L], F32R, isOutput=False)
    ar_d = nc.declare_dram_parameter("ar", [8 * P, P], F32R, isOutput=False)
    ai_d = nc.declare_dram_parameter("ai", [8 * P, P], F32R, isOutput=False)
    nai_d = nc.declare_dram_parameter("nai", [8 * P, P], F32R, isOutput=False)
    br_d = nc.declare_dram_parameter("br", [8 * P, P], BF16, isOutput=False)
    bi_d = nc.declare_dram_parameter("bi", [8 * P, P], BF16, isOutput=False)
    nbi_d = nc.declare_dram_parameter("nbi", [8 * P, P], BF16, isOutput=False)
    out_d = nc.declare_dram_parameter("out", [T, 2 * L], F32, isOutput=True)

    with tile.TileContext(nc) as tc:
        with (
            tc.tile_pool(name="const", bufs=1) as const_pool,
            tc.tile_pool(name="mats", bufs=1) as mat_pool,
            tc.tile_pool(name="x", bufs=8) as x_pool,
            tc.tile_pool(name="xt", bufs=20) as xt_pool,
            tc.tile_pool(name="ya", bufs=12) as ya_pool,
            tc.tile_pool(name="bin", bufs=1) as bin_pool,
            tc.tile_pool(name="yb", bufs=4) as yb_pool,
            tc.tile_pool(name="osb", bufs=4) as o_pool,
            tc.tile_pool(name="ps", bufs=8, space=bass.MemorySpace.PSUM) as ps_pool,
        ):
            ident = const_pool.tile([P, P], F32)
            make_identity(nc, ident[:])
            ident_h = const_pool.tile([P, P], BF16)
            nc.vector.tensor_copy(ident_h[:], ident[:])
            ident_r = const_pool.tile([P, P], F32R)
            nc.vector.tensor_copy(ident_r[:], ident[:])

            # HAM warmup while the (small) stationaries stream in.
            warm = ps_pool.tile([P, 4 * P], F32, tag="ps")
            for _ in range(22):
                for j in range(4):
                    nc.tensor.transpose(
                        warm[:, j * P : (j + 1) * P], ident[:], ident[:]
                    )

            # Persistent double-buffered shuffle destination; memset once so
            # downstream readers of the stepped-partition DMA writes are
            # observable (sim init tracking) — overlaps with warmup/mats DMA.
            bn_bufs = []
            bn_memsets = []
            for i in range(2):
                bnb = bin_pool.tile([P, 8, 2 * TC], BF16, tag=f"bin{i}")
                bn_memsets.append(nc.gpsimd.memset(bnb[:], 0.0))
                bn_bufs.append(bnb)

            # Mats go through the gpsimd SWDGE queues so the 48 dma_starts do
            # not serialize ahead of chunk-0 row loads on the two HWDGE queues.
            mats = {}
            for nm, dram, dt_ in (
                ("ar", ar_d, F32R), ("ai", ai_d, F32R), ("nai", nai_d, F32R),
                ("br", br_d, BF16), ("bi", bi_d, BF16), ("nbi", nbi_d, BF16),
            ):
                for r in range(8):
                    t_ = mat_pool.tile([P, P], dt_, tag=f"{nm}{r}")
                    nc.gpsimd.dma_start(out=t_[:], in_=dram[r * P : (r + 1) * P, :])
                    mats[nm, r] = t_

            def emit_front(ch):
                """T_in + stage A + shuffle for chunk ch."""
                tok0 = ch * TC
                rows = {}
                for pl, dram in ((0, xr_d), (1, xi_d)):
                    for tt in range(TC // P):
                        rt = x_pool.tile([P, P, 8], F32R, tag="rows")
                        r0 = tok0 + tt * P
                        eng = nc.scalar if (tt % 2) else nc.sync
                        eng.dma_start(out=rt[:], in_=dram[r0 : r0 + P, :])
                        rows[pl, tt] = rt

                xT = {}
                for pl in range(2):
                    for r in range(8):
                        tp = ps_pool.tile([P, 4 * P], F32R, tag="ps")
                        for tt in range(TC // P):
                            nc.tensor.transpose(
                                tp[:, tt * P : (tt + 1) * P],
                                rows[pl, tt][:, :, r],
                                ident_r[:],
                            )
                        dst = xt_pool.tile([P, TC], F32R, tag="xT")
                        nc.scalar.copy(dst[:], tp[:])
                        xT[pl, r] = dst

                yA = {}
                for r in range(8):
                    g = _rev(r, 3)
                    acr = ps_pool.tile([P, TC], F32, tag="ps")
                    aci = ps_pool.tile([P, TC], F32, tag="ps")
                    nc.tensor.matmul(acr[:], mats["ar", r], xT[0, r][:], start=True, stop=False)
                    nc.tensor.matmul(acr[:], mats["nai", r], xT[1, r][:], start=False, stop=True)
                    nc.tensor.matmul(aci[:], mats["ai", r], xT[0, r][:], start=True, stop=False)
                    nc.tensor.matmul(aci[:], mats["ar", r], xT[1, r][:], start=False, stop=True)
                    ya = ya_pool.tile([P, 2 * TC], BF16, tag="ya")
                    nc.vector.tensor_copy(ya[:, 0:TC], acr[:])
                    nc.vector.tensor_copy(ya[:, TC : 2 * TC], aci[:])
                    yA[g] = ya

                # shuffle: Bin[s*8+g, t2, :] = yA[g][s*8+t2, :] — one plain DMA
                # per g; one partition per SBUF port group on both sides.
                bn = bn_bufs[ch % 2]
                for g in range(8):
                    eng = nc.scalar if (g % 2) else nc.sync
                    eng.dma_start(out=bn[g:P:8, :, :], in_=yA[g][:])
                return bn

            def emit_back(ch, bn):
                """Stage B + T_out + interleave + store for chunk ch."""
                tok0 = ch * TC
                out_sb = []
                for tt in range(TC // P):
                    osb = o_pool.tile([P, 2 * L], F32, tag="osb")
                    out_sb.append(osb)
                for t2 in range(8):
                    obr = ps_pool.tile([P, TC], F32, tag="ps")
                    obi = ps_pool.tile([P, TC], F32, tag="ps")
                    b_re = bn[:, t2, 0:TC]
                    b_im = bn[:, t2, TC : 2 * TC]
                    nc.tensor.matmul(obr[:], mats["br", t2], b_re, start=True, stop=False)
                    nc.tensor.matmul(obr[:], mats["nbi", t2], b_im, start=False, stop=True)
                    nc.tensor.matmul(obi[:], mats["bi", t2], b_re, start=True, stop=False)
                    nc.tensor.matmul(obi[:], mats["br", t2], b_im, start=False, stop=True)
                    yb = yb_pool.tile([P, 2 * TC], BF16, tag="yb")
                    nc.scalar.copy(yb[:, 0:TC], obr[:])
                    nc.scalar.copy(yb[:, TC:], obi[:])

                    base = 2 * _rev(t2, 3)
                    for tt in range(TC // P):
                        tp2 = ps_pool.tile([P, 2, 16, 8], BF16, tag="ps")
                        nc.tensor.transpose(
                            tp2[:, 0], yb[:, tt * P : (tt + 1) * P], ident_h[:]
                        )
                        nc.tensor.transpose(
                            tp2[:, 1], yb[:, TC + tt * P : TC + (tt + 1) * P], ident_h[:]
                        )
                        osr = out_sb[tt][:].rearrange(
                            "q (m v lo) -> q lo v m", m=8, v=16, lo=16
                        )
                        nc.vector.tensor_copy(osr[:, base : base + 2, :, :], tp2[:])

                for tt in range(TC // P):
                    r0 = tok0 + tt * P
                    eng = nc.scalar if (tt % 2) else nc.sync
                    eng.dma_start(out=out_d[r0 : r0 + P, :], in_=out_sb[tt][:])

            # Software pipeline: back-half of chunk ch-1 is emitted after the
            # front-half (and shuffle issue) of chunk ch, so the PE stream has
            # B/T_out work in hand while chunk ch's shuffle is in flight.
            prev = None
            for ch in range(NCH):
                bn = emit_front(ch)
                if prev is not None:
                    emit_back(prev[0], prev[1])
                prev = (ch, bn)
            emit_back(prev[0], prev[1])

    nc.compile()
    return nc


def _build_program_v4():
    """Two-stage butterfly with all transposes moved to the host.

    x arrives pre-transposed and r-grouped in HBM as bf16 rows
    (plane, r, p) x tok, so stage-A moving operands are plain contiguous
    loads.  Stage A: acc[row', tok] = A_r^T x_r with the A/B stage
    matrices stationary; the stepped-partition SBUF shuffle regroups
    (s,t2) -> (s,g) partitions for stage B; stage-B results [j', tok]
    are stored position-major and the host undoes the butterfly output
    permutation + transpose.  No PE transposes, no output interleave.
    """
    nc = bacc.Bacc(
        "TRN2", target_bir_lowering=False, debug=False, num_devices=N_CORES,
        detect_race_conditions=False,
    )

    xt_d = nc.declare_dram_parameter("xt", [16 * P, T], BF16, isOutput=False)
    # All 48 stationaries packed: [P, (6 kinds x 8 r) * P] so one DMA with
    # 12 KiB partition lines loads everything (48 separate [P,P] DMAs would
    # be 256 B/line, descriptor-overhead-bound).
    mat_d = nc.declare_dram_parameter("mat", [P, 48 * P], BF16, isOutput=False)
    # bf16 output, chunk-major: row (ch, t2, j') holds [re TC | im TC] so
    # every store writes full contiguous 2 KiB dram rows.  Host upcasts.
    out_d = nc.declare_dram_parameter("out", [NCH * 8 * P, 2 * TC], BF16, isOutput=True)

    with tile.TileContext(nc) as tc:
        with (
            tc.tile_pool(name="const", bufs=1) as const_pool,
            tc.tile_pool(name="mats", bufs=1) as mat_pool,
            tc.tile_pool(name="x", bufs=1) as x_pool,
            tc.tile_pool(name="ya", bufs=3) as ya_pool,
            tc.tile_pool(name="bin", bufs=1) as bin_pool,
            tc.tile_pool(name="osb", bufs=1) as o_pool,
            tc.tile_pool(name="ps", bufs=4, space=bass.MemorySpace.PSUM) as ps_pool,
        ):
            ident = const_pool.tile([P, P], F32)
            make_identity(nc, ident[:])

            # Short HAM warmup (~4.7us cold) covering the chunk-0 DMA window.
            warm = ps_pool.tile([P, 2, TC], F32, tag="ps")
            for _ in range(11):
                for j in range(4):
                    nc.tensor.transpose(
                        warm[:, j // 2, (j % 2) * P : (j % 2 + 1) * P],
                        ident[:], ident[:],
                    )

            # One shuffle destination per chunk: all fronts are emitted before
            # any back, so no buffer reuse hazards at all.  The stepped-
            # partition shuffle DMAs fully overwrite each buffer; no init
            # needed (race detection is disabled).
            bn_bufs = []
            for i in range(NCH):
                bnb = bin_pool.tile([P, 8, 2, TC], BF16, tag=f"bin{i}")
                bn_bufs.append(bnb)

            # Packed stationaries on the gpsimd SWDGE queue: bulk load that
            # must not block the latency-critical HWDGE queues.
            mat_all = mat_pool.tile([P, 48 * P], BF16, tag="mat")
            nc.gpsimd.dma_start(out=mat_all[:], in_=mat_d[:, :])
            mats = {}
            for ki, nm in enumerate(("ar", "ai", "nai", "br", "bi", "nbi")):
                for r in range(8):
                    idx = ki * 8 + r
                    mats[nm, r] = mat_all[:, idx * P : (idx + 1) * P]

            # Full x panel upfront on the HWDGE queues (empty at the head):
            # 16 DMAs with 4 KiB partition lines; r=0's tiles land first so
            # stage A starts after ~1.5us.  Resident all run (64 KiB/part).
            xs = {}
            for r in range(8):
                for pl in range(2):
                    xtile = x_pool.tile([P, T], BF16, tag=f"x{pl}_{r}")
                    row0 = (pl * 8 + r) * P
                    eng = nc.scalar if (pl % 2) else nc.sync
                    eng.dma_start(out=xtile[:], in_=xt_d[row0 : row0 + P, :])
                    xs[pl, r] = xtile

            def emit_front(ch):
                """Stage A + cast + shuffle for chunk ch."""
                csl = slice(ch * TC, (ch + 1) * TC)
                bn = bn_bufs[ch]
                for r in range(8):
                    g = _rev(r, 3)
                    acc = ps_pool.tile([P, 2, TC], F32, tag="ps")
                    acr = acc[:, 0, :]
                    aci = acc[:, 1, :]
                    nc.tensor.matmul(acr, mats["ar", r], xs[0, r][:, csl], start=True, stop=False)
                    nc.tensor.matmul(aci, mats["ar", r], xs[1, r][:, csl], start=True, stop=False)
                    nc.tensor.matmul(aci, mats["ai", r], xs[0, r][:, csl], start=False, stop=True)
                    nc.tensor.matmul(acr, mats["nai", r], xs[1, r][:, csl], start=False, stop=True)
                    ya = ya_pool.tile([P, 2, TC], BF16, tag=f"ya{g}")
                    if r % 2:
                        nc.scalar.copy(ya[:], acc[:])
                    else:
                        nc.vector.tensor_copy(ya[:], acc[:])
                    # Shuffle: bn[s*8+g, t2, c, :] = ya[s*8+t2, c, :]
                    eng2 = nc.scalar if (g % 2) else nc.sync
                    eng2.dma_start(out=bn[g:P:8, :, :, :], in_=ya[:])
                return bn

            def emit_back(ch, bn):
                """Stage B + store (position-major, bf16) for chunk ch."""
                for t2 in range(8):
                    ob = ps_pool.tile([P, 2, TC], F32, tag="ps")
                    obr = ob[:, 0, :]
                    obi = ob[:, 1, :]
                    b_re = bn[:, t2, 0, :]
                    b_im = bn[:, t2, 1, :]
                    nc.tensor.matmul(obr, mats["br", t2], b_re, start=True, stop=False)
                    nc.tensor.matmul(obi, mats["br", t2], b_im, start=True, stop=False)
                    nc.tensor.matmul(obi, mats["bi", t2], b_re, start=False, stop=True)
                    nc.tensor.matmul(obr, mats["nbi", t2], b_im, start=False, stop=True)
                    osb = o_pool.tile([P, 2, TC], BF16, tag=f"osb{t2}")
                    if t2 % 2:
                        nc.scalar.copy(osb[:], ob[:])
                    else:
                        nc.vector.tensor_copy(osb[:], ob[:])
                    row0 = (ch * 8 + t2) * P
                    eng = nc.sync if (t2 % 2) else nc.scalar
                    eng.dma_start(out=out_d[row0 : row0 + P, :], in_=osb[:])

            # 2-chunk lookahead: B(ch) is emitted two fronts after F(ch), so
            # its shuffle has ~2 chunks of port time to land before the PE
            # reaches it.
            bns = {}
            order = []
            for ch in range(NCH):
                order.append(("F", ch))
                if ch >= 2:
                    order.append(("B", ch - 2))
            order += [("B", NCH - 2), ("B", NCH - 1)]
            for kind, ch in order:
                if kind == "F":
                    bns[ch] = emit_front(ch)
                else:
                    emit_back(ch, bns[ch])

    nc.compile()
    return nc


def _build_program_v5(int8_shuffle: bool):
    """Token-sharded two-stage butterfly: each core runs TOK5 tokens through
    all 4 mesh-batches.  Cuts the x input to 2 MiB/core (vs 8) at the cost of
    4 MiB of stationaries, minimizing SBUF-AXI-port traffic (the measured
    bottleneck).  Optional int8 inter-stage shuffle halves the port cost of
    the partition-regroup DMA (scales folded into the stage matrices on host;
    clamped vector casts; int8->bf16 upcast split across engines)."""
    nc = bacc.Bacc(
        "TRN2", target_bir_lowering=False, debug=False, num_devices=N_CORES,
        detect_race_conditions=False,
    )
    B = MESH_BATCH
    TCv = TOK5  # 512 tokens = one chunk per batch

    xt_d = nc.declare_dram_parameter("xt", [P, 8, 2, TCv], BF16, isOutput=False)
    mat_d = nc.declare_dram_parameter("mat", [P, B, 4, 8, P], BF16, isOutput=False)
    # out rows (b*2+q)*P + j', free [t2' in 4, c in 2, TCv]; t2 = q*4+t2'.
    out_d = nc.declare_dram_parameter("out", [B * 2 * P, 4 * 2 * TCv], BF16,
                                      isOutput=True)

    ydt = I8 if int8_shuffle else BF16

    with tile.TileContext(nc) as tc:
        with (
            tc.tile_pool(name="const", bufs=1) as const_pool,
            tc.tile_pool(name="mats", bufs=1) as mat_pool,
            tc.tile_pool(name="x", bufs=1) as x_pool,
            tc.tile_pool(name="ya", bufs=10) as ya_pool,
            tc.tile_pool(name="bin", bufs=1) as bin_pool,
            tc.tile_pool(name="bnh", bufs=2) as bnh_pool,
            tc.tile_pool(name="osb", bufs=3) as o_pool,
            tc.tile_pool(name="ps", bufs=4, space=bass.MemorySpace.PSUM) as ps_pool,
        ):
            ident = const_pool.tile([P, P], F32)
            make_identity(nc, ident[:])

            # HAM warmup covering the head DMA window (~9us of PE activity).
            warm = ps_pool.tile([P, 2, TCv], F32, tag="ps")
            for _ in range(10):
                for j in range(4):
                    nc.tensor.transpose(
                        warm[:, j // 2, (j % 2) * P : (j % 2 + 1) * P],
                        ident[:], ident[:],
                    )

            # Stationaries per batch (kinds ar, ai, br, bi) + derived
            # negations.  mat0 goes FIRST on the sync ring (it gates A0 and
            # the ring is otherwise empty, so it lands in ~2.5us); the x
            # panel follows on sync; mats 1-3 stream on the gpsimd ring.
            matb, negb = [], []
            for b in range(B):
                mb = mat_pool.tile([P, 4, 8, P], BF16, tag=f"mat{b}",
                                   name=f"mat{b}")
                ng = mat_pool.tile([P, 2, 8, P], BF16, tag=f"neg{b}",
                                   name=f"neg{b}")
                matb.append(mb)
                negb.append(ng)

            # Head loads: x split across the sync and scalar rings, the
            # stationaries FIFO-ordered (batch 0 first) on the gpsimd ring.
            # Measured best; variants that serialize x on one ring or put
            # mats on the HWDGE rings delay the first shuffles and lose
            # 5-10us.
            xs = x_pool.tile([P, 8, 2, TCv], BF16, tag="xs")
            nc.sync.dma_start(out=xs[:, 0:4], in_=xt_d[:, 0:4])
            nc.scalar.dma_start(out=xs[:, 4:8], in_=xt_d[:, 4:8])
            for b in range(B):
                nc.gpsimd.dma_start(out=matb[b][:], in_=mat_d[:, b])
            negs_done = [False] * B

            bn = []
            for b in range(B):
                bnb = bin_pool.tile([P, 8, 2, TCv], ydt, tag=f"bin{b}")
                bn.append(bnb)

            def emit_A(b):
                mb, ng = matb[b], negb[b]
                if not negs_done[b]:
                    # ng[:,0] = -ai here; -bi is deferred to emit_B so each
                    # negation only costs the scalar FIFO ~1us per phase.
                    nc.scalar.mul(ng[:, 0], mb[:, 1], -1.0)
                    negs_done[b] = True
                for r in range(8):
                    g = _rev(r, 3)
                    acc = ps_pool.tile([P, 2, TCv], F32, tag="ps")
                    ar_ = mb[:, 0, r, :]
                    ai_ = mb[:, 1, r, :]
                    nai = ng[:, 0, r, :]
                    xr_ = xs[:, r, 0, :]
                    xi_ = xs[:, r, 1, :]
                    nc.tensor.matmul(acc[:, 0, :], ar_, xr_, start=True, stop=False)
                    nc.tensor.matmul(acc[:, 0, :], nai, xi_, start=False, stop=True)
                    nc.tensor.matmul(acc[:, 1, :], ai_, xr_, start=True, stop=False)
                    nc.tensor.matmul(acc[:, 1, :], ar_, xi_, start=False, stop=True)
                    ya = ya_pool.tile([P, 2, TCv], ydt, tag="ya")
                    if r in (1, 3, 5):
                        nc.scalar.copy(ya[:], acc[:])
                    else:
                        nc.vector.tensor_copy(ya[:], acc[:])
                    # per-u half shuffles: B(b) t2<4 waits only on the u=0
                    # halves; same 2 KiB descriptor runs.
                    e_lo = nc.scalar if (g % 2) else nc.sync
                    e_hi = nc.sync if (g % 2) else nc.scalar
                    e_lo.dma_start(out=bn[b][g:P:8, 0:4], in_=ya[0:64])
                    e_hi.dma_start(out=bn[b][g:P:8, 4:8], in_=ya[64:128])

            def emit_casts(b):
                """int8 bn -> bf16 for the stage-B moving operand."""
                bh = bnh_pool.tile([P, 8, 2, TCv], BF16, tag=f"bnh{b % 2}")
                nc.gpsimd.tensor_copy(bh[:, 0:3], bn[b][:, 0:3])
                nc.vector.tensor_copy(bh[:, 3:5], bn[b][:, 3:5])
                nc.gpsimd.tensor_copy(bh[:, 5:7], bn[b][:, 5:7])
                nc.scalar.copy(bh[:, 7:8], bn[b][:, 7:8])
                return bh

            def emit_B(b, bh):
                mb, ng = matb[b], negb[b]
                # ng[:,1] = -bi (see emit_A)
                nc.scalar.mul(ng[:, 1], mb[:, 3], -1.0)
                src = bh if bh is not None else bn[b]
                osb = None
                for t2 in range(8):
                    acc = ps_pool.tile([P, 2, TCv], F32, tag="ps")
                    br_ = mb[:, 2, t2, :]
                    bi_ = mb[:, 3, t2, :]
                    nbi = ng[:, 1, t2, :]
                    yre = src[:, t2, 0, :]
                    yim = src[:, t2, 1, :]
                    nc.tensor.matmul(acc[:, 0, :], br_, yre, start=True, stop=False)
                    nc.tensor.matmul(acc[:, 0, :], nbi, yim, start=False, stop=True)
                    nc.tensor.matmul(acc[:, 1, :], bi_, yre, start=True, stop=False)
                    nc.tensor.matmul(acc[:, 1, :], br_, yim, start=False, stop=True)
                    if t2 % 2 == 0:
                        osb = o_pool.tile([P, 2, 2, TCv], BF16, tag="osb")
                    if t2 % 2:
                        nc.scalar.copy(osb[:, 1], acc[:])
                    else:
                        nc.vector.tensor_copy(osb[:, 0], acc[:])
                    if t2 % 2 == 1:
                        # out_d rows (b*2+q)*P+j', free [t2' in 4, c, t]; this
                        # pair covers t2' in {t2-1, t2} of quad q = t2 // 4.
                        q, t2p = divmod(t2 - 1, 4)
                        row0 = (b * 2 + q) * P
                        dst = out_d[row0 : row0 + P, :].rearrange(
                            "p (tp c t) -> p tp c t", tp=4, c=2
                        )[:, t2p : t2p + 2]
                        eng2 = nc.sync if (t2 % 4 == 1) else nc.scalar
                        eng2.dma_start(out=dst, in_=osb[:])

            # Pipeline: B(b) emitted after A(b+1) so the b-shuffle has a full
            # A-phase of DMA time to land before the PE needs it.
            emit_A(0)
            for b in range(B):
                if b + 1 < B:
                    emit_A(b + 1)
                bh = emit_casts(b) if int8_shuffle else None
                emit_B(b, bh)

    nc.compile()
    return nc


def _build_program_v8():
    """v7 with 4-kind stationaries (ar, ai, br, bi — 4 MiB instead of 6).
    The negated operands move to the moving side: xs carries a third plane
    nxi = -xi (negated once at the head), and bn carries a third slot
    nyim = -y_im (negated after each shuffle lands, split across vector and
    scalar).  re = ar@xr + ai@nxi, im = ai@xr + ar@xi, and likewise for B."""
    nc = bacc.Bacc(
        "TRN2", target_bir_lowering=False, debug=False, num_devices=N_CORES,
        detect_race_conditions=False,
    )
    B = MESH_BATCH
    TCv = TOK5

    xt_d = nc.declare_dram_parameter("xt", [P, 8, 2, TCv], BF16, isOutput=False)
    # kinds: 0=ar 1=ai 2=br 3=bi
    mat_d = nc.declare_dram_parameter("mat", [P, B, 4, 8, P], BF16, isOutput=False)
    out_d = nc.declare_dram_parameter("out", [B * 2 * P, 4 * 2 * TCv], BF16,
                                      isOutput=True)

    with tile.TileContext(nc) as tc:
        with (
            tc.tile_pool(name="const", bufs=1) as const_pool,
            tc.tile_pool(name="mats", bufs=1) as mat_pool,
            tc.tile_pool(name="x", bufs=1) as x_pool,
            tc.tile_pool(name="ya", bufs=14) as ya_pool,
            tc.tile_pool(name="bin", bufs=1) as bin_pool,
            tc.tile_pool(name="osb", bufs=4) as o_pool,
            tc.tile_pool(name="ps", bufs=4, space=bass.MemorySpace.PSUM) as ps_pool,
        ):
            # --- all input DMAs first: nothing blocks the rings ---
            xs = x_pool.tile([P, 8, 2, TCv], BF16, tag="xs")
            for i in range(4):
                nc.sync.dma_start(out=xs[:, 2 * i : 2 * i + 2],
                                  in_=xt_d[:, 2 * i : 2 * i + 2])
            xn = x_pool.tile([P, 8, TCv], BF16, tag="xn")

            matA, matB = [], []
            for b in range(B):
                mA = mat_pool.tile([P, 2, 8, P], BF16, tag=f"matA{b}",
                                   name=f"matA{b}")
                mB = mat_pool.tile([P, 2, 8, P], BF16, tag=f"matB{b}",
                                   name=f"matB{b}")
                matA.append(mA)
                matB.append(mB)
            # balance the mats across the scalar and gpsimd rings, earliest
            # batches first, so each matX_b lands just ahead of its phase.
            for b in range(B):
                eng = nc.scalar if b < 2 else nc.gpsimd
                eng.dma_start(out=matA[b][:], in_=mat_d[:, b, 0:2])
                eng.dma_start(out=matB[b][:], in_=mat_d[:, b, 2:4])

            ident = const_pool.tile([P, P], F32)
            make_identity(nc, ident[:])

            warm = ps_pool.tile([P, 2, TCv], F32, tag="ps")
            for _ in range(5):
                for j in range(4):
                    nc.tensor.transpose(
                        warm[:, j // 2, (j % 2) * P : (j % 2 + 1) * P],
                        ident[:], ident[:],
                    )

            # nxi planes, negated per-r on vector while mats stream in.
            for r in range(8):
                nc.vector.tensor_scalar_mul(xn[:, r], xs[:, r, 1], -1.0)

            bn = [
                bin_pool.tile([P, 8, 2, TCv], BF16, tag=f"bin{b}", name=f"bin{b}")
                for b in range(B)
            ]
            # nyim planes (negated post-shuffle)
            bnn = [
                bin_pool.tile([P, 8, TCv], BF16, tag=f"binn{b}", name=f"binn{b}")
                for b in range(B)
            ]

            def emit_A(b):
                mA = matA[b]
                for r in range(8):
                    g = _rev(r, 3)
                    acc = ps_pool.tile([P, 2, TCv], F32, tag="ps")
                    ar_ = mA[:, 0, r, :]
                    ai_ = mA[:, 1, r, :]
                    xr_ = xs[:, r, 0, :]
                    xi_ = xs[:, r, 1, :]
                    nxi = xn[:, r, :]
                    nc.tensor.matmul(acc[:, 0, :], ar_, xr_, start=True, stop=False)
                    nc.tensor.matmul(acc[:, 0, :], ai_, nxi, start=False, stop=True)
                    nc.tensor.matmul(acc[:, 1, :], ai_, xr_, start=True, stop=False)
                    nc.tensor.matmul(acc[:, 1, :], ar_, xi_, start=False, stop=True)
                    ya = ya_pool.tile([P, 2, TCv], BF16, tag="ya")
                    if r % 2:
                        nc.scalar.copy(ya[:], acc[:])
                    else:
                        nc.vector.tensor_copy(ya[:], acc[:])
                    eng2 = nc.scalar if (g % 2) else nc.sync
                    eng2.dma_start(out=bn[b][g:P:8], in_=ya[:])

            def emit_negs(b):
                # nyim = -yim once the b-shuffle has landed; split across
                # engines so neither eats a full phase.
                nc.vector.tensor_scalar_mul(bnn[b][:, 0:4], bn[b][:, 0:4, 1], -1.0)
                nc.scalar.mul(bnn[b][:, 4:8], bn[b][:, 4:8, 1], -1.0)

            def emit_B(b):
                mB = matB[b]
                osb = None
                for t2 in range(8):
                    acc = ps_pool.tile([P, 2, TCv], F32, tag="ps")
                    br_ = mB[:, 0, t2, :]
                    bi_ = mB[:, 1, t2, :]
                    yre = bn[b][:, t2, 0, :]
                    yim = bn[b][:, t2, 1, :]
                    nyim = bnn[b][:, t2, :]
                    nc.tensor.matmul(acc[:, 0, :], br_, yre, start=True, stop=False)
                    nc.tensor.matmul(acc[:, 0, :], bi_, nyim, start=False, stop=True)
                    nc.tensor.matmul(acc[:, 1, :], bi_, yre, start=True, stop=False)
                    nc.tensor.matmul(acc[:, 1, :], br_, yim, start=False, stop=True)
                    if t2 % 2 == 0:
                        osb = o_pool.tile([P, 2, 2, TCv], BF16, tag="osb")
                    if t2 % 2:
                        nc.scalar.copy(osb[:, 1], acc[:])
                    else:
                        nc.vector.tensor_copy(osb[:, 0], acc[:])
                    if t2 % 2 == 1:
                        # out_d rows (b*2+q)*P+j', free [t2' in 4, c, t]; this
                        # pair covers t2' in {t2-1, t2} of quad q = t2 // 4.
                        q, t2p = divmod(t2 - 1, 4)
                        row0 = (b * 2 + q) * P
                        dst = out_d[row0 : row0 + P, :].rearrange(
                            "p (tp c t) -> p tp c t", tp=4, c=2
                        )[:, t2p : t2p + 2]
                        eng2 = nc.sync if (t2 % 4 == 1) else nc.scalar
                        eng2.dma_start(out=dst, in_=osb[:])

            # B(b) two phases after A(b): each shuffle gets ~2 phases of
            # port time before the PE needs it.
            emit_A(0)
            emit_A(1)
            emit_A(2)
            emit_negs(0)
            emit_B(0)
            emit_A(3)
            emit_negs(1)
            emit_B(1)
            emit_negs(2)
            emit_B(2)
            emit_negs(3)
            emit_B(3)

    nc.compile()
    return nc


def _build_program_v7():
    """v5 pipeline (unpaired shuffle, balanced A/B interleave) with the v6
    6-kind stationaries, plus head fixes: DMA triggers are emitted before
    make_identity (which otherwise blocks the sync/gpsimd FIFOs for ~1.5us),
    stationaries are split into A/B-kind halves with batch 0 on the scalar
    HWDGE ring so stage A can start as early as possible, and the PE warmup
    is sized to the head gap."""
    nc = bacc.Bacc(
        "TRN2", target_bir_lowering=False, debug=False, num_devices=N_CORES,
        detect_race_conditions=False,
    )
    B = MESH_BATCH
    TCv = TOK5

    xt_d = nc.declare_dram_parameter("xt", [P, 8, 2, TCv], BF16, isOutput=False)
    # kinds: 0=ar 1=ai 2=nai 3=br 4=bi 5=nbi
    mat_d = nc.declare_dram_parameter("mat", [P, B, 6, 8, P], BF16, isOutput=False)
    out_d = nc.declare_dram_parameter("out", [B * 2 * P, 4 * 2 * TCv], BF16,
                                      isOutput=True)

    with tile.TileContext(nc) as tc:
        with (
            tc.tile_pool(name="const", bufs=1) as const_pool,
            tc.tile_pool(name="mats", bufs=1) as mat_pool,
            tc.tile_pool(name="x", bufs=1) as x_pool,
            tc.tile_pool(name="ya", bufs=6) as ya_pool,
            tc.tile_pool(name="bin", bufs=1) as bin_pool,
            tc.tile_pool(name="osb", bufs=3) as o_pool,
            tc.tile_pool(name="ps", bufs=4, space=bass.MemorySpace.PSUM) as ps_pool,
        ):
            # --- all input DMAs first: nothing blocks the rings ---
            xs = x_pool.tile([P, 8, 2, TCv], BF16, tag="xs")
            nc.sync.dma_start(out=xs[:, 0:4], in_=xt_d[:, 0:4])
            nc.sync.dma_start(out=xs[:, 4:8], in_=xt_d[:, 4:8])

            matA, matB = [], []
            for b in range(B):
                mA = mat_pool.tile([P, 3, 8, P], BF16, tag=f"matA{b}",
                                   name=f"matA{b}")
                mB = mat_pool.tile([P, 3, 8, P], BF16, tag=f"matB{b}",
                                   name=f"matB{b}")
                matA.append(mA)
                matB.append(mB)
            # batch 0 on the (otherwise idle) scalar ring for earliest arrival;
            # the rest stream in FIFO order on the gpsimd SWDGE ring.
            nc.scalar.dma_start(out=matA[0][:], in_=mat_d[:, 0, 0:3])
            nc.scalar.dma_start(out=matB[0][:], in_=mat_d[:, 0, 3:6])
            for b in range(1, B):
                nc.gpsimd.dma_start(out=matA[b][:], in_=mat_d[:, b, 0:3])
                nc.gpsimd.dma_start(out=matB[b][:], in_=mat_d[:, b, 3:6])

            ident = const_pool.tile([P, P], F32)
            make_identity(nc, ident[:])

            # Short HAM warmup sized to the ~4us head gap.
            warm = ps_pool.tile([P, 2, TCv], F32, tag="ps")
            for _ in range(5):
                for j in range(4):
                    nc.tensor.transpose(
                        warm[:, j // 2, (j % 2) * P : (j % 2 + 1) * P],
                        ident[:], ident[:],
                    )

            bn = [
                bin_pool.tile([P, 8, 2, TCv], BF16, tag=f"bin{b}", name=f"bin{b}")
                for b in range(B)
            ]

            def emit_A(b):
                mA = matA[b]
                for r in range(8):
                    g = _rev(r, 3)
                    acc = ps_pool.tile([P, 2, TCv], F32, tag="ps")
                    ar_ = mA[:, 0, r, :]
                    ai_ = mA[:, 1, r, :]
                    nai = mA[:, 2, r, :]
                    xr_ = xs[:, r, 0, :]
                    xi_ = xs[:, r, 1, :]
                    nc.tensor.matmul(acc[:, 0, :], ar_, xr_, start=True, stop=False)
                    nc.tensor.matmul(acc[:, 0, :], nai, xi_, start=False, stop=True)
                    nc.tensor.matmul(acc[:, 1, :], ai_, xr_, start=True, stop=False)
                    nc.tensor.matmul(acc[:, 1, :], ar_, xi_, start=False, stop=True)
                    ya = ya_pool.tile([P, 2, TCv], BF16, tag="ya")
                    if r % 2:
                        nc.scalar.copy(ya[:], acc[:])
                    else:
                        nc.vector.tensor_copy(ya[:], acc[:])
                    eng2 = nc.scalar if (g % 2) else nc.sync
                    eng2.dma_start(out=bn[b][g:P:8], in_=ya[:])

            def emit_B(b):
                mB = matB[b]
                osb = None
                for t2 in range(8):
                    acc = ps_pool.tile([P, 2, TCv], F32, tag="ps")
                    br_ = mB[:, 0, t2, :]
                    bi_ = mB[:, 1, t2, :]
                    nbi = mB[:, 2, t2, :]
                    yre = bn[b][:, t2, 0, :]
                    yim = bn[b][:, t2, 1, :]
                    nc.tensor.matmul(acc[:, 0, :], br_, yre, start=True, stop=False)
                    nc.tensor.matmul(acc[:, 0, :], nbi, yim, start=False, stop=True)
                    nc.tensor.matmul(acc[:, 1, :], bi_, yre, start=True, stop=False)
                    nc.tensor.matmul(acc[:, 1, :], br_, yim, start=False, stop=True)
                    if t2 % 2 == 0:
                        osb = o_pool.tile([P, 2, 2, TCv], BF16, tag="osb")
                    if t2 % 2:
                        nc.scalar.copy(osb[:, 1], acc[:])
                    else:
                        nc.vector.tensor_copy(osb[:, 0], acc[:])
                    if t2 % 2 == 1:
                        # out_d rows (b*2+q)*P+j', free [t2' in 4, c, t]; this
                        # pair covers t2' in {t2-1, t2} of quad q = t2 // 4.
                        q, t2p = divmod(t2 - 1, 4)
                        row0 = (b * 2 + q) * P
                        dst = out_d[row0 : row0 + P, :].rearrange(
                            "p (tp c t) -> p tp c t", tp=4, c=2
                        )[:, t2p : t2p + 2]
                        eng2 = nc.sync if (t2 % 4 == 1) else nc.scalar
                        eng2.dma_start(out=dst, in_=osb[:])

            for step in ("A0", "A1", "B0", "A2", "B1", "A3", "B2", "B3"):
                (emit_A if step[0] == "A" else emit_B)(int(step[1]))

    nc.compile()
    return nc


def _build_program_v6():
    """v5 + (a) all 6 stationary kinds from HBM (no on-device negation: the
    1.9us scalar negates stalled the PSUM-evac path at each phase head) and
    (b) batch-paired shuffle: stage-A results for batches (2p, 2p+1) share one
    ya2 tile, so each partition-regroup DMA moves 4 KiB runs (halves the
    descriptor count of the port-bound shuffle)."""
    nc = bacc.Bacc(
        "TRN2", target_bir_lowering=False, debug=False, num_devices=N_CORES,
        detect_race_conditions=False,
    )
    B = MESH_BATCH
    TCv = TOK5

    xt_d = nc.declare_dram_parameter("xt", [P, 8, 2, TCv], BF16, isOutput=False)
    # kinds: 0=ar 1=ai 2=nai 3=br 4=bi 5=nbi
    mat_d = nc.declare_dram_parameter("mat", [P, B, 6, 8, P], BF16, isOutput=False)
    out_d = nc.declare_dram_parameter("out", [B * 2 * P, 4 * 2 * TCv], BF16,
                                      isOutput=True)

    with tile.TileContext(nc) as tc:
        with (
            tc.tile_pool(name="const", bufs=1) as const_pool,
            tc.tile_pool(name="mats", bufs=1) as mat_pool,
            tc.tile_pool(name="x", bufs=1) as x_pool,
            tc.tile_pool(name="ya", bufs=9) as ya_pool,
            tc.tile_pool(name="bin", bufs=1) as bin_pool,
            tc.tile_pool(name="osb", bufs=2) as o_pool,
            tc.tile_pool(name="ps", bufs=4, space=bass.MemorySpace.PSUM) as ps_pool,
        ):
            ident = const_pool.tile([P, P], F32)
            make_identity(nc, ident[:])

            warm = ps_pool.tile([P, 2, TCv], F32, tag="ps")
            for _ in range(9):
                for j in range(4):
                    nc.tensor.transpose(
                        warm[:, j // 2, (j % 2) * P : (j % 2 + 1) * P],
                        ident[:], ident[:],
                    )

            xs = x_pool.tile([P, 8, 2, TCv], BF16, tag="xs")
            nc.sync.dma_start(out=xs[:, 0:4], in_=xt_d[:, 0:4])
            nc.sync.dma_start(out=xs[:, 4:8], in_=xt_d[:, 4:8])

            matb = []
            for b in range(B):
                mb = mat_pool.tile([P, 6, 8, P], BF16, tag=f"mat{b}")
                nc.gpsimd.dma_start(out=mb[:], in_=mat_d[:, b])
                matb.append(mb)

            bn2 = [
                bin_pool.tile([P, 8, 2, 2, TCv], BF16, tag=f"bin{p}",
                              name=f"bin{p}")
                for p in range(2)
            ]
            ya2 = {}

            def emit_A(b):
                mb = matb[b]
                pair, half = divmod(b, 2)
                for r in range(8):
                    g = _rev(r, 3)
                    acc = ps_pool.tile([P, 2, TCv], F32, tag="ps")
                    ar_ = mb[:, 0, r, :]
                    ai_ = mb[:, 1, r, :]
                    nai = mb[:, 2, r, :]
                    xr_ = xs[:, r, 0, :]
                    xi_ = xs[:, r, 1, :]
                    nc.tensor.matmul(acc[:, 0, :], ar_, xr_, start=True, stop=False)
                    nc.tensor.matmul(acc[:, 0, :], nai, xi_, start=False, stop=True)
                    nc.tensor.matmul(acc[:, 1, :], ai_, xr_, start=True, stop=False)
                    nc.tensor.matmul(acc[:, 1, :], ar_, xi_, start=False, stop=True)
                    if half == 0:
                        ya2[pair, g] = ya_pool.tile(
                            [P, 2, 2, TCv], BF16, tag="ya", name=f"ya{pair}_{g}"
                        )
                    dst = ya2[pair, g][:, half]
                    if r % 2:
                        nc.scalar.copy(dst, acc[:])
                    else:
                        nc.vector.tensor_copy(dst, acc[:])
                    if half == 1:
                        eng2 = nc.scalar if (g % 2) else nc.sync
                        eng2.dma_start(
                            out=bn2[pair][g:P:8], in_=ya2[pair, g][:]
                        )

            def emit_B(b):
                mb = matb[b]
                pair, half = divmod(b, 2)
                osb = None
                for t2 in range(8):
                    acc = ps_pool.tile([P, 2, TCv], F32, tag="ps")
                    br_ = mb[:, 3, t2, :]
                    bi_ = mb[:, 4, t2, :]
                    nbi = mb[:, 5, t2, :]
                    yre = bn2[pair][:, t2, half, 0, :]
                    yim = bn2[pair][:, t2, half, 1, :]
                    nc.tensor.matmul(acc[:, 0, :], br_, yre, start=True, stop=False)
                    nc.tensor.matmul(acc[:, 0, :], nbi, yim, start=False, stop=True)
                    nc.tensor.matmul(acc[:, 1, :], bi_, yre, start=True, stop=False)
                    nc.tensor.matmul(acc[:, 1, :], br_, yim, start=False, stop=True)
                    if t2 % 2 == 0:
                        osb = o_pool.tile([P, 2, 2, TCv], BF16, tag="osb")
                    if t2 % 2:
                        nc.scalar.copy(osb[:, 1], acc[:])
                    else:
                        nc.vector.tensor_copy(osb[:, 0], acc[:])
                    if t2 % 2 == 1:
                        # out_d rows (b*2+q)*P+j', free [t2' in 4, c, t]; this
                        # pair covers t2' in {t2-1, t2} of quad q = t2 // 4.
                        q, t2p = divmod(t2 - 1, 4)
                        row0 = (b * 2 + q) * P
                        dst = out_d[row0 : row0 + P, :].rearrange(
                            "p (tp c t) -> p tp c t", tp=4, c=2
                        )[:, t2p : t2p + 2]
                        eng2 = nc.sync if (t2 % 4 == 1) else nc.scalar
                        eng2.dma_start(out=dst, in_=osb[:])

            for step in ("A0", "A1", "A2", "B0", "A3", "B1", "B2", "B3"):
                (emit_A if step[0] == "A" else emit_B)(int(step[1]))

    nc.compile()
    return nc


_CACHED = {}


def _host_prep_v4(x_re, x_im, phases):
    """Host-side: transposed/r-grouped bf16 x per (core-half), bf16 mats."""
    import ml_dtypes

    Astat, Bstat = _stage_matrices(phases)
    bf = ml_dtypes.bfloat16
    ar = Astat.real.reshape(MESH_BATCH, 8 * P, P).astype(bf)
    ai = Astat.imag.reshape(MESH_BATCH, 8 * P, P).astype(bf)
    br = Bstat.real.reshape(MESH_BATCH, 8 * P, P).astype(bf)
    bi = Bstat.imag.reshape(MESH_BATCH, 8 * P, P).astype(bf)

    half = N_TOKENS // 2
    xts = []
    for h in range(2):
        planes = []
        for xp in (x_re, x_im):
            # [T, L] -> [L, T] -> (p, r) rows -> [r, p, T]
            xT = xp[h * half : (h + 1) * half].T.reshape(P, 8, half)
            planes.append(xT.transpose(1, 0, 2))
        xt = np.concatenate(planes, axis=0).reshape(16 * P, half)
        xts.append(np.ascontiguousarray(xt).astype(bf))
    return ar, ai, br, bi, xts


_JCOLS = None


def _jcols():
    global _JCOLS
    if _JCOLS is None:
        idx = np.arange(P)
        v_, m_ = np.divmod(idx, 8)
        _JCOLS = [P * m_ + 8 * v_ + _rev(t2, 3) for t2 in range(8)]
    return _JCOLS


def _host_prep_v5(x_re, x_im, phases, int8_shuffle, six_kinds=False):
    """Pack stationaries [P, B, K, 8, P] bf16 (K=4: ar, ai, br, bi; K=6 adds
    nai, nbi; int8 scales folded) and per-core x panels [P, 8, 2, TOK5]."""
    import ml_dtypes

    bf = ml_dtypes.bfloat16
    _USPLIT[0] = True
    try:
        Astat, Bstat = _stage_matrices(phases)
    finally:
        _USPLIT[0] = False
    s = YSCALE if int8_shuffle else 1.0
    Astat = Astat * np.float32(s)
    Bstat = Bstat * np.float32(1.0 / s)
    if six_kinds:
        # kinds: ar, ai, nai, br, bi, nbi
        mat = np.empty((P, MESH_BATCH, 6, 8, P), dtype=bf)
        mat[:, :, 0] = Astat.real.astype(bf).transpose(2, 0, 1, 3)
        mat[:, :, 1] = Astat.imag.astype(bf).transpose(2, 0, 1, 3)
        mat[:, :, 2] = (-Astat.imag).astype(bf).transpose(2, 0, 1, 3)
        mat[:, :, 3] = Bstat.real.astype(bf).transpose(2, 0, 1, 3)
        mat[:, :, 4] = Bstat.imag.astype(bf).transpose(2, 0, 1, 3)
        mat[:, :, 5] = (-Bstat.imag).astype(bf).transpose(2, 0, 1, 3)
    else:
        # mat[p, b, kind, r, m]
        mat = np.empty((P, MESH_BATCH, 4, 8, P), dtype=bf)
        mat[:, :, 0] = Astat.real.astype(bf).transpose(2, 0, 1, 3)
        mat[:, :, 1] = Astat.imag.astype(bf).transpose(2, 0, 1, 3)
        mat[:, :, 2] = Bstat.real.astype(bf).transpose(2, 0, 1, 3)
        mat[:, :, 3] = Bstat.imag.astype(bf).transpose(2, 0, 1, 3)
    mat = np.ascontiguousarray(mat)

    xts = []
    for c in range(N_CORES):
        t0 = c * TOK5
        panes = []
        for xp in (x_re, x_im):
            # [TOK5, L] -> [L, TOK5] -> [P, 8, TOK5]  (L-index = 8p + r)
            panes.append(xp[t0 : t0 + TOK5].T.reshape(P, 8, TOK5))
        xt = np.stack(panes, axis=2)  # [P, 8, 2, TOK5]
        xts.append(np.ascontiguousarray(xt).astype(bf))
    return mat, xts


def kernel(x_re: np.ndarray, x_im: np.ndarray, phases: np.ndarray) -> np.ndarray:
    global LAST_RESULTS

    x_re = np.ascontiguousarray(x_re, dtype=np.float32)
    x_im = np.ascontiguousarray(x_im, dtype=np.float32)
    phases = np.ascontiguousarray(phases, dtype=np.float32)

    if VERSION in (5, 6, 7, 8):
        six = VERSION in (6, 7)
        mat, xts = _host_prep_v5(x_re, x_im, phases, INT8_SHUFFLE and not six,
                                 six_kinds=six)
        key = (VERSION, INT8_SHUFFLE and not six)
        if key not in _CACHED:
            _CACHED[key] = (
                _build_program_v8() if VERSION == 8
                else _build_program_v7() if VERSION == 7
                else _build_program_v6() if six
                else _build_program_v5(INT8_SHUFFLE)
            )
        nc = _CACHED[key]
        in_maps = [{"xt": xts[c], "mat": mat} for c in range(N_CORES)]
        res = run_bass_kernel_spmd(nc, in_maps, list(range(N_CORES)), trace=TRACE)
        LAST_RESULTS = res
        jcols = _jcols()
        out = np.empty((MESH_BATCH, N_TOKENS, L), dtype=np.complex64)
        for c in range(N_CORES):
            t0 = c * TOK5
            sl = slice(t0, t0 + TOK5)
            arr = np.asarray(res.results[c]["out"], dtype=np.float32).reshape(
                MESH_BATCH, 2, P, 4, 2, TOK5
            )
            for t2 in range(8):
                q, t2p = divmod(t2, 4)
                for b in range(MESH_BATCH):
                    cplx = (
                        arr[b, q, :, t2p, 0, :] + 1j * arr[b, q, :, t2p, 1, :]
                    ).astype(np.complex64)  # [P, TOK5]
                    out[b, sl, jcols[t2]] = cplx
        return out

    half = N_TOKENS // 2
    in_maps = []
    if VERSION == 4:
        ar, ai, br, bi, xts = _host_prep_v4(x_re, x_im, phases)
        if 4 not in _CACHED:
            _CACHED[4] = _build_program_v4()
        nc = _CACHED[4]
        packed = []
        for b in range(MESH_BATCH):
            kinds = [ar[b], ai[b], -ai[b], br[b], bi[b], -bi[b]]
            m = np.concatenate([k.reshape(8, P, P) for k in kinds], axis=0)
            packed.append(np.ascontiguousarray(m.transpose(1, 0, 2).reshape(P, 48 * P)))
        for c in range(N_CORES):
            b, h = c // 2, c % 2
            in_maps.append({"xt": xts[h], "mat": packed[b]})
        res = run_bass_kernel_spmd(nc, in_maps, list(range(N_CORES)), trace=TRACE)
        LAST_RESULTS = res
        jcols = _jcols()
        out = np.empty((MESH_BATCH, N_TOKENS, L), dtype=np.complex64)
        for c in range(N_CORES):
            b, h = c // 2, c % 2
            # [NCH, 8, P, 2, TC] bf16 -> upcast once
            arr = np.asarray(res.results[c]["out"], dtype=np.float32).reshape(
                NCH, 8, P, 2, TC
            )
            sl = slice(h * half, (h + 1) * half)
            for t2 in range(8):
                cplx = (arr[:, t2, :, 0, :] + 1j * arr[:, t2, :, 1, :]).astype(
                    np.complex64
                )  # [NCH, P, TC]
                out[b, sl, jcols[t2]] = cplx.transpose(1, 0, 2).reshape(P, half)
        return out
    if VERSION == 2:
        W = _build_W(phases)                  # (B, L, L) complex64
        Wr = np.ascontiguousarray(W.real, dtype=np.float32)
        Wi = np.ascontiguousarray(W.imag, dtype=np.float32)
        if 2 not in _CACHED:
            _CACHED[2] = _build_program()
        nc = _CACHED[2]
        for c in range(N_CORES):
            b, h = c // 2, c % 2
            in_maps.append(
                {
                    "xr": x_re[h * half : (h + 1) * half],
                    "xi": x_im[h * half : (h + 1) * half],
                    "wr": Wr[b],
                    "wi": Wi[b],
                }
            )
    else:
        import ml_dtypes

        Astat, Bstat = _stage_matrices(phases)
        ar = np.ascontiguousarray(Astat.real.reshape(MESH_BATCH, 8 * P, P))
        ai = np.ascontiguousarray(Astat.imag.reshape(MESH_BATCH, 8 * P, P))
        br = Bstat.real.reshape(MESH_BATCH, 8 * P, P).astype(ml_dtypes.bfloat16)
        bi = Bstat.imag.reshape(MESH_BATCH, 8 * P, P).astype(ml_dtypes.bfloat16)
        if 3 not in _CACHED:
            _CACHED[3] = _build_program_v3()
        nc = _CACHED[3]
        for c in range(N_CORES):
            b, h = c // 2, c % 2
            in_maps.append(
                {
                    "xr": x_re[h * half : (h + 1) * half],
                    "xi": x_im[h * half : (h + 1) * half],
                    "ar": ar[b],
                    "ai": ai[b],
                    "nai": np.ascontiguousarray(-ai[b]),
                    "br": br[b],
                    "bi": bi[b],
                    "nbi": np.ascontiguousarray(-bi[b]),
                }
            )

    res = run_bass_kernel_spmd(nc, in_maps, list(range(N_CORES)), trace=TRACE)
    LAST_RESULTS = res

    out = np.empty((MESH_BATCH, N_TOKENS, L), dtype=np.complex64)
    for c in range(N_CORES):
        b, h = c // 2, c % 2
        out[b, h * half : (h + 1) * half] = (
            res.results[c]["out"].view(np.complex64).reshape(half, L)
        )
    return out



# revision 48
# speedup vs baseline: 1.1245x; 1.0128x over previous
"""Trainium2 Bass kernel for nn_BatchTrainableButterfly.

The reference applies, per mesh-batch b, a trainable butterfly network
(10 levels of phase shifters + 2x2 directional couplers with butterfly
permutations, plus a final phase layer and bit-reversals) to every token
row x[n, :].  For fixed phases the whole network is a linear map on
C^1024; it factors into two 128-wide PE stages (A = bitrev + levels 0..6,
block-diagonal over 8 column groups; B = levels 7..9 + final phase +
bitrev, an 8x8 mix across groups), which is 4x fewer MACs than the dense
1024x1024 matmul.

Active VERSION=5 (token-sharded): each core owns N_TOKENS/8 = 512 tokens
for ALL 4 mesh batches, which minimizes SBUF-AXI-port traffic — the
measured bottleneck (x 2 MiB + stationaries 4 MiB + inter-stage shuffle
8 MiB (counted twice: SBUF->SBUF reads AND writes cross the same 16 AXI
ports at ~435 GB/s) + out 8 MiB ~= 30 port-MiB ~= 72 us floor).  All
transposes live on the host: x arrives pre-transposed/r-grouped bf16;
out leaves position-major bf16 and the host inverts the permutation.
Per batch: 8 A-groups (4 bf16 matmuls N=512 each, fp32 PSUM pairs,
vector/scalar evacuation), a stepped-partition SBUF shuffle split into
per-(g, t2-half) DMAs (finer stage-B dependencies; stage-A's output row
order u*64+s*4+t2' makes the halves contiguous 64-partition slices),
then 8 B-groups with paired output stores (4 KiB dram lines).  Software
pipeline A0 A1 B0 A2 B1 A3 B2 B3 balances PE phases against the
port-saturated shuffle/store traffic; stationary negations (-ai, -bi)
are derived on device off the critical path.  ~98 us vs 113-118 us for
the v4 batch-sharded baseline; rel err 3.4e-3 (bf16).

An int8 shuffle variant (INT8_SHUFFLE) passes accuracy (1.1e-2) but is
slower: DVE/ACT 8-bit output casts run at half rate and gpsimd int8
upcasts at ~3 ns/elem, swamping the DMA savings.
"""

import math

import numpy as np

import concourse.tile as tile
from concourse import bacc, bass, mybir
from concourse.bass_utils import run_bass_kernel_spmd
from concourse.masks import make_identity

P = 128          # partitions
L = 1024         # butterfly length
N_TOKENS = 4096
MESH_BATCH = 4
N_CORES = 8
T = (N_TOKENS * MESH_BATCH) // N_CORES  # 2048 token-rows per core
NT = T // P      # 16 token tiles per core
KC = L // P      # 8 contraction chunks
NLEV = int(math.log2(L))  # 10

F32 = mybir.dt.float32
F32R = mybir.dt.float32r
BF16 = mybir.dt.bfloat16

TC = 512          # tokens per pipeline chunk (v3)
NCH = T // TC     # 4 chunks

I8 = mybir.dt.int8

# v5: token sharding — each core owns TOK5 tokens for ALL 4 mesh batches.
TOK5 = N_TOKENS // N_CORES   # 512 tokens per core
YSCALE = 127.0 / (4.75 * 11.3137)  # int8 shuffle: 127 / (4.75 sigma_y)

TRACE = False
LAST_RESULTS = None
VERSION = 5       # active: token-sharded two-stage butterfly (see module docstring)
INT8_SHUFFLE = False

# ----------------------------------------------------------------------
# Host side: build the per-batch transfer matrices from the phases.
# ----------------------------------------------------------------------


def _bitrev(n):
    m = int(math.log2(n))
    perm = np.arange(n).reshape(n, 1)
    for _ in range(m):
        n1 = perm.shape[0] // 2
        perm = np.hstack((perm[:n1], perm[n1:]))
    return perm.squeeze(0)


def _forward_indices(length):
    idx = []
    ar = np.arange(length)
    for level in range(int(math.log2(length)) - 1):
        bs = 2 ** (level + 2)
        ind = ar.reshape(-1, length // bs, 2, bs // 2).transpose(0, 1, 3, 2)
        idx.append(ind.reshape(-1))
    return idx


def _build_W(phases):
    """phases (B, NLEV+1, L//2, 2) -> W (B, L, L) complex64 with out = x @ W."""
    B = phases.shape[0]
    br = _bitrev(L)
    fidx = _forward_indices(L)
    dc = np.array([[1.0, 1.0j], [1.0j, 1.0]], dtype=np.complex64)

    x = np.broadcast_to(np.eye(L, dtype=np.complex64), (B, L, L)).copy()
    x = x[..., br]
    for level in range(NLEV):
        x = x.reshape(B, L, L // 2, 2)
        ph = phases[:, level : level + 1, :, :]            # (B, 1, L//2, 2)
        x = x * np.exp(1j * ph.astype(np.complex64))
        x = x @ dc
        x = x.reshape(B, L, L)
        if level < NLEV - 1:
            x = x[..., fidx[level]]
    ph = phases[:, NLEV - 1 : NLEV, :, :].reshape(B, 1, L)
    x = x * np.exp(1j * ph.astype(np.complex64))
    x = x[..., br]
    return (x / np.float32(np.sqrt(L))).astype(np.complex64)


def _rev(v, n):
    r = 0
    for _ in range(n):
        r = (r << 1) | (v & 1)
        v >>= 1
    return r


def _stage_matrices(phases):
    """Two-stage factorization of the butterfly network.

    Stage A = input bitrev + levels 0..6 (perms 0..5, no trailing perm):
    block-diagonal; column-block g is fed by x columns {i : i = 8p + r},
    r = rev3(g).  Stage B = perm fidx[6] + levels 7..9 + final phase +
    final bitrev + scale: per-position 8x8 mixing across the 8 blocks.

    Returns per batch the PE stationaries:
      Astat[b, r] (128,128) cplx : lhsT with K=p (x idx 8p+r), M=pos.
      Bstat[b,t2] (128,128) cplx : lhsT with K = g*16+s (source y(g, t2*16+s)),
                                   M = v*8+m -> out col j = 128m + 8v + rev3(t2).
    Cross-component entries of the extracted B submatrix are exactly 0.
    """
    B_ = phases.shape[0]
    br = _bitrev(L)
    fidx = _forward_indices(L)
    dc = np.array([[1.0, 1.0j], [1.0j, 1.0]], dtype=np.complex64)

    def levels(x, lo, hi, pre_br=False, post_final=False, pre_perm=None):
        if pre_br:
            x = x[..., br]
        if pre_perm is not None:
            x = x[..., pre_perm]
        for level in range(lo, hi):
            x = x.reshape(B_, L, L // 2, 2)
            x = x * np.exp(1j * phases[:, level, None, :, :].astype(np.complex64))
            x = x @ dc
            x = x.reshape(B_, L, L)
            if level < NLEV - 1 and level != 6:
                x = x[..., fidx[level]]
        if post_final:
            x = x * np.exp(
                1j * phases[:, NLEV - 1, None, :, :].reshape(B_, 1, L).astype(np.complex64)
            )
            x = x[..., br]
            x = x / np.float32(np.sqrt(L))
        return x

    eye = np.broadcast_to(np.eye(L, dtype=np.complex64), (B_, L, L)).copy()
    A = levels(eye.copy(), 0, 7, pre_br=True)
    Bm = levels(eye.copy(), 7, NLEV, post_final=True, pre_perm=fidx[6])

    # Stage-A output row order: row' = s*8 + t2 for pos p'' = t2*16 + s, so the
    # inter-stage shuffle is one plain DMA per g: yA_g[:] -> Bin[g:128:8,:,:]
    # (dst partition k = s*8 + g, free = (t2, tok)).
    ar_ = np.arange(P)
    if _USPLIT[0]:
        # row' = u*64 + s*4 + t2' with t2 = u*4 + t2': the shuffle splits
        # into per-u 64-partition DMAs (finer B dependencies, same runs).
        u_ = ar_ >> 6
        s2 = (ar_ & 63) >> 2
        t2p = ar_ & 3
        posperm = (u_ * 4 + t2p) * 16 + s2         # row' -> p''
    else:
        posperm = (ar_ & 7) * 16 + (ar_ >> 3)      # row' -> p''
    Astat = np.empty((B_, 8, P, P), dtype=np.complex64)
    for r in range(8):
        g = _rev(r, 3)
        Astat[:, r] = A[:, ar_ * 8 + r][:, :, g * P + posperm]

    s_, g_ = np.divmod(ar_, 8)                     # k = s*8 + g
    v_, m_ = np.divmod(ar_, 8)
    Bstat = np.empty((B_, 8, P, P), dtype=np.complex64)
    for t2 in range(8):
        rows = g_ * P + t2 * 16 + s_
        cols = P * m_ + 8 * v_ + _rev(t2, 3)
        Bstat[:, t2] = Bm[:, rows][:, :, cols]
    return Astat, Bstat


# ----------------------------------------------------------------------
# Device side: complex matmul kernel (SPMD, one (batch, half) per core).
# ----------------------------------------------------------------------

_USPLIT = [False]

_CACHED_NC = None


def _build_program():
    nc = bacc.Bacc(
        "TRN2", target_bir_lowering=False, debug=False, num_devices=N_CORES
    )

    xr_d = nc.declare_dram_parameter("xr", [T, L], F32, isOutput=False)
    xi_d = nc.declare_dram_parameter("xi", [T, L], F32, isOutput=False)
    wr_d = nc.declare_dram_parameter("wr", [L, L], F32R, isOutput=False)
    wi_d = nc.declare_dram_parameter("wi", [L, L], F32R, isOutput=False)
    out_d = nc.declare_dram_parameter("out", [T, 2 * L], F32, isOutput=True)

    with tile.TileContext(nc) as tc:
        with (
            tc.tile_pool(name="const", bufs=1) as const_pool,
            tc.tile_pool(name="w", bufs=1) as w_pool,
            tc.tile_pool(name="x", bufs=3) as x_pool,
            tc.tile_pool(name="xt", bufs=2) as xt_pool,
            tc.tile_pool(name="osb", bufs=3) as o_pool,
            tc.tile_pool(name="ps", bufs=8, space=bass.MemorySpace.PSUM) as ps_pool,
        ):
            ident = const_pool.tile([P, P], F32)
            make_identity(nc, ident[:])

            # Warm the PE HAM while W streams in: dummy transposes keep the
            # tensor engine busy >3.4us so it reaches full clock before the
            # real matmuls start.
            warm = ps_pool.tile([P, 4 * P], F32, tag="ps")
            for _ in range(12):
                for j in range(4):
                    nc.tensor.transpose(
                        warm[:, j * P : (j + 1) * P], ident[:], ident[:]
                    )

            # Stream W into SBUF once: per k-chunk tiles (P x L), natural layout
            # (partition = contraction row within chunk, free = output column).
            # k-major order so the first token tile's accumulation can start
            # after only a few chunks have landed.
            w_sb = {}
            for k in range(KC):
                for nm, dram in (("wr", wr_d), ("wi", wi_d)):
                    t_ = w_pool.tile([P, L], F32R, tag=f"{nm}{k}")
                    nc.sync.dma_start(out=t_[:], in_=dram[k * P : (k + 1) * P, :])
                    w_sb[nm, k] = t_
                # -Wi derived on device: saves a third of the W stream, which
                # gates the kernel head while PE waits on weights.
                nwi = w_pool.tile([P, L], F32R, tag=f"nwi{k}")
                nc.vector.tensor_scalar_mul(nwi[:], w_sb["wi", k][:], -1.0)
                w_sb["nwi", k] = nwi

            for t in range(NT):
                rows = slice(t * P, (t + 1) * P)
                xr_rows = x_pool.tile([P, L], F32, tag="xr_rows")
                xi_rows = x_pool.tile([P, L], F32, tag="xi_rows")
                nc.sync.dma_start(out=xr_rows[:], in_=xr_d[rows, :])
                nc.sync.dma_start(out=xi_rows[:], in_=xi_d[rows, :])

                # Transpose the token tile: xT chunks live at
                # xT[:, k*P:(k+1)*P] = x_rows[:, k*P:(k+1)*P].T
                xrT = xt_pool.tile([P, L], F32R, tag="xrT")
                xiT = xt_pool.tile([P, L], F32R, tag="xiT")
                for src, dst in ((xr_rows, xrT), (xi_rows, xiT)):
                    for g in range(2):
                        tp = ps_pool.tile([P, 4 * P], F32, tag="ps")
                        for j in range(4):
                            k = g * 4 + j
                            nc.tensor.transpose(
                                tp[:, j * P : (j + 1) * P],
                                src[:, k * P : (k + 1) * P],
                                ident[:],
                            )
                        nc.scalar.copy(dst[:, g * 4 * P : (g + 1) * 4 * P], tp[:])

                # Accumulate the four real matmul outputs.
                #   re_n = sum_k xrT_k @ wr_k[n] + xiT_k @ nwi_k[n]
                #   im_n = sum_k xrT_k @ wi_k[n] + xiT_k @ wr_k[n]
                out_sb = o_pool.tile([P, L, 2], F32, tag="out_sb")
                for n in range(2):
                    ncol = slice(n * 512, (n + 1) * 512)
                    acc_re = ps_pool.tile([P, 512], F32, tag="ps")
                    acc_im = ps_pool.tile([P, 512], F32, tag="ps")
                    for k in range(KC):
                        xrT_k = xrT[:, k * P : (k + 1) * P]
                        xiT_k = xiT[:, k * P : (k + 1) * P]
                        first = k == 0
                        last = k == KC - 1
                        nc.tensor.matmul(
                            acc_re[:], xrT_k, w_sb["wr", k][:, ncol],
                            start=first, stop=False,
                        )
                        nc.tensor.matmul(
                            acc_re[:], xiT_k, w_sb["nwi", k][:, ncol],
                            start=False, stop=last,
                        )
                        nc.tensor.matmul(
                            acc_im[:], xrT_k, w_sb["wi", k][:, ncol],
                            start=first, stop=False,
                        )
                        nc.tensor.matmul(
                            acc_im[:], xiT_k, w_sb["wr", k][:, ncol],
                            start=False, stop=last,
                        )
                    # Interleave re/im into complex64 memory order.
                    nc.vector.tensor_copy(out_sb[:, n * 512 : (n + 1) * 512, 0], acc_re[:])
                    nc.vector.tensor_copy(out_sb[:, n * 512 : (n + 1) * 512, 1], acc_im[:])

                nc.sync.dma_start(out=out_d[rows, :], in_=out_sb[:])

    nc.compile()
    return nc


def _build_program_v3():
    # detect_race_conditions=False: the rust race detector false-positives on
    # the stepped-partition shuffle DMA vs writes to a *different* bin buffer
    # (disjoint SBUF regions sharing a shadow zone). Same-tensor deps are
    # tracked normally and validated by the CoreSim numeric check.
    nc = bacc.Bacc(
        "TRN2", target_bir_lowering=False, debug=False, num_devices=N_CORES,
        detect_race_conditions=False,
    )

    xr_d = nc.declare_dram_parameter("xr", [T, L], F32R, isOutput=False)
    xi_d = nc.declare_dram_parameter("xi", [T, L], F32R, isOutput=False)
    ar_d = nc.declare_dram_parameter("ar", [8 * P, P], F32R, isOutput=False)
    ai_d = nc.declare_dram_parameter("ai", [8 * P, P], F32R, isOutput=False)
    nai_d = nc.declare_dram_parameter("nai", [8 * P, P], F32R, isOutput=False)
    br_d = nc.declare_dram_parameter("br", [8 * P, P], BF16, isOutput=False)
    bi_d = nc.declare_dram_parameter("bi", [8 * P, P], BF16, isOutput=False)
    nbi_d = nc.declare_dram_parameter("nbi", [8 * P, P], BF16, isOutput=False)
    out_d = nc.declare_dram_parameter("out", [T, 2 * L], F32, isOutput=True)

    with tile.TileContext(nc) as tc:
        with (
            tc.tile_pool(name="const", bufs=1) as const_pool,
            tc.tile_pool(name="mats", bufs=1) as mat_pool,
            tc.tile_pool(name="x", bufs=8) as x_pool,
            tc.tile_pool(name="xt", bufs=20) as xt_pool,
            tc.tile_pool(name="ya", bufs=12) as ya_pool,
            tc.tile_pool(name="bin", bufs=1) as bin_pool,
            tc.tile_pool(name="yb", bufs=4) as yb_pool,
            tc.tile_pool(name="osb", bufs=4) as o_pool,
            tc.tile_pool(name="ps", bufs=8, space=bass.MemorySpace.PSUM) as ps_pool,
        ):
            ident = const_pool.tile([P, P], F32)
            make_identity(nc, ident[:])
            ident_h = const_pool.tile([P, P], BF16)
            nc.vector.tensor_copy(ident_h[:], ident[:])
            ident_r = const_pool.tile([P, P], F32R)
            nc.vector.tensor_copy(ident_r[:], ident[:])

            # HAM warmup while the (small) stationaries stream in.
            warm = ps_pool.tile([P, 4 * P], F32, tag="ps")
            for _ in range(22):
                for j in range(4):
                    nc.tensor.transpose(
                        warm[:, j * P : (j + 1) * P], ident[:], ident[:]
                    )

            # Persistent double-buffered shuffle destination; memset once so
            # downstream readers of the stepped-partition DMA writes are
            # observable (sim init tracking) — overlaps with warmup/mats DMA.
            bn_bufs = []
            bn_memsets = []
            for i in range(2):
                bnb = bin_pool.tile([P, 8, 2 * TC], BF16, tag=f"bin{i}")
                bn_memsets.append(nc.gpsimd.memset(bnb[:], 0.0))
                bn_bufs.append(bnb)

            # Mats go through the gpsimd SWDGE queues so the 48 dma_starts do
            # not serialize ahead of chunk-0 row loads on the two HWDGE queues.
            mats = {}
            for nm, dram, dt_ in (
                ("ar", ar_d, F32R), ("ai", ai_d, F32R), ("nai", nai_d, F32R),
                ("br", br_d, BF16), ("bi", bi_d, BF16), ("nbi", nbi_d, BF16),
            ):
                for r in range(8):
                    t_ = mat_pool.tile([P, P], dt_, tag=f"{nm}{r}")
                    nc.gpsimd.dma_start(out=t_[:], in_=dram[r * P : (r + 1) * P, :])
                    mats[nm, r] = t_

            def emit_front(ch):
                """T_in + stage A + shuffle for chunk ch."""
                tok0 = ch * TC
                rows = {}
                for pl, dram in ((0, xr_d), (1, xi_d)):
                    for tt in range(TC // P):
                        rt = x_pool.tile([P, P, 8], F32R, tag="rows")
                        r0 = tok0 + tt * P
                        eng = nc.scalar if (tt % 2) else nc.sync
                        eng.dma_start(out=rt[:], in_=dram[r0 : r0 + P, :])
                        rows[pl, tt] = rt

                xT = {}
                for pl in range(2):
                    for r in range(8):
                        tp = ps_pool.tile([P, 4 * P], F32R, tag="ps")
                        for tt in range(TC // P):
                            nc.tensor.transpose(
                                tp[:, tt * P : (tt + 1) * P],
                                rows[pl, tt][:, :, r],
                                ident_r[:],
                            )
                        dst = xt_pool.tile([P, TC], F32R, tag="xT")
                        nc.scalar.copy(dst[:], tp[:])
                        xT[pl, r] = dst

                yA = {}
                for r in range(8):
                    g = _rev(r, 3)
                    acr = ps_pool.tile([P, TC], F32, tag="ps")
                    aci = ps_pool.tile([P, TC], F32, tag="ps")
                    nc.tensor.matmul(acr[:], mats["ar", r], xT[0, r][:], start=True, stop=False)
                    nc.tensor.matmul(acr[:], mats["nai", r], xT[1, r][:], start=False, stop=True)
                    nc.tensor.matmul(aci[:], mats["ai", r], xT[0, r][:], start=True, stop=False)
                    nc.tensor.matmul(aci[:], mats["ar", r], xT[1, r][:], start=False, stop=True)
                    ya = ya_pool.tile([P, 2 * TC], BF16, tag="ya")
                    nc.vector.tensor_copy(ya[:, 0:TC], acr[:])
                    nc.vector.tensor_copy(ya[:, TC : 2 * TC], aci[:])
                    yA[g] = ya

                # shuffle: Bin[s*8+g, t2, :] = yA[g][s*8+t2, :] — one plain DMA
                # per g; one partition per SBUF port group on both sides.
                bn = bn_bufs[ch % 2]
                for g in range(8):
                    eng = nc.scalar if (g % 2) else nc.sync
                    eng.dma_start(out=bn[g:P:8, :, :], in_=yA[g][:])
                return bn

            def emit_back(ch, bn):
                """Stage B + T_out + interleave + store for chunk ch."""
                tok0 = ch * TC
                out_sb = []
                for tt in range(TC // P):
                    osb = o_pool.tile([P, 2 * L], F32, tag="osb")
                    out_sb.append(osb)
                for t2 in range(8):
                    obr = ps_pool.tile([P, TC], F32, tag="ps")
                    obi = ps_pool.tile([P, TC], F32, tag="ps")
                    b_re = bn[:, t2, 0:TC]
                    b_im = bn[:, t2, TC : 2 * TC]
                    nc.tensor.matmul(obr[:], mats["br", t2], b_re, start=True, stop=False)
                    nc.tensor.matmul(obr[:], mats["nbi", t2], b_im, start=False, stop=True)
                    nc.tensor.matmul(obi[:], mats["bi", t2], b_re, start=True, stop=False)
                    nc.tensor.matmul(obi[:], mats["br", t2], b_im, start=False, stop=True)
                    yb = yb_pool.tile([P, 2 * TC], BF16, tag="yb")
                    nc.scalar.copy(yb[:, 0:TC], obr[:])
                    nc.scalar.copy(yb[:, TC:], obi[:])

                    base = 2 * _rev(t2, 3)
                    for tt in range(TC // P):
                        tp2 = ps_pool.tile([P, 2, 16, 8], BF16, tag="ps")
                        nc.tensor.transpose(
                            tp2[:, 0], yb[:, tt * P : (tt + 1) * P], ident_h[:]
                        )
                        nc.tensor.transpose(
                            tp2[:, 1], yb[:, TC + tt * P : TC + (tt + 1) * P], ident_h[:]
                        )
                        osr = out_sb[tt][:].rearrange(
                            "q (m v lo) -> q lo v m", m=8, v=16, lo=16
                        )
                        nc.vector.tensor_copy(osr[:, base : base + 2, :, :], tp2[:])

                for tt in range(TC // P):
                    r0 = tok0 + tt * P
                    eng = nc.scalar if (tt % 2) else nc.sync
                    eng.dma_start(out=out_d[r0 : r0 + P, :], in_=out_sb[tt][:])

            # Software pipeline: back-half of chunk ch-1 is emitted after the
            # front-half (and shuffle issue) of chunk ch, so the PE stream has
            # B/T_out work in hand while chunk ch's shuffle is in flight.
            prev = None
            for ch in range(NCH):
                bn = emit_front(ch)
                if prev is not None:
                    emit_back(prev[0], prev[1])
                prev = (ch, bn)
            emit_back(prev[0], prev[1])

    nc.compile()
    return nc


def _build_program_v4():
    """Two-stage butterfly with all transposes moved to the host.

    x arrives pre-transposed and r-grouped in HBM as bf16 rows
    (plane, r, p) x tok, so stage-A moving operands are plain contiguous
    loads.  Stage A: acc[row', tok] = A_r^T x_r with the A/B stage
    matrices stationary; the stepped-partition SBUF shuffle regroups
    (s,t2) -> (s,g) partitions for stage B; stage-B results [j', tok]
    are stored position-major and the host undoes the butterfly output
    permutation + transpose.  No PE transposes, no output interleave.
    """
    nc = bacc.Bacc(
        "TRN2", target_bir_lowering=False, debug=False, num_devices=N_CORES,
        detect_race_conditions=False,
    )

    xt_d = nc.declare_dram_parameter("xt", [16 * P, T], BF16, isOutput=False)
    # All 48 stationaries packed: [P, (6 kinds x 8 r) * P] so one DMA with
    # 12 KiB partition lines loads everything (48 separate [P,P] DMAs would
    # be 256 B/line, descriptor-overhead-bound).
    mat_d = nc.declare_dram_parameter("mat", [P, 48 * P], BF16, isOutput=False)
    # bf16 output, chunk-major: row (ch, t2, j') holds [re TC | im TC] so
    # every store writes full contiguous 2 KiB dram rows.  Host upcasts.
    out_d = nc.declare_dram_parameter("out", [NCH * 8 * P, 2 * TC], BF16, isOutput=True)

    with tile.TileContext(nc) as tc:
        with (
            tc.tile_pool(name="const", bufs=1) as const_pool,
            tc.tile_pool(name="mats", bufs=1) as mat_pool,
            tc.tile_pool(name="x", bufs=1) as x_pool,
            tc.tile_pool(name="ya", bufs=3) as ya_pool,
            tc.tile_pool(name="bin", bufs=1) as bin_pool,
            tc.tile_pool(name="osb", bufs=1) as o_pool,
            tc.tile_pool(name="ps", bufs=4, space=bass.MemorySpace.PSUM) as ps_pool,
        ):
            ident = const_pool.tile([P, P], F32)
            make_identity(nc, ident[:])

            # Short HAM warmup (~4.7us cold) covering the chunk-0 DMA window.
            warm = ps_pool.tile([P, 2, TC], F32, tag="ps")
            for _ in range(11):
                for j in range(4):
                    nc.tensor.transpose(
                        warm[:, j // 2, (j % 2) * P : (j % 2 + 1) * P],
                        ident[:], ident[:],
                    )

            # One shuffle destination per chunk: all fronts are emitted before
            # any back, so no buffer reuse hazards at all.  The stepped-
            # partition shuffle DMAs fully overwrite each buffer; no init
            # needed (race detection is disabled).
            bn_bufs = []
            for i in range(NCH):
                bnb = bin_pool.tile([P, 8, 2, TC], BF16, tag=f"bin{i}")
                bn_bufs.append(bnb)

            # Packed stationaries on the gpsimd SWDGE queue: bulk load that
            # must not block the latency-critical HWDGE queues.
            mat_all = mat_pool.tile([P, 48 * P], BF16, tag="mat")
            nc.gpsimd.dma_start(out=mat_all[:], in_=mat_d[:, :])
            mats = {}
            for ki, nm in enumerate(("ar", "ai", "nai", "br", "bi", "nbi")):
                for r in range(8):
                    idx = ki * 8 + r
                    mats[nm, r] = mat_all[:, idx * P : (idx + 1) * P]

            # Full x panel upfront on the HWDGE queues (empty at the head):
            # 16 DMAs with 4 KiB partition lines; r=0's tiles land first so
            # stage A starts after ~1.5us.  Resident all run (64 KiB/part).
            xs = {}
            for r in range(8):
                for pl in range(2):
                    xtile = x_pool.tile([P, T], BF16, tag=f"x{pl}_{r}")
                    row0 = (pl * 8 + r) * P
                    eng = nc.scalar if (pl % 2) else nc.sync
                    eng.dma_start(out=xtile[:], in_=xt_d[row0 : row0 + P, :])
                    xs[pl, r] = xtile

            def emit_front(ch):
                """Stage A + cast + shuffle for chunk ch."""
                csl = slice(ch * TC, (ch + 1) * TC)
                bn = bn_bufs[ch]
                for r in range(8):
                    g = _rev(r, 3)
                    acc = ps_pool.tile([P, 2, TC], F32, tag="ps")
                    acr = acc[:, 0, :]
                    aci = acc[:, 1, :]
                    nc.tensor.matmul(acr, mats["ar", r], xs[0, r][:, csl], start=True, stop=False)
                    nc.tensor.matmul(aci, mats["ar", r], xs[1, r][:, csl], start=True, stop=False)
                    nc.tensor.matmul(aci, mats["ai", r], xs[0, r][:, csl], start=False, stop=True)
                    nc.tensor.matmul(acr, mats["nai", r], xs[1, r][:, csl], start=False, stop=True)
                    ya = ya_pool.tile([P, 2, TC], BF16, tag=f"ya{g}")
                    if r % 2:
                        nc.scalar.copy(ya[:], acc[:])
                    else:
                        nc.vector.tensor_copy(ya[:], acc[:])
                    # Shuffle: bn[s*8+g, t2, c, :] = ya[s*8+t2, c, :]
                    eng2 = nc.scalar if (g % 2) else nc.sync
                    eng2.dma_start(out=bn[g:P:8, :, :, :], in_=ya[:])
                return bn

            def emit_back(ch, bn):
                """Stage B + store (position-major, bf16) for chunk ch."""
                for t2 in range(8):
                    ob = ps_pool.tile([P, 2, TC], F32, tag="ps")
                    obr = ob[:, 0, :]
                    obi = ob[:, 1, :]
                    b_re = bn[:, t2, 0, :]
                    b_im = bn[:, t2, 1, :]
                    nc.tensor.matmul(obr, mats["br", t2], b_re, start=True, stop=False)
                    nc.tensor.matmul(obi, mats["br", t2], b_im, start=True, stop=False)
                    nc.tensor.matmul(obi, mats["bi", t2], b_re, start=False, stop=True)
                    nc.tensor.matmul(obr, mats["nbi", t2], b_im, start=False, stop=True)
                    osb = o_pool.tile([P, 2, TC], BF16, tag=f"osb{t2}")
                    if t2 % 2:
                        nc.scalar.copy(osb[:], ob[:])
                    else:
                        nc.vector.tensor_copy(osb[:], ob[:])
                    row0 = (ch * 8 + t2) * P
                    eng = nc.sync if (t2 % 2) else nc.scalar
                    eng.dma_start(out=out_d[row0 : row0 + P, :], in_=osb[:])

            # 2-chunk lookahead: B(ch) is emitted two fronts after F(ch), so
            # its shuffle has ~2 chunks of port time to land before the PE
            # reaches it.
            bns = {}
            order = []
            for ch in range(NCH):
                order.append(("F", ch))
                if ch >= 2:
                    order.append(("B", ch - 2))
            order += [("B", NCH - 2), ("B", NCH - 1)]
            for kind, ch in order:
                if kind == "F":
                    bns[ch] = emit_front(ch)
                else:
                    emit_back(ch, bns[ch])

    nc.compile()
    return nc


def _build_program_v5(int8_shuffle: bool):
    """Token-sharded two-stage butterfly: each core runs TOK5 tokens through
    all 4 mesh-batches.  Cuts the x input to 2 MiB/core (vs 8) at the cost of
    4 MiB of stationaries, minimizing SBUF-AXI-port traffic (the measured
    bottleneck).  Optional int8 inter-stage shuffle halves the port cost of
    the partition-regroup DMA (scales folded into the stage matrices on host;
    clamped vector casts; int8->bf16 upcast split across engines)."""
    nc = bacc.Bacc(
        "TRN2", target_bir_lowering=False, debug=False, num_devices=N_CORES,
        detect_race_conditions=False,
    )
    B = MESH_BATCH
    TCv = TOK5  # 512 tokens = one chunk per batch

    xt_d = nc.declare_dram_parameter("xt", [P, 8, 2, TCv], BF16, isOutput=False)
    mat_d = nc.declare_dram_parameter("mat", [P, B, 4, 8, P], BF16, isOutput=False)
    # out rows (b*2+q)*P + j', free [t2' in 4, c in 2, TCv]; t2 = q*4+t2'.
    out_d = nc.declare_dram_parameter("out", [B * 2 * P, 4 * 2 * TCv], BF16,
                                      isOutput=True)

    ydt = I8 if int8_shuffle else BF16

    with tile.TileContext(nc) as tc:
        with (
            tc.tile_pool(name="const", bufs=1) as const_pool,
            tc.tile_pool(name="mats", bufs=1) as mat_pool,
            tc.tile_pool(name="x", bufs=1) as x_pool,
            tc.tile_pool(name="ya", bufs=10) as ya_pool,
            tc.tile_pool(name="bin", bufs=1) as bin_pool,
            tc.tile_pool(name="bnh", bufs=2) as bnh_pool,
            tc.tile_pool(name="osb", bufs=3) as o_pool,
            tc.tile_pool(name="ps", bufs=4, space=bass.MemorySpace.PSUM) as ps_pool,
        ):
            ident = const_pool.tile([P, P], F32)
            make_identity(nc, ident[:])

            # HAM warmup covering the head DMA window (~9us of PE activity).
            warm = ps_pool.tile([P, 2, TCv], F32, tag="ps")
            for _ in range(10):
                for j in range(4):
                    nc.tensor.transpose(
                        warm[:, j // 2, (j % 2) * P : (j % 2 + 1) * P],
                        ident[:], ident[:],
                    )

            # Stationaries per batch (kinds ar, ai, br, bi) + derived
            # negations.  mat0 goes FIRST on the sync ring (it gates A0 and
            # the ring is otherwise empty, so it lands in ~2.5us); the x
            # panel follows on sync; mats 1-3 stream on the gpsimd ring.
            matb, negb = [], []
            for b in range(B):
                mb = mat_pool.tile([P, 4, 8, P], BF16, tag=f"mat{b}",
                                   name=f"mat{b}")
                ng = mat_pool.tile([P, 2, 8, P], BF16, tag=f"neg{b}",
                                   name=f"neg{b}")
                matb.append(mb)
                negb.append(ng)

            # Head loads: x split across the sync and scalar rings, the
            # stationaries FIFO-ordered (batch 0 first) on the gpsimd ring.
            # Measured best; variants that serialize x on one ring or put
            # mats on the HWDGE rings delay the first shuffles and lose
            # 5-10us.
            xs = x_pool.tile([P, 8, 2, TCv], BF16, tag="xs")
            nc.sync.dma_start(out=xs[:, 0:4], in_=xt_d[:, 0:4])
            nc.scalar.dma_start(out=xs[:, 4:8], in_=xt_d[:, 4:8])
            for b in range(B):
                nc.gpsimd.dma_start(out=matb[b][:], in_=mat_d[:, b])
            negs_done = [False] * B

            bn = []
            for b in range(B):
                bnb = bin_pool.tile([P, 8, 2, TCv], ydt, tag=f"bin{b}")
                bn.append(bnb)

            def emit_A(b):
                mb, ng = matb[b], negb[b]
                if not negs_done[b]:
                    # ng[:,0] = -ai here; -bi is deferred to emit_B so each
                    # negation only costs the scalar FIFO ~1us per phase.
                    nc.scalar.mul(ng[:, 0], mb[:, 1], -1.0)
                    negs_done[b] = True
                for r in range(8):
                    g = _rev(r, 3)
                    acc = ps_pool.tile([P, 2, TCv], F32, tag="ps")
                    ar_ = mb[:, 0, r, :]
                    ai_ = mb[:, 1, r, :]
                    nai = ng[:, 0, r, :]
                    xr_ = xs[:, r, 0, :]
                    xi_ = xs[:, r, 1, :]
                    nc.tensor.matmul(acc[:, 0, :], ar_, xr_, start=True, stop=False)
                    nc.tensor.matmul(acc[:, 0, :], nai, xi_, start=False, stop=True)
                    nc.tensor.matmul(acc[:, 1, :], ai_, xr_, start=True, stop=False)
                    nc.tensor.matmul(acc[:, 1, :], ar_, xi_, start=False, stop=True)
                    ya = ya_pool.tile([P, 2, TCv], ydt, tag="ya")
                    if r in (1, 3, 5):
                        nc.scalar.copy(ya[:], acc[:])
                    else:
                        nc.vector.tensor_copy(ya[:], acc[:])
                    # per-u half shuffles: B(b) t2<4 waits only on the u=0
                    # halves; same 2 KiB descriptor runs.
                    e_lo = nc.scalar if (g % 2) else nc.sync
                    e_hi = nc.sync if (g % 2) else nc.scalar
                    e_lo.dma_start(out=bn[b][g:P:8, 0:4], in_=ya[0:64])
                    e_hi.dma_start(out=bn[b][g:P:8, 4:8], in_=ya[64:128])

            def emit_casts(b):
                """int8 bn -> bf16 for the stage-B moving operand."""
                bh = bnh_pool.tile([P, 8, 2, TCv], BF16, tag=f"bnh{b % 2}")
                nc.gpsimd.tensor_copy(bh[:, 0:3], bn[b][:, 0:3])
                nc.vector.tensor_copy(bh[:, 3:5], bn[b][:, 3:5])
                nc.gpsimd.tensor_copy(bh[:, 5:7], bn[b][:, 5:7])
                nc.scalar.copy(bh[:, 7:8], bn[b][:, 7:8])
                return bh

            def emit_B(b, bh):
                mb, ng = matb[b], negb[b]
                # ng[:,1] = -bi (see emit_A)
                nc.scalar.mul(ng[:, 1], mb[:, 3], -1.0)
                src = bh if bh is not None else bn[b]
                osb = None
                for t2 in range(8):
                    acc = ps_pool.tile([P, 2, TCv], F32, tag="ps")
                    br_ = mb[:, 2, t2, :]
                    bi_ = mb[:, 3, t2, :]
                    nbi = ng[:, 1, t2, :]
                    yre = src[:, t2, 0, :]
                    yim = src[:, t2, 1, :]
                    nc.tensor.matmul(acc[:, 0, :], br_, yre, start=True, stop=False)
                    nc.tensor.matmul(acc[:, 0, :], nbi, yim, start=False, stop=True)
                    nc.tensor.matmul(acc[:, 1, :], bi_, yre, start=True, stop=False)
                    nc.tensor.matmul(acc[:, 1, :], br_, yim, start=False, stop=True)
                    if t2 % 2 == 0:
                        osb = o_pool.tile([P, 2, 2, TCv], BF16, tag="osb")
                    if t2 % 2:
                        nc.scalar.copy(osb[:, 1], acc[:])
                    else:
                        nc.vector.tensor_copy(osb[:, 0], acc[:])
                    if t2 % 2 == 1:
                        # out_d rows (b*2+q)*P+j', free [t2' in 4, c, t]; this
                        # pair covers t2' in {t2-1, t2} of quad q = t2 // 4.
                        q, t2p = divmod(t2 - 1, 4)
                        row0 = (b * 2 + q) * P
                        dst = out_d[row0 : row0 + P, :].rearrange(
                            "p (tp c t) -> p tp c t", tp=4, c=2
                        )[:, t2p : t2p + 2]
                        eng2 = nc.sync if (t2 % 4 == 1) else nc.scalar
                        eng2.dma_start(out=dst, in_=osb[:])

            # Pipeline: B(b) emitted after A(b+1) so the b-shuffle has a full
            # A-phase of DMA time to land before the PE needs it.
            emit_A(0)
            for b in range(B):
                if b + 1 < B:
                    emit_A(b + 1)
                bh = emit_casts(b) if int8_shuffle else None
                emit_B(b, bh)

    nc.compile()
    return nc


def _build_program_v8():
    """v7 with 4-kind stationaries (ar, ai, br, bi — 4 MiB instead of 6).
    The negated operands move to the moving side: xs carries a third plane
    nxi = -xi (negated once at the head), and bn carries a third slot
    nyim = -y_im (negated after each shuffle lands, split across vector and
    scalar).  re = ar@xr + ai@nxi, im = ai@xr + ar@xi, and likewise for B."""
    nc = bacc.Bacc(
        "TRN2", target_bir_lowering=False, debug=False, num_devices=N_CORES,
        detect_race_conditions=False,
    )
    B = MESH_BATCH
    TCv = TOK5

    xt_d = nc.declare_dram_parameter("xt", [P, 8, 2, TCv], BF16, isOutput=False)
    # kinds: 0=ar 1=ai 2=br 3=bi
    mat_d = nc.declare_dram_parameter("mat", [P, B, 4, 8, P], BF16, isOutput=False)
    out_d = nc.declare_dram_parameter("out", [B * 2 * P, 4 * 2 * TCv], BF16,
                                      isOutput=True)

    with tile.TileContext(nc) as tc:
        with (
            tc.tile_pool(name="const", bufs=1) as const_pool,
            tc.tile_pool(name="mats", bufs=1) as mat_pool,
            tc.tile_pool(name="x", bufs=1) as x_pool,
            tc.tile_pool(name="ya", bufs=14) as ya_pool,
            tc.tile_pool(name="bin", bufs=1) as bin_pool,
            tc.tile_pool(name="osb", bufs=4) as o_pool,
            tc.tile_pool(name="ps", bufs=4, space=bass.MemorySpace.PSUM) as ps_pool,
        ):
            # --- all input DMAs first: nothing blocks the rings ---
            xs = x_pool.tile([P, 8, 2, TCv], BF16, tag="xs")
            for i in range(4):
                nc.sync.dma_start(out=xs[:, 2 * i : 2 * i + 2],
                                  in_=xt_d[:, 2 * i : 2 * i + 2])
            xn = x_pool.tile([P, 8, TCv], BF16, tag="xn")

            matA, matB = [], []
            for b in range(B):
                mA = mat_pool.tile([P, 2, 8, P], BF16, tag=f"matA{b}",
                                   name=f"matA{b}")
                mB = mat_pool.tile([P, 2, 8, P], BF16, tag=f"matB{b}",
                                   name=f"matB{b}")
                matA.append(mA)
                matB.append(mB)
            # balance the mats across the scalar and gpsimd rings, earliest
            # batches first, so each matX_b lands just ahead of its phase.
            for b in range(B):
                eng = nc.scalar if b < 2 else nc.gpsimd
                eng.dma_start(out=matA[b][:], in_=mat_d[:, b, 0:2])
                eng.dma_start(out=matB[b][:], in_=mat_d[:, b, 2:4])

            ident = const_pool.tile([P, P], F32)
            make_identity(nc, ident[:])

            warm = ps_pool.tile([P, 2, TCv], F32, tag="ps")
            for _ in range(5):
                for j in range(4):
                    nc.tensor.transpose(
                        warm[:, j // 2, (j % 2) * P : (j % 2 + 1) * P],
                        ident[:], ident[:],
                    )

            # nxi planes, negated per-r on vector while mats stream in.
            for r in range(8):
                nc.vector.tensor_scalar_mul(xn[:, r], xs[:, r, 1], -1.0)

            bn = [
                bin_pool.tile([P, 8, 2, TCv], BF16, tag=f"bin{b}", name=f"bin{b}")
                for b in range(B)
            ]
            # nyim planes (negated post-shuffle)
            bnn = [
                bin_pool.tile([P, 8, TCv], BF16, tag=f"binn{b}", name=f"binn{b}")
                for b in range(B)
            ]

            def emit_A(b):
                mA = matA[b]
                for r in range(8):
                    g = _rev(r, 3)
                    acc = ps_pool.tile([P, 2, TCv], F32, tag="ps")
                    ar_ = mA[:, 0, r, :]
                    ai_ = mA[:, 1, r, :]
                    xr_ = xs[:, r, 0, :]
                    xi_ = xs[:, r, 1, :]
                    nxi = xn[:, r, :]
                    nc.tensor.matmul(acc[:, 0, :], ar_, xr_, start=True, stop=False)
                    nc.tensor.matmul(acc[:, 0, :], ai_, nxi, start=False, stop=True)
                    nc.tensor.matmul(acc[:, 1, :], ai_, xr_, start=True, stop=False)
                    nc.tensor.matmul(acc[:, 1, :], ar_, xi_, start=False, stop=True)
                    ya = ya_pool.tile([P, 2, TCv], BF16, tag="ya")
                    if r % 2:
                        nc.scalar.copy(ya[:], acc[:])
                    else:
                        nc.vector.tensor_copy(ya[:], acc[:])
                    eng2 = nc.scalar if (g % 2) else nc.sync
                    eng2.dma_start(out=bn[b][g:P:8], in_=ya[:])

            def emit_negs(b):
                # nyim = -yim once the b-shuffle has landed; split across
                # engines so neither eats a full phase.
                nc.vector.tensor_scalar_mul(bnn[b][:, 0:4], bn[b][:, 0:4, 1], -1.0)
                nc.scalar.mul(bnn[b][:, 4:8], bn[b][:, 4:8, 1], -1.0)

            def emit_B(b):
                mB = matB[b]
                osb = None
                for t2 in range(8):
                    acc = ps_pool.tile([P, 2, TCv], F32, tag="ps")
                    br_ = mB[:, 0, t2, :]
                    bi_ = mB[:, 1, t2, :]
                    yre = bn[b][:, t2, 0, :]
                    yim = bn[b][:, t2, 1, :]
                    nyim = bnn[b][:, t2, :]
                    nc.tensor.matmul(acc[:, 0, :], br_, yre, start=True, stop=False)
                    nc.tensor.matmul(acc[:, 0, :], bi_, nyim, start=False, stop=True)
                    nc.tensor.matmul(acc[:, 1, :], bi_, yre, start=True, stop=False)
                    nc.tensor.matmul(acc[:, 1, :], br_, yim, start=False, stop=True)
                    if t2 % 2 == 0:
                        osb = o_pool.tile([P, 2, 2, TCv], BF16, tag="osb")
                    if t2 % 2:
                        nc.scalar.copy(osb[:, 1], acc[:])
                    else:
                        nc.vector.tensor_copy(osb[:, 0], acc[:])
                    if t2 % 2 == 1:
                        # out_d rows (b*2+q)*P+j', free [t2' in 4, c, t]; this
                        # pair covers t2' in {t2-1, t2} of quad q = t2 // 4.
                        q, t2p = divmod(t2 - 1, 4)
                        row0 = (b * 2 + q) * P
                        dst = out_d[row0 : row0 + P, :].rearrange(
                            "p (tp c t) -> p tp c t", tp=4, c=2
                        )[:, t2p : t2p + 2]
                        eng2 = nc.sync if (t2 % 4 == 1) else nc.scalar
                        eng2.dma_start(out=dst, in_=osb[:])

            # B(b) two phases after A(b): each shuffle gets ~2 phases of
            # port time before the PE needs it.
            emit_A(0)
            emit_A(1)
            emit_A(2)
            emit_negs(0)
            emit_B(0)
            emit_A(3)
            emit_negs(1)
            emit_B(1)
            emit_negs(2)
            emit_B(2)
            emit_negs(3)
            emit_B(3)

    nc.compile()
    return nc


def _build_program_v7():
    """v5 pipeline (unpaired shuffle, balanced A/B interleave) with the v6
    6-kind stationaries, plus head fixes: DMA triggers are emitted before
    make_identity (which otherwise blocks the sync/gpsimd FIFOs for ~1.5us),
    stationaries are split into A/B-kind halves with batch 0 on the scalar
    HWDGE ring so stage A can start as early as possible, and the PE warmup
    is sized to the head gap."""
    nc = bacc.Bacc(
        "TRN2", target_bir_lowering=False, debug=False, num_devices=N_CORES,
        detect_race_conditions=False,
    )
    B = MESH_BATCH
    TCv = TOK5

    xt_d = nc.declare_dram_parameter("xt", [P, 8, 2, TCv], BF16, isOutput=False)
    # kinds: 0=ar 1=ai 2=nai 3=br 4=bi 5=nbi
    mat_d = nc.declare_dram_parameter("mat", [P, B, 6, 8, P], BF16, isOutput=False)
    out_d = nc.declare_dram_parameter("out", [B * 2 * P, 4 * 2 * TCv], BF16,
                                      isOutput=True)

    with tile.TileContext(nc) as tc:
        with (
            tc.tile_pool(name="const", bufs=1) as const_pool,
            tc.tile_pool(name="mats", bufs=1) as mat_pool,
            tc.tile_pool(name="x", bufs=1) as x_pool,
            tc.tile_pool(name="ya", bufs=6) as ya_pool,
            tc.tile_pool(name="bin", bufs=1) as bin_pool,
            tc.tile_pool(name="osb", bufs=3) as o_pool,
            tc.tile_pool(name="ps", bufs=4, space=bass.MemorySpace.PSUM) as ps_pool,
        ):
            # --- all input DMAs first: nothing blocks the rings ---
            xs = x_pool.tile([P, 8, 2, TCv], BF16, tag="xs")
            nc.sync.dma_start(out=xs[:, 0:4], in_=xt_d[:, 0:4])
            nc.sync.dma_start(out=xs[:, 4:8], in_=xt_d[:, 4:8])

            matA, matB = [], []
            for b in range(B):
                mA = mat_pool.tile([P, 3, 8, P], BF16, tag=f"matA{b}",
                                   name=f"matA{b}")
                mB = mat_pool.tile([P, 3, 8, P], BF16, tag=f"matB{b}",
                                   name=f"matB{b}")
                matA.append(mA)
                matB.append(mB)
            # batch 0 on the (otherwise idle) scalar ring for earliest arrival;
            # the rest stream in FIFO order on the gpsimd SWDGE ring.
            nc.scalar.dma_start(out=matA[0][:], in_=mat_d[:, 0, 0:3])
            nc.scalar.dma_start(out=matB[0][:], in_=mat_d[:, 0, 3:6])
            for b in range(1, B):
                nc.gpsimd.dma_start(out=matA[b][:], in_=mat_d[:, b, 0:3])
                nc.gpsimd.dma_start(out=matB[b][:], in_=mat_d[:, b, 3:6])

            ident = const_pool.tile([P, P], F32)
            make_identity(nc, ident[:])

            # Short HAM warmup sized to the ~4us head gap.
            warm = ps_pool.tile([P, 2, TCv], F32, tag="ps")
            for _ in range(5):
                for j in range(4):
                    nc.tensor.transpose(
                        warm[:, j // 2, (j % 2) * P : (j % 2 + 1) * P],
                        ident[:], ident[:],
                    )

            bn = [
                bin_pool.tile([P, 8, 2, TCv], BF16, tag=f"bin{b}", name=f"bin{b}")
                for b in range(B)
            ]

            def emit_A(b):
                mA = matA[b]
                for r in range(8):
                    g = _rev(r, 3)
                    acc = ps_pool.tile([P, 2, TCv], F32, tag="ps")
                    ar_ = mA[:, 0, r, :]
                    ai_ = mA[:, 1, r, :]
                    nai = mA[:, 2, r, :]
                    xr_ = xs[:, r, 0, :]
                    xi_ = xs[:, r, 1, :]
                    nc.tensor.matmul(acc[:, 0, :], ar_, xr_, start=True, stop=False)
                    nc.tensor.matmul(acc[:, 0, :], nai, xi_, start=False, stop=True)
                    nc.tensor.matmul(acc[:, 1, :], ai_, xr_, start=True, stop=False)
                    nc.tensor.matmul(acc[:, 1, :], ar_, xi_, start=False, stop=True)
                    ya = ya_pool.tile([P, 2, TCv], BF16, tag="ya")
                    if r % 2:
                        nc.scalar.copy(ya[:], acc[:])
                    else:
                        nc.vector.tensor_copy(ya[:], acc[:])
                    eng2 = nc.scalar if (g % 2) else nc.sync
                    eng2.dma_start(out=bn[b][g:P:8], in_=ya[:])

            def emit_B(b):
                mB = matB[b]
                osb = None
                for t2 in range(8):
                    acc = ps_pool.tile([P, 2, TCv], F32, tag="ps")
                    br_ = mB[:, 0, t2, :]
                    bi_ = mB[:, 1, t2, :]
                    nbi = mB[:, 2, t2, :]
                    yre = bn[b][:, t2, 0, :]
                    yim = bn[b][:, t2, 1, :]
                    nc.tensor.matmul(acc[:, 0, :], br_, yre, start=True, stop=False)
                    nc.tensor.matmul(acc[:, 0, :], nbi, yim, start=False, stop=True)
                    nc.tensor.matmul(acc[:, 1, :], bi_, yre, start=True, stop=False)
                    nc.tensor.matmul(acc[:, 1, :], br_, yim, start=False, stop=True)
                    if t2 % 2 == 0:
                        osb = o_pool.tile([P, 2, 2, TCv], BF16, tag="osb")
                    if t2 % 2:
                        nc.scalar.copy(osb[:, 1], acc[:])
                    else:
                        nc.vector.tensor_copy(osb[:, 0], acc[:])
                    if t2 % 2 == 1:
                        # out_d rows (b*2+q)*P+j', free [t2' in 4, c, t]; this
                        # pair covers t2' in {t2-1, t2} of quad q = t2 // 4.
                        q, t2p = divmod(t2 - 1, 4)
                        row0 = (b * 2 + q) * P
                        dst = out_d[row0 : row0 + P, :].rearrange(
                            "p (tp c t) -> p tp c t", tp=4, c=2
                        )[:, t2p : t2p + 2]
                        eng2 = nc.sync if (t2 % 4 == 1) else nc.scalar
                        eng2.dma_start(out=dst, in_=osb[:])

            for step in ("A0", "A1", "B0", "A2", "B1", "A3", "B2", "B3"):
                (emit_A if step[0] == "A" else emit_B)(int(step[1]))

    nc.compile()
    return nc


def _build_program_v6():
    """v5 + (a) all 6 stationary kinds from HBM (no on-device negation: the
    1.9us scalar negates stalled the PSUM-evac path at each phase head) and
    (b) batch-paired shuffle: stage-A results for batches (2p, 2p+1) share one
    ya2 tile, so each partition-regroup DMA moves 4 KiB runs (halves the
    descriptor count of the port-bound shuffle)."""
    nc = bacc.Bacc(
        "TRN2", target_bir_lowering=False, debug=False, num_devices=N_CORES,
        detect_race_conditions=False,
    )
    B = MESH_BATCH
    TCv = TOK5

    xt_d = nc.declare_dram_parameter("xt", [P, 8, 2, TCv], BF16, isOutput=False)
    # kinds: 0=ar 1=ai 2=nai 3=br 4=bi 5=nbi
    mat_d = nc.declare_dram_parameter("mat", [P, B, 6, 8, P], BF16, isOutput=False)
    out_d = nc.declare_dram_parameter("out", [B * 2 * P, 4 * 2 * TCv], BF16,
                                      isOutput=True)

    with tile.TileContext(nc) as tc:
        with (
            tc.tile_pool(name="const", bufs=1) as const_pool,
            tc.tile_pool(name="mats", bufs=1) as mat_pool,
            tc.tile_pool(name="x", bufs=1) as x_pool,
            tc.tile_pool(name="ya", bufs=9) as ya_pool,
            tc.tile_pool(name="bin", bufs=1) as bin_pool,
            tc.tile_pool(name="osb", bufs=2) as o_pool,
            tc.tile_pool(name="ps", bufs=4, space=bass.MemorySpace.PSUM) as ps_pool,
        ):
            ident = const_pool.tile([P, P], F32)
            make_identity(nc, ident[:])

            warm = ps_pool.tile([P, 2, TCv], F32, tag="ps")
            for _ in range(9):
                for j in range(4):
                    nc.tensor.transpose(
                        warm[:, j // 2, (j % 2) * P : (j % 2 + 1) * P],
                        ident[:], ident[:],
                    )

            xs = x_pool.tile([P, 8, 2, TCv], BF16, tag="xs")
            nc.sync.dma_start(out=xs[:, 0:4], in_=xt_d[:, 0:4])
            nc.sync.dma_start(out=xs[:, 4:8], in_=xt_d[:, 4:8])

            matb = []
            for b in range(B):
                mb = mat_pool.tile([P, 6, 8, P], BF16, tag=f"mat{b}")
                nc.gpsimd.dma_start(out=mb[:], in_=mat_d[:, b])
                matb.append(mb)

            bn2 = [
                bin_pool.tile([P, 8, 2, 2, TCv], BF16, tag=f"bin{p}",
                              name=f"bin{p}")
                for p in range(2)
            ]
            ya2 = {}

            def emit_A(b):
                mb = matb[b]
                pair, half = divmod(b, 2)
                for r in range(8):
                    g = _rev(r, 3)
                    acc = ps_pool.tile([P, 2, TCv], F32, tag="ps")
                    ar_ = mb[:, 0, r, :]
                    ai_ = mb[:, 1, r, :]
                    nai = mb[:, 2, r, :]
                    xr_ = xs[:, r, 0, :]
                    xi_ = xs[:, r, 1, :]
                    nc.tensor.matmul(acc[:, 0, :], ar_, xr_, start=True, stop=False)
                    nc.tensor.matmul(acc[:, 0, :], nai, xi_, start=False, stop=True)
                    nc.tensor.matmul(acc[:, 1, :], ai_, xr_, start=True, stop=False)
                    nc.tensor.matmul(acc[:, 1, :], ar_, xi_, start=False, stop=True)
                    if half == 0:
                        ya2[pair, g] = ya_pool.tile(
                            [P, 2, 2, TCv], BF16, tag="ya", name=f"ya{pair}_{g}"
                        )
                    dst = ya2[pair, g][:, half]
                    if r % 2:
                        nc.scalar.copy(dst, acc[:])
                    else:
                        nc.vector.tensor_copy(dst, acc[:])
                    if half == 1:
                        eng2 = nc.scalar if (g % 2) else nc.sync
                        eng2.dma_start(
                            out=bn2[pair][g:P:8], in_=ya2[pair, g][:]
                        )

            def emit_B(b):
                mb = matb[b]
                pair, half = divmod(b, 2)
                osb = None
                for t2 in range(8):
                    acc = ps_pool.tile([P, 2, TCv], F32, tag="ps")
                    br_ = mb[:, 3, t2, :]
                    bi_ = mb[:, 4, t2, :]
                    nbi = mb[:, 5, t2, :]
                    yre = bn2[pair][:, t2, half, 0, :]
                    yim = bn2[pair][:, t2, half, 1, :]
                    nc.tensor.matmul(acc[:, 0, :], br_, yre, start=True, stop=False)
                    nc.tensor.matmul(acc[:, 0, :], nbi, yim, start=False, stop=True)
                    nc.tensor.matmul(acc[:, 1, :], bi_, yre, start=True, stop=False)
                    nc.tensor.matmul(acc[:, 1, :], br_, yim, start=False, stop=True)
                    if t2 % 2 == 0:
                        osb = o_pool.tile([P, 2, 2, TCv], BF16, tag="osb")
                    if t2 % 2:
                        nc.scalar.copy(osb[:, 1], acc[:])
                    else:
                        nc.vector.tensor_copy(osb[:, 0], acc[:])
                    if t2 % 2 == 1:
                        # out_d rows (b*2+q)*P+j', free [t2' in 4, c, t]; this
                        # pair covers t2' in {t2-1, t2} of quad q = t2 // 4.
                        q, t2p = divmod(t2 - 1, 4)
                        row0 = (b * 2 + q) * P
                        dst = out_d[row0 : row0 + P, :].rearrange(
                            "p (tp c t) -> p tp c t", tp=4, c=2
                        )[:, t2p : t2p + 2]
                        eng2 = nc.sync if (t2 % 4 == 1) else nc.scalar
                        eng2.dma_start(out=dst, in_=osb[:])

            for step in ("A0", "A1", "A2", "B0", "A3", "B1", "B2", "B3"):
                (emit_A if step[0] == "A" else emit_B)(int(step[1]))

    nc.compile()
    return nc


_CACHED = {}


def _host_prep_v4(x_re, x_im, phases):
    """Host-side: transposed/r-grouped bf16 x per (core-half), bf16 mats."""
    import ml_dtypes

    Astat, Bstat = _stage_matrices(phases)
    bf = ml_dtypes.bfloat16
    ar = Astat.real.reshape(MESH_BATCH, 8 * P, P).astype(bf)
    ai = Astat.imag.reshape(MESH_BATCH, 8 * P, P).astype(bf)
    br = Bstat.real.reshape(MESH_BATCH, 8 * P, P).astype(bf)
    bi = Bstat.imag.reshape(MESH_BATCH, 8 * P, P).astype(bf)

    half = N_TOKENS // 2
    xts = []
    for h in range(2):
        planes = []
        for xp in (x_re, x_im):
            # [T, L] -> [L, T] -> (p, r) rows -> [r, p, T]
            xT = xp[h * half : (h + 1) * half].T.reshape(P, 8, half)
            planes.append(xT.transpose(1, 0, 2))
        xt = np.concatenate(planes, axis=0).reshape(16 * P, half)
        xts.append(np.ascontiguousarray(xt).astype(bf))
    return ar, ai, br, bi, xts


_JCOLS = None


def _jcols():
    global _JCOLS
    if _JCOLS is None:
        idx = np.arange(P)
        v_, m_ = np.divmod(idx, 8)
        _JCOLS = [P * m_ + 8 * v_ + _rev(t2, 3) for t2 in range(8)]
    return _JCOLS


def _host_prep_v5(x_re, x_im, phases, int8_shuffle, six_kinds=False):
    """Pack stationaries [P, B, K, 8, P] bf16 (K=4: ar, ai, br, bi; K=6 adds
    nai, nbi; int8 scales folded) and per-core x panels [P, 8, 2, TOK5]."""
    import ml_dtypes

    bf = ml_dtypes.bfloat16
    _USPLIT[0] = True
    try:
        Astat, Bstat = _stage_matrices(phases)
    finally:
        _USPLIT[0] = False
    s = YSCALE if int8_shuffle else 1.0
    Astat = Astat * np.float32(s)
    Bstat = Bstat * np.float32(1.0 / s)
    if six_kinds:
        # kinds: ar, ai, nai, br, bi, nbi
        mat = np.empty((P, MESH_BATCH, 6, 8, P), dtype=bf)
        mat[:, :, 0] = Astat.real.astype(bf).transpose(2, 0, 1, 3)
        mat[:, :, 1] = Astat.imag.astype(bf).transpose(2, 0, 1, 3)
        mat[:, :, 2] = (-Astat.imag).astype(bf).transpose(2, 0, 1, 3)
        mat[:, :, 3] = Bstat.real.astype(bf).transpose(2, 0, 1, 3)
        mat[:, :, 4] = Bstat.imag.astype(bf).transpose(2, 0, 1, 3)
        mat[:, :, 5] = (-Bstat.imag).astype(bf).transpose(2, 0, 1, 3)
    else:
        # mat[p, b, kind, r, m]
        mat = np.empty((P, MESH_BATCH, 4, 8, P), dtype=bf)
        mat[:, :, 0] = Astat.real.astype(bf).transpose(2, 0, 1, 3)
        mat[:, :, 1] = Astat.imag.astype(bf).transpose(2, 0, 1, 3)
        mat[:, :, 2] = Bstat.real.astype(bf).transpose(2, 0, 1, 3)
        mat[:, :, 3] = Bstat.imag.astype(bf).transpose(2, 0, 1, 3)
    mat = np.ascontiguousarray(mat)

    xts = []
    for c in range(N_CORES):
        t0 = c * TOK5
        panes = []
        for xp in (x_re, x_im):
            # [TOK5, L] -> [L, TOK5] -> [P, 8, TOK5]  (L-index = 8p + r)
            panes.append(xp[t0 : t0 + TOK5].T.reshape(P, 8, TOK5))
        xt = np.stack(panes, axis=2)  # [P, 8, 2, TOK5]
        xts.append(np.ascontiguousarray(xt).astype(bf))
    return mat, xts


def kernel(x_re: np.ndarray, x_im: np.ndarray, phases: np.ndarray) -> np.ndarray:
    global LAST_RESULTS

    x_re = np.ascontiguousarray(x_re, dtype=np.float32)
    x_im = np.ascontiguousarray(x_im, dtype=np.float32)
    phases = np.ascontiguousarray(phases, dtype=np.float32)

    if VERSION in (5, 6, 7, 8):
        six = VERSION in (6, 7)
        mat, xts = _host_prep_v5(x_re, x_im, phases, INT8_SHUFFLE and not six,
                                 six_kinds=six)
        key = (VERSION, INT8_SHUFFLE and not six)
        if key not in _CACHED:
            _CACHED[key] = (
                _build_program_v8() if VERSION == 8
                else _build_program_v7() if VERSION == 7
                else _build_program_v6() if six
                else _build_program_v5(INT8_SHUFFLE)
            )
        nc = _CACHED[key]
        in_maps = [{"xt": xts[c], "mat": mat} for c in range(N_CORES)]
        res = run_bass_kernel_spmd(nc, in_maps, list(range(N_CORES)), trace=TRACE)
        LAST_RESULTS = res
        jcols = _jcols()
        out = np.empty((MESH_BATCH, N_TOKENS, L), dtype=np.complex64)
        for c in range(N_CORES):
            t0 = c * TOK5
            sl = slice(t0, t0 + TOK5)
            arr = np.asarray(res.results[c]["out"], dtype=np.float32).reshape(
                MESH_BATCH, 2, P, 4, 2, TOK5
            )
            for t2 in range(8):
                q, t2p = divmod(t2, 4)
                for b in range(MESH_BATCH):
                    cplx = (
                        arr[b, q, :, t2p, 0, :] + 1j * arr[b, q, :, t2p, 1, :]
                    ).astype(np.complex64)  # [P, TOK5]
                    out[b, sl, jcols[t2]] = cplx
        return out

    half = N_TOKENS // 2
    in_maps = []
    if VERSION == 4:
        ar, ai, br, bi, xts = _host_prep_v4(x_re, x_im, phases)
        if 4 not in _CACHED:
            _CACHED[4] = _build_program_v4()
        nc = _CACHED[4]
        packed = []
        for b in range(MESH_BATCH):
            kinds = [ar[b], ai[b], -ai[b], br[b], bi[b], -bi[b]]
            m = np.concatenate([k.reshape(8, P, P) for k in kinds], axis=0)
            packed.append(np.ascontiguousarray(m.transpose(1, 0, 2).reshape(P, 48 * P)))
        for c in range(N_CORES):
            b, h = c // 2, c % 2
            in_maps.append({"xt": xts[h], "mat": packed[b]})
        res = run_bass_kernel_spmd(nc, in_maps, list(range(N_CORES)), trace=TRACE)
        LAST_RESULTS = res
        jcols = _jcols()
        out = np.empty((MESH_BATCH, N_TOKENS, L), dtype=np.complex64)
        for c in range(N_CORES):
            b, h = c // 2, c % 2
            # [NCH, 8, P, 2, TC] bf16 -> upcast once
            arr = np.asarray(res.results[c]["out"], dtype=np.float32).reshape(
                NCH, 8, P, 2, TC
            )
            sl = slice(h * half, (h + 1) * half)
            for t2 in range(8):
                cplx = (arr[:, t2, :, 0, :] + 1j * arr[:, t2, :, 1, :]).astype(
                    np.complex64
                )  # [NCH, P, TC]
                out[b, sl, jcols[t2]] = cplx.transpose(1, 0, 2).reshape(P, half)
        return out
    if VERSION == 2:
        W = _build_W(phases)                  # (B, L, L) complex64
        Wr = np.ascontiguousarray(W.real, dtype=np.float32)
        Wi = np.ascontiguousarray(W.imag, dtype=np.float32)
        if 2 not in _CACHED:
            _CACHED[2] = _build_program()
        nc = _CACHED[2]
        for c in range(N_CORES):
            b, h = c // 2, c % 2
            in_maps.append(
                {
                    "xr": x_re[h * half : (h + 1) * half],
                    "xi": x_im[h * half : (h + 1) * half],
                    "wr": Wr[b],
                    "wi": Wi[b],
                }
            )
    else:
        import ml_dtypes

        Astat, Bstat = _stage_matrices(phases)
        ar = np.ascontiguousarray(Astat.real.reshape(MESH_BATCH, 8 * P, P))
        ai = np.ascontiguousarray(Astat.imag.reshape(MESH_BATCH, 8 * P, P))
        br = Bstat.real.reshape(MESH_BATCH, 8 * P, P).astype(ml_dtypes.bfloat16)
        bi = Bstat.imag.reshape(MESH_BATCH, 8 * P, P).astype(ml_dtypes.bfloat16)
        if 3 not in _CACHED:
            _CACHED[3] = _build_program_v3()
        nc = _CACHED[3]
        for c in range(N_CORES):
            b, h = c // 2, c % 2
            in_maps.append(
                {
                    "xr": x_re[h * half : (h + 1) * half],
                    "xi": x_im[h * half : (h + 1) * half],
                    "ar": ar[b],
                    "ai": ai[b],
                    "nai": np.ascontiguousarray(-ai[b]),
                    "br": br[b],
                    "bi": bi[b],
                    "nbi": np.ascontiguousarray(-bi[b]),
                }
            )

    res = run_bass_kernel_spmd(nc, in_maps, list(range(N_CORES)), trace=TRACE)
    LAST_RESULTS = res

    out = np.empty((MESH_BATCH, N_TOKENS, L), dtype=np.complex64)
    for c in range(N_CORES):
        b, h = c // 2, c % 2
        out[b, h * half : (h + 1) * half] = (
            res.results[c]["out"].view(np.complex64).reshape(half, L)
        )
    return out

